# revision 1
# baseline (speedup 1.0000x reference)
"""BivectorRotarySelfAttention TRN2 kernel.

Sharding: 8 cores = 4 batches x 2 head-halves. Each core computes one batch's
attention for 8 heads (2 kv heads) and a partial output projection; host sums
the two head-half partials per batch.

Per-core dataflow (transposed layouts: features in partitions, seq in free):
  xT    = recombine(dma_transpose(x_hi), dma_transpose(x_lo))      [f32r]
  qT/kT/vT = W-blocks.T @ xT   (PSUM-accumulated f32r matmuls)
  rope via PE permutation-matmul + 2 DVE muls + 1 add
  scores S^T[m,q]: 4 K=64 matmuls (S0,S1 / C0,C1 row-packed pairs)
  raw = S0*S1 + c'*C0*C1 ; E = exp(alpha*raw + key_mask_bias)  [bf16]
  causal: affine_select on diagonal blocks (GPSIMD)
  outT[d,q] = v-blocks.T @ E (bf16), rowsums via ones-matmul broadcast
  y[l,:] += (outT_h * recip_rowsum) @ Wo_h   (bf16)
"""
import sys
if '/opt/trn_rl_repo' not in sys.path:
    sys.path.insert(0, '/opt/trn_rl_repo')

import numpy as np
import ml_dtypes

import concourse.bass as bass
import concourse.mybir as mybir
import concourse.tile as tile
from concourse import bacc
from concourse.bass_utils import run_bass_kernel_spmd

F32 = mybir.dt.float32
F32R = mybir.dt.float32r
BF16 = mybir.dt.bfloat16

B, L, D, H, HKV = 4, 1024, 2048, 16, 4
HD = D // H            # 128
HD2 = HD // 2          # 64
NH = 8                 # heads per core
NKV = 2                # kv heads per core
NB = L // 128          # 8 blocks of 128
AluOp = mybir.AluOpType
Act = mybir.ActivationFunctionType

_CACHED = {}


def _chunks_for_stripe(mb):
    """Q-column chunks [(qs, qe)] covering [128*mb, 1024), split at 256-multiples."""
    q0 = 128 * mb
    out = []
    while q0 < L:
        qe = min(L, (q0 // 256 + 1) * 256)
        out.append((q0, qe))
        q0 = qe
    return out


def build_program():
    nc = bacc.Bacc("TRN2", target_bir_lowering=False, debug=False)

    # ---- dram params (per-core shapes) ----
    xh = nc.declare_dram_parameter("xh", [L, D], BF16, isOutput=False)
    xl = nc.declare_dram_parameter("xl", [L, D], BF16, isOutput=False)
    wq = nc.declare_dram_parameter("wq", [128, 16, NH * 128], F32R, isOutput=False)
    wk = nc.declare_dram_parameter("wk", [128, 16, NKV * 128], F32R, isOutput=False)
    wv = nc.declare_dram_parameter("wv", [128, 16, NKV * 128], F32R, isOutput=False)
    wo = nc.declare_dram_parameter("wo", [128, NH, D], BF16, isOutput=False)
    cosq = nc.declare_dram_parameter("cosq", [128, NH, L], F32, isOutput=False)
    sinq = nc.declare_dram_parameter("sinq", [128, NH, L], F32, isOutput=False)
    cosk = nc.declare_dram_parameter("cosk", [128, NKV, L], F32, isOutput=False)
    sink = nc.declare_dram_parameter("sink", [128, NKV, L], F32, isOutput=False)
    maskb = nc.declare_dram_parameter("maskb", [128, NB], F32, isOutput=False)
    cprime = nc.declare_dram_parameter("cprime", [128, NH], F32, isOutput=False)
    alpha = nc.declare_dram_parameter("alpha", [128, NH], F32, isOutput=False)
    pmrot = nc.declare_dram_parameter("pmrot", [128, 128], F32R, isOutput=False)
    pmswap = nc.declare_dram_parameter("pmswap", [128, 128], F32R, isOutput=False)
    onesb = nc.declare_dram_parameter("onesb", [128, 128], BF16, isOutput=False)
    identb = nc.declare_dram_parameter("identb", [128, 128], BF16, isOutput=False)
    y = nc.declare_dram_parameter("y", [L, D], F32, isOutput=True)

    with tile.TileContext(nc) as tc:
        with (
            tc.tile_pool(name="persist", bufs=1) as pp,
            tc.tile_pool(name="psum", bufs=1, space="PSUM") as psp,
        ):
            # persistent tiles
            consts = {}
            for nm, src, dt_ in [("pmrot", pmrot, F32R), ("pmswap", pmswap, F32R),
                                 ("onesb", onesb, BF16), ("identb", identb, BF16),
                                 ("maskb", maskb, F32), ("cprime", cprime, F32),
                                 ("alpha", alpha, F32)]:
                t = pp.tile(list(src.shape), dt_, tag=nm, name=nm)
                nc.sync.dma_start(t[:], src[:])
                consts[nm] = t

            xt = [pp.tile([128, L], F32R, tag=f"xt{ib}", name=f"xt{ib}")
                  for ib in range(16)]
            krt = [pp.tile([128, L], F32R, tag=f"krt{g}", name=f"krt{g}")
                   for g in range(NKV)]
            kswap = [pp.tile([128, L], F32R, tag=f"ksw{g}", name=f"ksw{g}")
                     for g in range(NKV)]
            vblk = [pp.tile([128, 128], BF16, tag=f"vb{i}", name=f"vb{i}")
                    for i in range(NKV * NB)]
            outtn = [pp.tile([128, L], BF16, tag=f"ot{h}", name=f"ot{h}")
                     for h in range(NH)]

            # ---------------- prologue: xT + k/v proj + k rope + v transpose
            with tc.tile_pool(name="pro", bufs=1) as ppro:
                # x transpose-load + recombine
                for ib in range(16):
                    th = ppro.tile([128, L], BF16, tag="xh_t", bufs=3)
                    tl = ppro.tile([128, L], BF16, tag="xl_t", bufs=3)
                    nc.sync.dma_start_transpose(th[:], xh[:, ib * 128:(ib + 1) * 128])
                    nc.sync.dma_start_transpose(tl[:], xl[:, ib * 128:(ib + 1) * 128])
                    nc.vector.tensor_add(xt[ib][:], th[:], tl[:])

                wk_t = ppro.tile([128, 16, NKV * 128], F32R, tag="wk")
                wv_t = ppro.tile([128, 16, NKV * 128], F32R, tag="wv")
                nc.sync.dma_start(wk_t[:], wk[:])
                nc.sync.dma_start(wv_t[:], wv[:])

                kt_s = []
                for g in range(NKV):
                    ps = psp.tile([128, L], F32, tag="pj", bufs=1)
                    for ib in range(16):
                        for c in range(2):
                            nc.tensor.matmul(
                                ps[:, c * 512:(c + 1) * 512],
                                wk_t[:, ib, g * 128:(g + 1) * 128],
                                xt[ib][:, c * 512:(c + 1) * 512],
                                start=(ib == 0), stop=(ib == 15))
                    kt = ppro.tile([128, L], F32R, tag="kt_s", bufs=2)
                    nc.any.tensor_copy(kt[:], ps[:])
                    kt_s.append(kt)

                # k rope
                for g in range(NKV):
                    psr = psp.tile([128, L], F32, tag="pj", bufs=1)
                    for c in range(2):
                        nc.tensor.matmul(psr[:, c * 512:(c + 1) * 512],
                                         consts["pmrot"][:],
                                         kt_s[g][:, c * 512:(c + 1) * 512])
                    t1 = ppro.tile([128, L], F32, tag="rtmp", bufs=4)
                    t2 = ppro.tile([128, L], F32, tag="rtmp", bufs=4)
                    csl = ppro.tile([128, L], F32, tag="ktab", bufs=4)
                    snl = ppro.tile([128, L], F32, tag="ktab", bufs=4)
                    nc.sync.dma_start(csl[:], cosk[:, g, :])
                    nc.sync.dma_start(snl[:], sink[:, g, :])
                    nc.vector.tensor_mul(t1[:], psr[:], snl[:])
                    nc.vector.tensor_mul(t2[:], kt_s[g][:].bitcast(F32), csl[:])
                    nc.vector.tensor_add(krt[g][:], t1[:], t2[:])
                    # kswap = partition-swap of krt
                    psw = psp.tile([128, L], F32, tag="pj", bufs=1)
                    for c in range(2):
                        nc.tensor.matmul(psw[:, c * 512:(c + 1) * 512],
                                         consts["pmswap"][:],
                                         krt[g][:, c * 512:(c + 1) * 512])
                    nc.any.tensor_copy(kswap[g][:], psw[:])

                # v proj (bf16 out) + transpose to [m, d] blocks
                for g in range(NKV):
                    ps = psp.tile([128, L], F32, tag="pj", bufs=1)
                    for ib in range(16):
                        for c in range(2):
                            nc.tensor.matmul(
                                ps[:, c * 512:(c + 1) * 512],
                                wv_t[:, ib, g * 128:(g + 1) * 128],
                                xt[ib][:, c * 512:(c + 1) * 512],
                                start=(ib == 0), stop=(ib == 15))
                    vt = ppro.tile([128, L], BF16, tag="vt_s", bufs=2)
                    nc.any.tensor_copy(vt[:], ps[:])
                    for mb in range(NB):
                        pv = psp.tile([128, 128], BF16, tag="pj", bufs=1)
                        nc.tensor.transpose(pv[:], vt[:, mb * 128:(mb + 1) * 128],
                                            consts["identb"][:])
                        nc.vector.tensor_copy(vblk[g * NB + mb][:], pv[:])

            # ---------------- head loop
            with tc.tile_pool(name="hl", bufs=1) as ph:
                for h in range(NH):
                    g = h // 4  # local kv head
                    wq_t = ph.tile([128, 16, 128], F32R, tag="wq_h", bufs=2)
                    nc.sync.dma_start(wq_t[:], wq[:, :, h * 128:(h + 1) * 128])
                    cq = ph.tile([128, L], F32, tag="tabq", bufs=2)
                    sq = ph.tile([128, L], F32, tag="tabq", bufs=2)
                    nc.sync.dma_start(cq[:], cosq[:, h, :])
                    nc.sync.dma_start(sq[:], sinq[:, h, :])

                    psq = psp.tile([128, L], F32, tag="pj", bufs=1)
                    for ib in range(16):
                        for c in range(2):
                            nc.tensor.matmul(
                                psq[:, c * 512:(c + 1) * 512],
                                wq_t[:, ib, :],
                                xt[ib][:, c * 512:(c + 1) * 512],
                                start=(ib == 0), stop=(ib == 15))
                    qt_s = ph.tile([128, L], F32R, tag="qt_s", bufs=2)
                    nc.any.tensor_copy(qt_s[:], psq[:])

                    psr = psp.tile([128, L], F32, tag="pj", bufs=1)
                    for c in range(2):
                        nc.tensor.matmul(psr[:, c * 512:(c + 1) * 512],
                                         consts["pmrot"][:],
                                         qt_s[:, c * 512:(c + 1) * 512])
                    t1 = ph.tile([128, L], F32, tag="qtmp", bufs=2)
                    t2 = ph.tile([128, L], F32, tag="qtmp", bufs=2)
                    nc.vector.tensor_mul(t1[:], psr[:], sq[:])
                    nc.vector.tensor_mul(t2[:], qt_s[:].bitcast(F32), cq[:])
                    qrt = ph.tile([128, L], F32R, tag="qrt", bufs=2)
                    nc.vector.tensor_add(qrt[:], t1[:], t2[:])

                    # scores -> E tiles
                    etiles = []
                    for mb in range(NB):
                        w = L - 128 * mb
                        et = ph.tile([128, w], BF16, tag=f"esc{mb}", bufs=3,
                                     name=f"esc_h{mb}")
                        etiles.append(et)
                    for mb in range(NB):
                        kb = slice(mb * 128, (mb + 1) * 128)
                        for (qs, qe) in _chunks_for_stripe(mb):
                            s = qe - qs
                            psA = psp.tile([128, 2 * s], F32, tag="scA", bufs=1,
                                           name="psA")
                            psB = psp.tile([128, 2 * s], F32, tag="scB", bufs=1,
                                           name="psB")
                            nc.tensor.matmul(psA[:, 0:s], krt[g][0:64, kb],
                                             qrt[0:64, qs:qe])
                            nc.tensor.matmul(psA[:, s:2 * s], kswap[g][0:64, kb],
                                             qrt[0:64, qs:qe])
                            nc.tensor.matmul(psB[:, 0:s], krt[g][64:128, kb],
                                             qrt[64:128, qs:qe])
                            nc.tensor.matmul(psB[:, s:2 * s], kswap[g][64:128, kb],
                                             qrt[64:128, qs:qe])
                            bs = ph.tile([128, 2 * s], F32, tag="bs", bufs=3)
                            nc.any.tensor_copy(bs[:], psB[:])
                            tp = ph.tile([128, 2 * s], F32, tag="tprod", bufs=3)
                            nc.vector.tensor_mul(tp[:], psA[:], bs[:])
                            raw = ph.tile([128, s], F32, tag="raw", bufs=3)
                            nc.vector.scalar_tensor_tensor(
                                raw[:], tp[:, s:2 * s], consts["cprime"][:, h:h + 1],
                                tp[:, 0:s], op0=AluOp.mult, op1=AluOp.add)
                            esl = etiles[mb][:, qs - 128 * mb: qe - 128 * mb]
                            nc.scalar.activation(esl, raw[:], Act.Exp,
                                                 bias=consts["maskb"][:, mb:mb + 1],
                                                 scale=consts["alpha"][:, h:h + 1])
                            if qs == 128 * mb:
                                # causal triangle on the diagonal 128 cols
                                nc.gpsimd.affine_select(
                                    etiles[mb][:, 0:128], etiles[mb][:, 0:128],
                                    pattern=[[1, 128]], compare_op=AluOp.is_ge,
                                    fill=0.0, base=0, channel_multiplier=-1)

                    # attnv + rowsum
                    ps_o = psp.tile([128, L], F32, tag="acco", bufs=1, name="ps_o")
                    ps_rs = psp.tile([128, L], F32, tag="accr", bufs=1, name="ps_rs")
                    for c in range(2):
                        mbs = [mb for mb in range(NB) if 128 * mb < 512 * (c + 1)]
                        for i, mb in enumerate(mbs):
                            os_ = max(512 * c, 128 * mb)
                            oe = 512 * (c + 1)
                            esl = etiles[mb][:, os_ - 128 * mb: oe - 128 * mb]
                            st, sp = (i == 0), (i == len(mbs) - 1)
                            nc.tensor.matmul(ps_o[:, os_:oe], vblk[g * NB + mb][:],
                                             esl, start=st, stop=sp)
                            nc.tensor.matmul(ps_rs[:, os_:oe], consts["onesb"][:],
                                             esl, start=st, stop=sp)
                    rcp = ph.tile([128, L], F32, tag="rcp", bufs=1)
                    nc.vector.reciprocal_approx_fast(rcp[:], ps_rs[:])
                    nc.vector.tensor_mul(outtn[h][:], ps_o[:], rcp[:])

            # ---------------- epilogue: Wo projection
            with tc.tile_pool(name="ep", bufs=1) as pe:
                wo_t = []
                for hb in range(NH):
                    t = pe.tile([128, D], BF16, tag=f"wo{hb}", name=f"wo{hb}")
                    nc.sync.dma_start(t[:], wo[:, hb, :])
                    wo_t.append(t)
                for lb in range(NB):
                    for c in range(2):
                        psy = psp.tile([128, 1024], F32, tag="pj", bufs=1, name="psy")
                        for cc in range(2):
                            for hh in range(NH):
                                nc.tensor.matmul(
                                    psy[:, cc * 512:(cc + 1) * 512],
                                    outtn[hh][:, lb * 128:(lb + 1) * 128],
                                    wo_t[hh][:, c * 1024 + cc * 512:
                                            c * 1024 + (cc + 1) * 512],
                                    start=(hh == 0), stop=(hh == NH - 1))
                        yt = pe.tile([128, 1024], F32, tag="ytile", bufs=3)
                        nc.any.tensor_copy(yt[:], psy[:])
                        nc.sync.dma_start(
                            y[lb * 128:(lb + 1) * 128, c * 1024:(c + 1) * 1024], yt[:])

    nc.compile()
    return nc


def _host_prep(x, Wq, Wk, Wv, Wo, q_param, log_scale, cos, sin, mask):
    """Build the 8 per-core input maps."""
    x = np.asarray(x, np.float32)
    Wq = np.asarray(Wq, np.float32)
    Wk = np.asarray(Wk, np.float32)
    Wv = np.asarray(Wv, np.float32)
    Wo = np.asarray(Wo, np.float32)
    cos = np.asarray(cos, np.float32)[0]      # [L, H, 64]
    sin = np.asarray(sin, np.float32)[0]
    qp = np.asarray(q_param, np.float32).reshape(H)
    ls = np.asarray(log_scale, np.float32).reshape(H)
    mask = np.asarray(mask)

    p64 = np.arange(128) % 64

    PM = np.zeros((128, 128), np.float32)
    for dp in range(128):
        base, r = (dp // 64) * 64, dp % 64
        if r < 32:
            PM[base + r + 32, dp] = -1.0
        else:
            PM[base + r - 32, dp] = 1.0
    SW = np.zeros((128, 128), np.float32)
    for dp in range(128):
        SW[(dp + 64) % 128, dp] = 1.0
    ONES = np.ones((128, 128), ml_dtypes.bfloat16)
    IDENT = np.eye(128, dtype=ml_dtypes.bfloat16)

    in_maps = []
    for core in range(8):
        b, g2 = core // 2, core % 2
        heads = list(range(g2 * NH, (g2 + 1) * NH))
        kvs = list(range(g2 * NKV, (g2 + 1) * NKV))

        xb = x[b]
        xh = xb.astype(ml_dtypes.bfloat16)
        xlo = (xb - xh.astype(np.float32)).astype(ml_dtypes.bfloat16)

        wq_c = Wq[:, g2 * NH * 128:(g2 + 1) * NH * 128]
        wk_c = Wk[:, g2 * NKV * 128:(g2 + 1) * NKV * 128]
        wv_c = Wv[:, g2 * NKV * 128:(g2 + 1) * NKV * 128]
        wo_c = Wo[g2 * NH * 128:(g2 + 1) * NH * 128, :]

        wq_p = wq_c.reshape(16, 128, NH * 128).transpose(1, 0, 2).copy()
        wk_p = wk_c.reshape(16, 128, NKV * 128).transpose(1, 0, 2).copy()
        wv_p = wv_c.reshape(16, 128, NKV * 128).transpose(1, 0, 2).copy()
        wo_p = wo_c.reshape(NH, 128, D).transpose(1, 0, 2).astype(ml_dtypes.bfloat16)

        cosq_p = np.ascontiguousarray(cos[:, heads, :][:, :, p64].transpose(2, 1, 0))
        sinq_p = np.ascontiguousarray(sin[:, heads, :][:, :, p64].transpose(2, 1, 0))
        cosk_p = np.ascontiguousarray(cos[:, kvs, :][:, :, p64].transpose(2, 1, 0))
        sink_p = np.ascontiguousarray(sin[:, kvs, :][:, :, p64].transpose(2, 1, 0))

        mb = np.where(mask[b].reshape(NB, 128).T.astype(bool), 0.0, -1e9)
        mb = mb.astype(np.float32)

        cpr = np.tile((-2.0 * np.tanh(qp[heads]))[None, :], (128, 1))
        alp = np.tile((np.exp(ls[heads]) / HD)[None, :], (128, 1))

        in_maps.append({
            "xh": xh, "xl": xlo,
            "wq": wq_p.astype(np.float32), "wk": wk_p.astype(np.float32),
            "wv": wv_p.astype(np.float32), "wo": wo_p,
            "cosq": cosq_p, "sinq": sinq_p, "cosk": cosk_p, "sink": sink_p,
            "maskb": mb, "cprime": cpr.astype(np.float32),
            "alpha": alp.astype(np.float32),
            "pmrot": PM, "pmswap": SW, "onesb": ONES, "identb": IDENT,
        })
    return in_maps


def kernel(**inputs):
    if "nc" not in _CACHED:
        _CACHED["nc"] = build_program()
    nc = _CACHED["nc"]
    in_maps = _host_prep(**inputs)
    res = run_bass_kernel_spmd(nc, in_maps, list(range(8))).results
    out = np.empty((B, L, D), np.float32)
    for b in range(B):
        out[b] = res[2 * b]["y"] + res[2 * b + 1]["y"]
    return out



# revision 2
# speedup vs baseline: 1.0268x; 1.0268x over previous
"""BivectorRotarySelfAttention TRN2 kernel, v3.

Sharding: 8 cores = 4 batches x 2 head-halves; host sums the two head-half
partial y's per batch.

v3 vs v2:
 - Score PSUM rings are 3-deep (SA/SB [128,512]x3) so the PE runs ~3 chunks
   ahead of the vector chain; all other PSUM users are [128,512] halves in a
   shared 2-deep ring (P2). 6+6+4 KB = 16 KB exactly.
 - Engine rebalance: rope t2/add and all raw-combines on Pool, 2 of 20 bs
   copies on DVE, rest on ACT.
 - Qproj+rope for head h+1 runs mid-scores(h); attnv halves are interleaved
   into the chunk stream so PE never waits on the exp chain.
 - V projection trails K by 4 ib-steps to match wv DMA arrival; wk DMA is
   issued before everything except nothing (first), consts after wv.
 - Epilogue prefetches wo and accumulates head 7 last.
"""
import sys
if '/opt/trn_rl_repo' not in sys.path:
    sys.path.insert(0, '/opt/trn_rl_repo')

import numpy as np
import ml_dtypes

import concourse.bass as bass
import concourse.mybir as mybir
import concourse.tile as tile
from concourse import bacc
from concourse.bass_utils import run_bass_kernel_spmd

F32 = mybir.dt.float32
F32R = mybir.dt.float32r
BF16 = mybir.dt.bfloat16

B, L, D, H, HKV = 4, 1024, 2048, 16, 4
HD = D // H            # 128
HD2 = HD // 2          # 64
NH = 8                 # heads per core
NKV = 2                # kv heads per core
NB = L // 128          # 8 blocks of 128
AluOp = mybir.AluOpType
Act = mybir.ActivationFunctionType

_CACHED = {}


def _stripe_qlo(mb):
    # stripe mb covers q in [qlo, L) in 256-wide chunks; odd stripes start one
    # 128-block early (the extra region is causal-masked to zero).
    return 128 * (mb - (mb % 2))


def build_program():
    nc = bacc.Bacc("TRN2", target_bir_lowering=False, debug=False)

    # ---- dram params (per-core shapes) ----
    xh = nc.declare_dram_parameter("xh", [L, D], BF16, isOutput=False)
    wq = nc.declare_dram_parameter("wq", [128, 16, NH * 128], BF16, isOutput=False)
    wk = nc.declare_dram_parameter("wk", [128, 16, NKV * 128], BF16, isOutput=False)
    wv = nc.declare_dram_parameter("wv", [128, 16, NKV * 128], BF16, isOutput=False)
    wo = nc.declare_dram_parameter("wo", [128, NH, D], BF16, isOutput=False)
    costab = nc.declare_dram_parameter("costab", [128, L], F32, isOutput=False)
    sintab = nc.declare_dram_parameter("sintab", [128, L], F32, isOutput=False)
    maskb = nc.declare_dram_parameter("maskb", [128, NB], F32, isOutput=False)
    sqrtc = nc.declare_dram_parameter("sqrtc", [128, NH], F32, isOutput=False)
    alpha = nc.declare_dram_parameter("alpha", [128, NH], F32, isOutput=False)
    pmrot = nc.declare_dram_parameter("pmrot", [128, 128], F32R, isOutput=False)
    pmswap = nc.declare_dram_parameter("pmswap", [128, 128], F32R, isOutput=False)
    onesb = nc.declare_dram_parameter("onesb", [128, 128], BF16, isOutput=False)
    identb = nc.declare_dram_parameter("identb", [128, 128], BF16, isOutput=False)
    y = nc.declare_dram_parameter("y", [L, D], F32, isOutput=True)

    with tile.TileContext(nc) as tc:
        with (
            tc.tile_pool(name="persist", bufs=1) as pp,
            tc.tile_pool(name="psum", bufs=1, space="PSUM") as psp,
        ):
            # ---- persistent SBUF ----
            xt = [pp.tile([128, L], BF16, tag=f"xt{ib}", name=f"xt{ib}")
                  for ib in range(16)]
            krt = [pp.tile([128, L], F32R, tag=f"krt{g}", name=f"krt{g}")
                   for g in range(NKV)]
            kswap = [pp.tile([128, L], F32R, tag=f"ksw{g}", name=f"ksw{g}")
                     for g in range(NKV)]
            vblk = [pp.tile([128, 128], BF16, tag=f"vb{i}", name=f"vb{i}")
                    for i in range(NKV * NB)]
            outtn = [pp.tile([128, L], BF16, tag=f"ot{h}", name=f"ot{h}")
                     for h in range(NH)]
            wo_c0 = pp.tile([128, NH, 512], BF16, tag="wo_c0", name="wo_c0")
            consts = {}

            def sa(name):
                return psp.tile([128, 512], F32, tag="SA", bufs=3, name=name)

            def sb(name):
                return psp.tile([128, 512], F32, tag="SB", bufs=3, name=name)

            def p2(name, shape=None, dtype=F32):
                return psp.tile(shape or [128, 512], dtype, tag="P2", bufs=2,
                                name=name)

            # ---------------- prologue ----------------
            with tc.tile_pool(name="pro", bufs=1) as ppro:
                wk_t = ppro.tile([128, 16, NKV * 128], BF16, tag="wk")
                wv_t = ppro.tile([128, 16, NKV * 128], BF16, tag="wv")

                # DMA order: wk, 2 x-block pairs, wv, 2 pairs, consts, rest.
                nc.sync.dma_start(wk_t[:], wk[:])
                def xdma(ib):
                    nc.sync.dma_start_transpose(
                        xt[ib][:], xh[:, ib * 128:(ib + 1) * 128])

                wq_t0 = ppro.tile([128, 16, 128], BF16, tag="wq0", name="wq_t0")
                xdma(0)
                xdma(1)
                nc.sync.dma_start(wv_t[:], wv[:])
                for ib in range(2, 16):
                    xdma(ib)
                nc.sync.dma_start(wq_t0[:], wq[:, :, 0:128])
                for nm, src, dt_ in [("pmrot", pmrot, F32R),
                                     ("costab", costab, F32),
                                     ("sintab", sintab, F32),
                                     ("identb", identb, BF16),
                                     ("pmswap", pmswap, F32R),
                                     ("onesb", onesb, BF16),
                                     ("maskb", maskb, F32),
                                     ("sqrtc", sqrtc, F32),
                                     ("alpha", alpha, F32)]:
                    t = pp.tile(list(src.shape), dt_, tag=nm, name=nm)
                    nc.sync.dma_start(t[:], src[:])
                    consts[nm] = t

                # K/V projection accumulators: [128,512] halves.
                psk = [[sa("psk0a"), sa("psk0b")], [sb("psk1a"), sb("psk1b")]]
                psv = [[sa("psv0a"), sb("psv0b")], [p2("psv1a"), p2("psv1b")]]

                VOFF = 2
                for step in range(16 + VOFF):
                    if step < 16:
                        ib = step
                        for g in range(NKV):
                            for c in range(2):
                                nc.tensor.matmul(
                                    psk[g][c][:],
                                    wk_t[:, ib, g * 128:(g + 1) * 128],
                                    xt[ib][:, c * 512:(c + 1) * 512],
                                    start=(ib == 0), stop=(ib == 15))
                    if step >= VOFF:
                        ib = step - VOFF
                        for g in range(NKV):
                            for c in range(2):
                                nc.tensor.matmul(
                                    psv[g][c][:],
                                    wv_t[:, ib, g * 128:(g + 1) * 128],
                                    xt[ib][:, c * 512:(c + 1) * 512],
                                    start=(ib == 0), stop=(ib == 15))

                # PSUM->SBUF copies: v-g1 first (frees the P2 slots that
                # Qproj(0) needs), then interleaved kt/vt.
                vt_s = [ppro.tile([128, L], BF16, tag=f"vt_s{g}", name=f"vt{g}")
                        for g in range(NKV)]
                kt_s = [ppro.tile([128, L], F32R, tag=f"kt_s{g}", name=f"kt{g}")
                        for g in range(NKV)]
                for c in range(2):
                    nc.scalar.copy(vt_s[1][:, c * 512:(c + 1) * 512], psv[1][c][:])
                for g in range(NKV):
                    for c in range(2):
                        nc.scalar.copy(kt_s[g][:, c * 512:(c + 1) * 512],
                                       psk[g][c][:])
                for c in range(2):
                    nc.scalar.copy(vt_s[0][:, c * 512:(c + 1) * 512], psv[0][c][:])

                # Qproj(0): no dependency on the copies above except P2 slots;
                # covers the kt/vt copy chain on PE.
                qt_s0 = ppro.tile([128, L], F32R, tag="qt_s0", name="qt_s0")
                for cc in range(2):
                    psq = p2("psq0")
                    for ib in range(16):
                        nc.tensor.matmul(psq[:], wq_t0[:, ib, :],
                                         xt[ib][:, cc * 512:(cc + 1) * 512],
                                         start=(ib == 0), stop=(ib == 15))
                    nc.vector.tensor_copy(qt_s0[:, cc * 512:(cc + 1) * 512],
                                          psq[:])

                # k rope rotate-half part (SA slots freed by the kt copies)
                for g in range(NKV):
                    t1 = ppro.tile([128, L], F32, tag="rtmp", bufs=4)
                    t2 = ppro.tile([128, L], F32, tag="rtmp", bufs=4)
                    for c in range(2):
                        psr = sa(f"psr_k{c}")
                        nc.tensor.matmul(psr[:],
                                         consts["pmrot"][:],
                                         kt_s[g][:, c * 512:(c + 1) * 512])
                        nc.vector.tensor_mul(
                            t1[:, c * 512:(c + 1) * 512], psr[:],
                            consts["sintab"][:, c * 512:(c + 1) * 512])
                    nc.gpsimd.tensor_mul(t2[:], kt_s[g][:].bitcast(F32),
                                         consts["costab"][:])
                    nc.vector.tensor_add(krt[g][:], t1[:], t2[:])

                # v transpose to [m, d] blocks
                for g in range(NKV):
                    for mb in range(NB):
                        pv = p2("pv", shape=[128, 128], dtype=BF16)
                        nc.tensor.transpose(pv[:], vt_s[g][:, mb * 128:(mb + 1) * 128],
                                            consts["identb"][:])
                        nc.vector.tensor_copy(vblk[g * NB + mb][:], pv[:])

                # rope-q(0): rotate part (qt_s0 ready after its DVE copies)
                t1q = ppro.tile([128, L], F32, tag="rtmp", bufs=4, name="t1q0")
                t2q = ppro.tile([128, L], F32, tag="rtmp", bufs=4, name="t2q0")
                for cc in range(2):
                    psr2 = p2("psr_q0")
                    nc.tensor.matmul(psr2[:], consts["pmrot"][:],
                                     qt_s0[:, cc * 512:(cc + 1) * 512])
                    nc.vector.tensor_mul(
                        t1q[:, cc * 512:(cc + 1) * 512], psr2[:],
                        consts["sintab"][:, cc * 512:(cc + 1) * 512])
                nc.gpsimd.tensor_mul(t2q[:], qt_s0[:].bitcast(F32),
                                     consts["costab"][:])
                qrt0 = pp.tile([128, L], F32R, tag="qrt0", name="qrt0")
                nc.gpsimd.tensor_add(qrt0[:], t1q[:], t2q[:])
                qc0 = pp.tile([128, L], F32R, tag="qc0", name="qc0")
                nc.vector.tensor_scalar_mul(qc0[:], qrt0[:].bitcast(F32),
                                            consts["sqrtc"][:, 0:1])

                # k swap perms (krt chains complete under the ops above)
                for g in range(NKV):
                    for c in range(2):
                        psw = sb(f"psw_k{g}{c}")
                        nc.tensor.matmul(psw[:],
                                         consts["pmswap"][:],
                                         krt[g][:, c * 512:(c + 1) * 512])
                        nc.scalar.copy(kswap[g][:, c * 512:(c + 1) * 512],
                                       psw[:])

            # ---------------- head loop ----------------
            with tc.tile_pool(name="hl", bufs=1) as ph:
                etiles = {}

                wq_tiles = {}

                def wq_dma(h):
                    if h < NH:
                        wq_t = ph.tile([128, 16, 128], BF16, tag="wq_h", bufs=2,
                                       name=f"wq_t{h}")
                        nc.sync.dma_start(wq_t[:], wq[:, :, h * 128:(h + 1) * 128])
                        wq_tiles[h] = wq_t

                def qproj(h):
                    wq_t = wq_tiles.pop(h)
                    qt_s = ph.tile([128, L], F32R, tag="qt_s", bufs=2)
                    for cc in range(2):
                        psq = p2("psq")
                        for ib in range(16):
                            nc.tensor.matmul(
                                psq[:],
                                wq_t[:, ib, :],
                                xt[ib][:, cc * 512:(cc + 1) * 512],
                                start=(ib == 0), stop=(ib == 15))
                        nc.vector.tensor_copy(qt_s[:, cc * 512:(cc + 1) * 512],
                                              psq[:])
                    return qt_s

                def rope_q_half(h, qt_s, qrt, qc, cc):
                    """one 512-half of q-rope + c'-scaled copy, on DVE"""
                    hs = slice(cc * 512, (cc + 1) * 512)
                    t1 = ph.tile([128, 512], F32, tag="qtmp", bufs=2)
                    t2 = ph.tile([128, 512], F32, tag="qtmp", bufs=2)
                    psr2 = p2("psr_q")
                    nc.tensor.matmul(psr2[:], consts["pmrot"][:], qt_s[:, hs])
                    nc.vector.tensor_mul(t1[:], psr2[:], consts["sintab"][:, hs])
                    nc.vector.tensor_mul(t2[:], qt_s[:, hs].bitcast(F32),
                                         consts["costab"][:, hs])
                    nc.vector.tensor_add(qrt[:, hs], t1[:], t2[:])
                    nc.vector.tensor_scalar_mul(qc[:, hs], qrt[:, hs].bitcast(F32),
                                                consts["sqrtc"][:, h:h + 1])

                def make_etiles(h):
                    ets = []
                    for mb in range(NB):
                        qlo = _stripe_qlo(mb)
                        et = ph.tile([128, L - qlo], BF16, tag=f"esc{mb}", bufs=2,
                                     name=f"esc_h{mb}")
                        ets.append(et)
                    etiles[h] = ets

                def score_group(h, qrt, qc, mb, qs, npair):
                    """npair chunks (1 or 2) of stripe mb starting at qs; one
                    fused exp over the pair."""
                    g = h // 4
                    qlo = _stripe_qlo(mb)
                    kb = slice(mb * 128, (mb + 1) * 128)
                    ets = etiles[h]
                    raw = ph.tile([128, 512], F32, tag="raw", bufs=3)
                    for j in range(npair):
                        cqs = qs + 256 * j
                        cqe = cqs + 256
                        psA = sa("psA")
                        psB = sb("psB")
                        nc.tensor.matmul(psA[:, 0:256], krt[g][0:64, kb],
                                         qrt[0:64, cqs:cqe])
                        nc.tensor.matmul(psA[:, 256:512], kswap[g][0:64, kb],
                                         qc[0:64, cqs:cqe])
                        nc.tensor.matmul(psB[:, 0:256], krt[g][64:128, kb],
                                         qrt[64:128, cqs:cqe])
                        nc.tensor.matmul(psB[:, 256:512], kswap[g][64:128, kb],
                                         qc[64:128, cqs:cqe])
                        bs = ph.tile([128, 512], F32, tag="bs", bufs=3)
                        nc.scalar.copy(bs[:], psB[:])
                        tp = ph.tile([128, 512], F32, tag="tprod", bufs=3)
                        nc.vector.tensor_mul(tp[:], psA[:], bs[:])
                        nc.gpsimd.tensor_add(raw[:, 256 * j:256 * (j + 1)],
                                             tp[:, 0:256], tp[:, 256:512])
                    w = 256 * npair
                    esl = ets[mb][:, qs - qlo: qs - qlo + w]
                    nc.scalar.activation(esl, raw[:, 0:w], Act.Exp,
                                         bias=consts["maskb"][:, mb:mb + 1],
                                         scale=consts["alpha"][:, h:h + 1])
                    if qs == qlo:
                        # causal mask on the diagonal 256 cols:
                        # keep where (qlo + col) - (128*mb + part) >= 0
                        nc.gpsimd.affine_select(
                            ets[mb][:, 0:256], ets[mb][:, 0:256],
                            pattern=[[1, 256]], compare_op=AluOp.is_ge,
                            fill=0.0, base=qlo - 128 * mb,
                            channel_multiplier=-1)

                def attnv_half(h, c, use_sasb=False):
                    g = h // 4
                    ets = etiles[h]
                    if use_sasb:
                        ps_o, ps_rs = sa("ps_o"), sb("ps_rs")
                    else:
                        ps_o, ps_rs = p2("ps_o"), p2("ps_rs")
                    mbs = [mb for mb in range(NB) if 128 * mb < 512 * (c + 1)]
                    for i, mb in enumerate(mbs):
                        qlo = _stripe_qlo(mb)
                        os_ = max(512 * c, 128 * mb)
                        oe = 512 * (c + 1)
                        esl = ets[mb][:, os_ - qlo: oe - qlo]
                        st, sp = (i == 0), (i == len(mbs) - 1)
                        nc.tensor.matmul(ps_o[:, os_ - 512 * c: oe - 512 * c],
                                         vblk[g * NB + mb][:], esl,
                                         start=st, stop=sp)
                        nc.tensor.matmul(ps_rs[:, os_ - 512 * c: oe - 512 * c],
                                         consts["onesb"][:], esl,
                                         start=st, stop=sp)
                    rcp = ph.tile([128, 512], F32, tag="rcp", bufs=2)
                    nc.vector.reciprocal_approx_fast(rcp[:], ps_rs[:])
                    nc.vector.tensor_mul(outtn[h][:, c * 512:(c + 1) * 512],
                                         ps_o[:], rcp[:])

                # chunk groups (mb, qs, npair): a = groups with qs < 512 of
                # stripes 0-3 (cover attnv c=0), b = the rest
                a_set = [(0, 0, 2), (1, 0, 2), (2, 256, 1), (3, 256, 1)]
                b_set = [(0, 512, 2), (1, 512, 2), (2, 512, 2), (3, 512, 2),
                         (4, 512, 2), (5, 512, 2), (6, 768, 1), (7, 768, 1)]

                qrts = {0: (qrt0, qc0)}
                wq_dma(1)
                for h in range(NH):
                    make_etiles(h)
                    qrt, qc = qrts[h]
                    wq_dma(h + 2)
                    # Qproj(h+1) first: no dependency on head h's chains, so
                    # it covers the tail of head h-1's vector pipeline.
                    qt_n = qproj(h + 1) if h + 1 < NH else None
                    if h > 0:
                        attnv_half(h - 1, 1)
                    for g_ in a_set:
                        score_group(h, qrt, qc, *g_)
                    if qt_n is not None:
                        qrt_n = ph.tile([128, L], F32R, tag="qrt", bufs=2,
                                        name=f"qrt{h+1}")
                        qc_n = ph.tile([128, L], F32R, tag="qc", bufs=2,
                                       name=f"qc{h+1}")
                        qrts[h + 1] = (qrt_n, qc_n)
                        rope_q_half(h + 1, qt_n, qrt_n, qc_n, 0)
                    if h == NH - 1:
                        nc.sync.dma_start(wo_c0[:], wo[:, :, 0:512])
                    for i, g_ in enumerate(b_set):
                        if i == 1:
                            attnv_half(h, 0)
                        if i == 2 and qt_n is not None:
                            rope_q_half(h + 1, qt_n, qrt_n, qc_n, 1)
                        score_group(h, qrt, qc, *g_)
                    qrts.pop(h)

                # first epilogue block (heads 0-6) covers head 7's E tail,
                # then attnv(7,1); its hh=7 matmul lands in the epilogue.
                psy0 = p2("psy")
                for hh in range(NH - 1):
                    nc.tensor.matmul(psy0[:],
                                     outtn[hh][:, 0:128], wo_c0[:, hh, :],
                                     start=(hh == 0), stop=False)
                attnv_half(NH - 1, 1, use_sasb=True)

            # ---------------- epilogue: Wo projection ----------------
            with tc.tile_pool(name="ep", bufs=1) as pe:
                def ytile_out(psy, dc, lb):
                    yt = pe.tile([128, 512], F32, tag="ytile", bufs=3)
                    nc.scalar.copy(yt[:], psy[:])
                    nc.sync.dma_start(
                        y[lb * 128:(lb + 1) * 128, dc * 512:(dc + 1) * 512],
                        yt[:])

                # finish the lb=0 block started before attnv(7,1): lb=1's
                # heads 0-6 cover the outtn[7] norm latency.
                psy1 = p2("psy")
                for hh in range(NH - 1):
                    nc.tensor.matmul(psy1[:],
                                     outtn[hh][:, 128:256], wo_c0[:, hh, :],
                                     start=(hh == 0), stop=False)
                nc.tensor.matmul(psy0[:], outtn[NH - 1][:, 0:128],
                                 wo_c0[:, NH - 1, :], start=False, stop=True)
                ytile_out(psy0, 0, 0)
                nc.tensor.matmul(psy1[:], outtn[NH - 1][:, 128:256],
                                 wo_c0[:, NH - 1, :], start=False, stop=True)
                ytile_out(psy1, 0, 1)

                wo_ts = {0: wo_c0}
                for dc in range(4):
                    if dc + 1 < 4:
                        wo_n = pe.tile([128, NH, 512], BF16, tag="wo_c", bufs=2,
                                       name=f"wo_c{dc+1}")
                        nc.sync.dma_start(
                            wo_n[:], wo[:, :, (dc + 1) * 512:(dc + 2) * 512])
                        wo_ts[dc + 1] = wo_n
                    wo_t = wo_ts.pop(dc)
                    for lb in range(2 if dc == 0 else 0, NB):
                        psy = p2("psy")
                        for hh in range(NH):
                            nc.tensor.matmul(
                                psy[:],
                                outtn[hh][:, lb * 128:(lb + 1) * 128],
                                wo_t[:, hh, :],
                                start=(hh == 0), stop=(hh == NH - 1))
                        ytile_out(psy, dc, lb)

    nc.compile()
    return nc


def _host_prep(x, Wq, Wk, Wv, Wo, q_param, log_scale, cos, sin, mask):
    """Build the 8 per-core input maps."""
    x = np.asarray(x, np.float32)
    Wq = np.asarray(Wq, np.float32)
    Wk = np.asarray(Wk, np.float32)
    Wv = np.asarray(Wv, np.float32)
    Wo = np.asarray(Wo, np.float32)
    cos = np.asarray(cos, np.float32)[0]      # [L, H, 64]
    sin = np.asarray(sin, np.float32)[0]
    qp = np.asarray(q_param, np.float32).reshape(H)
    ls = np.asarray(log_scale, np.float32).reshape(H)
    mask = np.asarray(mask)

    p64 = np.arange(128) % 64
    # rope tables are identical across heads: use head 0
    cos_p = np.ascontiguousarray(cos[:, 0, :][:, p64].T)   # [128, L]
    sin_p = np.ascontiguousarray(sin[:, 0, :][:, p64].T)

    PM = np.zeros((128, 128), np.float32)
    for dp in range(128):
        base, r = (dp // 64) * 64, dp % 64
        if r < 32:
            PM[base + r + 32, dp] = -1.0
        else:
            PM[base + r - 32, dp] = 1.0
    SW = np.zeros((128, 128), np.float32)
    for dp in range(128):
        SW[(dp + 64) % 128, dp] = 1.0
    ONES = np.ones((128, 128), ml_dtypes.bfloat16)
    IDENT = np.eye(128, dtype=ml_dtypes.bfloat16)

    in_maps = []
    for core in range(8):
        b, g2 = core // 2, core % 2
        heads = list(range(g2 * NH, (g2 + 1) * NH))

        xhv = x[b].astype(ml_dtypes.bfloat16)

        wq_c = Wq[:, g2 * NH * 128:(g2 + 1) * NH * 128]
        wk_c = Wk[:, g2 * NKV * 128:(g2 + 1) * NKV * 128]
        wv_c = Wv[:, g2 * NKV * 128:(g2 + 1) * NKV * 128]
        wo_c = Wo[g2 * NH * 128:(g2 + 1) * NH * 128, :]

        wq_p = wq_c.reshape(16, 128, NH * 128).transpose(1, 0, 2).copy()
        wk_p = wk_c.reshape(16, 128, NKV * 128).transpose(1, 0, 2).copy()
        wv_p = wv_c.reshape(16, 128, NKV * 128).transpose(1, 0, 2).copy()
        wo_p = wo_c.reshape(NH, 128, D).transpose(1, 0, 2).astype(ml_dtypes.bfloat16)

        mb = np.where(mask[b].reshape(NB, 128).T.astype(bool), 0.0, -1e9)
        mb = mb.astype(np.float32)

        cpr = -2.0 * np.tanh(qp[heads])               # per-head c'
        sq = np.sqrt(np.abs(cpr))
        sqc = np.tile(sq[None, :], (128, 1))
        sqc[:64, :] *= np.sign(cpr)[None, :]
        alp = np.tile((np.exp(ls[heads]) / HD)[None, :], (128, 1))

        in_maps.append({
            "xh": xhv,
            "wq": wq_p.astype(ml_dtypes.bfloat16),
            "wk": wk_p.astype(ml_dtypes.bfloat16),
            "wv": wv_p.astype(ml_dtypes.bfloat16), "wo": wo_p,
            "costab": cos_p, "sintab": sin_p,
            "maskb": mb, "sqrtc": sqc.astype(np.float32),
            "alpha": alp.astype(np.float32),
            "pmrot": PM, "pmswap": SW, "onesb": ONES, "identb": IDENT,
        })
    return in_maps


def kernel(**inputs):
    if "nc" not in _CACHED:
        _CACHED["nc"] = build_program()
    nc = _CACHED["nc"]
    in_maps = _host_prep(**inputs)
    res = run_bass_kernel_spmd(nc, in_maps, list(range(8))).results
    out = np.empty((B, L, D), np.float32)
    for b in range(B):
        out[b] = res[2 * b]["y"] + res[2 * b + 1]["y"]
    return out


# revision 3
# speedup vs baseline: 1.0301x; 1.0032x over previous
"""BivectorRotarySelfAttention TRN2 kernel, v3.

Sharding: 8 cores = 4 batches x 2 head-halves; host sums the two head-half
partial y's per batch.

v3 vs v2:
 - Score PSUM rings are 3-deep (SA/SB [128,512]x3) so the PE runs ~3 chunks
   ahead of the vector chain; all other PSUM users are [128,512] halves in a
   shared 2-deep ring (P2). 6+6+4 KB = 16 KB exactly.
 - Engine rebalance: rope t2/add and all raw-combines on Pool, 2 of 20 bs
   copies on DVE, rest on ACT.
 - Qproj+rope for head h+1 runs mid-scores(h); attnv halves are interleaved
   into the chunk stream so PE never waits on the exp chain.
 - V projection trails K by 4 ib-steps to match wv DMA arrival; wk DMA is
   issued before everything except nothing (first), consts after wv.
 - Epilogue prefetches wo and accumulates head 7 last.
"""
import sys
if '/opt/trn_rl_repo' not in sys.path:
    sys.path.insert(0, '/opt/trn_rl_repo')

import numpy as np
import ml_dtypes

import concourse.bass as bass
import concourse.mybir as mybir
import concourse.tile as tile
from concourse import bacc
from concourse.bass_utils import run_bass_kernel_spmd

F32 = mybir.dt.float32
F32R = mybir.dt.float32r
BF16 = mybir.dt.bfloat16

B, L, D, H, HKV = 4, 1024, 2048, 16, 4
HD = D // H            # 128
HD2 = HD // 2          # 64
NH = 8                 # heads per core
NKV = 2                # kv heads per core
NB = L // 128          # 8 blocks of 128
AluOp = mybir.AluOpType
Act = mybir.ActivationFunctionType

_CACHED = {}


def _stripe_qlo(mb):
    # stripe mb covers q in [qlo, L) in 256-wide chunks; odd stripes start one
    # 128-block early (the extra region is causal-masked to zero).
    return 128 * (mb - (mb % 2))


def build_program():
    nc = bacc.Bacc("TRN2", target_bir_lowering=False, debug=False)

    # ---- dram params (per-core shapes) ----
    xh = nc.declare_dram_parameter("xh", [L, D], BF16, isOutput=False)
    wq = nc.declare_dram_parameter("wq", [NH, 128, 16, 128], BF16, isOutput=False)
    wk = nc.declare_dram_parameter("wk", [128, 16, NKV * 128], BF16, isOutput=False)
    wv = nc.declare_dram_parameter("wv", [128, 16, NKV * 128], BF16, isOutput=False)
    wo = nc.declare_dram_parameter("wo", [128, NH, D], BF16, isOutput=False)
    costab = nc.declare_dram_parameter("costab", [128, L], F32, isOutput=False)
    sintab = nc.declare_dram_parameter("sintab", [128, L], F32, isOutput=False)
    maskb = nc.declare_dram_parameter("maskb", [128, NB], F32, isOutput=False)
    sqrtc = nc.declare_dram_parameter("sqrtc", [128, NH], F32, isOutput=False)
    alpha = nc.declare_dram_parameter("alpha", [128, NH], F32, isOutput=False)
    pmrot = nc.declare_dram_parameter("pmrot", [128, 128], F32R, isOutput=False)
    pmswap = nc.declare_dram_parameter("pmswap", [128, 128], F32R, isOutput=False)
    onesb = nc.declare_dram_parameter("onesb", [128, 128], BF16, isOutput=False)
    identb = nc.declare_dram_parameter("identb", [128, 128], BF16, isOutput=False)
    y = nc.declare_dram_parameter("y", [L, D], F32, isOutput=True)

    with tile.TileContext(nc) as tc:
        with (
            tc.tile_pool(name="persist", bufs=1) as pp,
            tc.tile_pool(name="psum", bufs=1, space="PSUM") as psp,
        ):
            # ---- persistent SBUF ----
            xt = [pp.tile([128, L], BF16, tag=f"xt{ib}", name=f"xt{ib}")
                  for ib in range(16)]
            krt = [pp.tile([128, L], F32R, tag=f"krt{g}", name=f"krt{g}")
                   for g in range(NKV)]
            kswap = [pp.tile([128, L], F32R, tag=f"ksw{g}", name=f"ksw{g}")
                     for g in range(NKV)]
            vblk = [pp.tile([128, 128], BF16, tag=f"vb{i}", name=f"vb{i}")
                    for i in range(NKV * NB)]
            outtn = [pp.tile([128, L], BF16, tag=f"ot{h}", name=f"ot{h}")
                     for h in range(NH)]
            wo_c0 = pp.tile([128, NH, 512], BF16, tag="wo_c0", name="wo_c0")
            consts = {}

            def sa(name):
                return psp.tile([128, 512], F32, tag="SA", bufs=3, name=name)

            def sb(name):
                return psp.tile([128, 512], F32, tag="SB", bufs=3, name=name)

            def p2(name, shape=None, dtype=F32):
                return psp.tile(shape or [128, 512], dtype, tag="P2", bufs=2,
                                name=name)

            # ---------------- prologue ----------------
            with tc.tile_pool(name="pro", bufs=1) as ppro:
                wk_t = ppro.tile([128, 16, NKV * 128], BF16, tag="wk")
                wv_t = ppro.tile([128, 16, NKV * 128], BF16, tag="wv")

                # DMA order: wk, 2 x-block pairs, wv, 2 pairs, consts, rest.
                nc.sync.dma_start(wk_t[:], wk[:])
                def xdma(ib):
                    nc.sync.dma_start_transpose(
                        xt[ib][:], xh[:, ib * 128:(ib + 1) * 128])

                wq_t0 = ppro.tile([128, 16, 128], BF16, tag="wq0", name="wq_t0")
                xdma(0)
                xdma(1)
                nc.sync.dma_start(wv_t[:], wv[:])
                for ib in range(2, 16):
                    xdma(ib)
                nc.sync.dma_start(wq_t0[:], wq[0])
                for nm, src, dt_ in [("pmrot", pmrot, F32R),
                                     ("costab", costab, F32),
                                     ("sintab", sintab, F32),
                                     ("identb", identb, BF16),
                                     ("pmswap", pmswap, F32R),
                                     ("onesb", onesb, BF16),
                                     ("maskb", maskb, F32),
                                     ("sqrtc", sqrtc, F32),
                                     ("alpha", alpha, F32)]:
                    t = pp.tile(list(src.shape), dt_, tag=nm, name=nm)
                    nc.sync.dma_start(t[:], src[:])
                    consts[nm] = t

                # K/V projection accumulators: [128,512] halves.
                psk = [[sa("psk0a"), sa("psk0b")], [sb("psk1a"), sb("psk1b")]]
                psv = [[sa("psv0a"), sb("psv0b")], [p2("psv1a"), p2("psv1b")]]

                VOFF = 2
                for step in range(16 + VOFF):
                    if step < 16:
                        ib = step
                        for g in range(NKV):
                            for c in range(2):
                                nc.tensor.matmul(
                                    psk[g][c][:],
                                    wk_t[:, ib, g * 128:(g + 1) * 128],
                                    xt[ib][:, c * 512:(c + 1) * 512],
                                    start=(ib == 0), stop=(ib == 15))
                    if step >= VOFF:
                        ib = step - VOFF
                        for g in range(NKV):
                            for c in range(2):
                                nc.tensor.matmul(
                                    psv[g][c][:],
                                    wv_t[:, ib, g * 128:(g + 1) * 128],
                                    xt[ib][:, c * 512:(c + 1) * 512],
                                    start=(ib == 0), stop=(ib == 15))

                # PSUM->SBUF copies: v-g1 first (frees the P2 slots that
                # Qproj(0) needs), then interleaved kt/vt.
                vt_s = [ppro.tile([128, L], BF16, tag=f"vt_s{g}", name=f"vt{g}")
                        for g in range(NKV)]
                kt_s = [ppro.tile([128, L], F32R, tag=f"kt_s{g}", name=f"kt{g}")
                        for g in range(NKV)]
                for c in range(2):
                    nc.scalar.copy(vt_s[1][:, c * 512:(c + 1) * 512], psv[1][c][:])
                for g in range(NKV):
                    for c in range(2):
                        nc.scalar.copy(kt_s[g][:, c * 512:(c + 1) * 512],
                                       psk[g][c][:])
                for c in range(2):
                    nc.scalar.copy(vt_s[0][:, c * 512:(c + 1) * 512], psv[0][c][:])

                # Qproj(0): no dependency on the copies above except P2 slots;
                # covers the kt/vt copy chain on PE.
                qt_s0 = ppro.tile([128, L], F32R, tag="qt_s0", name="qt_s0")
                for cc in range(2):
                    psq = p2("psq0")
                    for ib in range(16):
                        nc.tensor.matmul(psq[:], wq_t0[:, ib, :],
                                         xt[ib][:, cc * 512:(cc + 1) * 512],
                                         start=(ib == 0), stop=(ib == 15))
                    nc.vector.tensor_copy(qt_s0[:, cc * 512:(cc + 1) * 512],
                                          psq[:])

                # k rope rotate-half part (SA slots freed by the kt copies)
                for g in range(NKV):
                    t1 = ppro.tile([128, L], F32, tag="rtmp", bufs=4)
                    t2 = ppro.tile([128, L], F32, tag="rtmp", bufs=4)
                    for c in range(2):
                        psr = sa(f"psr_k{c}")
                        nc.tensor.matmul(psr[:],
                                         consts["pmrot"][:],
                                         kt_s[g][:, c * 512:(c + 1) * 512])
                        nc.vector.tensor_mul(
                            t1[:, c * 512:(c + 1) * 512], psr[:],
                            consts["sintab"][:, c * 512:(c + 1) * 512])
                    nc.gpsimd.tensor_mul(t2[:], kt_s[g][:].bitcast(F32),
                                         consts["costab"][:])
                    nc.vector.tensor_add(krt[g][:], t1[:], t2[:])

                # v transpose to [m, d] blocks
                for g in range(NKV):
                    for mb in range(NB):
                        pv = p2("pv", shape=[128, 128], dtype=BF16)
                        nc.tensor.transpose(pv[:], vt_s[g][:, mb * 128:(mb + 1) * 128],
                                            consts["identb"][:])
                        nc.vector.tensor_copy(vblk[g * NB + mb][:], pv[:])

                # rope-q(0): rotate part (qt_s0 ready after its DVE copies)
                t1q = ppro.tile([128, L], F32, tag="rtmp", bufs=4, name="t1q0")
                t2q = ppro.tile([128, L], F32, tag="rtmp", bufs=4, name="t2q0")
                for cc in range(2):
                    psr2 = p2("psr_q0")
                    nc.tensor.matmul(psr2[:], consts["pmrot"][:],
                                     qt_s0[:, cc * 512:(cc + 1) * 512])
                    nc.vector.tensor_mul(
                        t1q[:, cc * 512:(cc + 1) * 512], psr2[:],
                        consts["sintab"][:, cc * 512:(cc + 1) * 512])
                nc.gpsimd.tensor_mul(t2q[:], qt_s0[:].bitcast(F32),
                                     consts["costab"][:])
                qrt0 = pp.tile([128, L], F32R, tag="qrt0", name="qrt0")
                nc.gpsimd.tensor_add(qrt0[:], t1q[:], t2q[:])
                qc0 = pp.tile([128, L], F32R, tag="qc0", name="qc0")
                nc.vector.tensor_scalar_mul(qc0[:], qrt0[:].bitcast(F32),
                                            consts["sqrtc"][:, 0:1])

                # k swap perms (krt chains complete under the ops above)
                for g in range(NKV):
                    for c in range(2):
                        psw = sb(f"psw_k{g}{c}")
                        nc.tensor.matmul(psw[:],
                                         consts["pmswap"][:],
                                         krt[g][:, c * 512:(c + 1) * 512])
                        nc.scalar.copy(kswap[g][:, c * 512:(c + 1) * 512],
                                       psw[:])

            # ---------------- head loop ----------------
            with tc.tile_pool(name="hl", bufs=1) as ph:
                etiles = {}

                wq_tiles = {}

                def wq_dma(h):
                    if h < NH:
                        wq_t = ph.tile([128, 16, 128], BF16, tag="wq_h", bufs=2,
                                       name=f"wq_t{h}")
                        nc.sync.dma_start(wq_t[:], wq[h])
                        wq_tiles[h] = wq_t

                def qproj(h):
                    wq_t = wq_tiles.pop(h)
                    qt_s = ph.tile([128, L], F32R, tag="qt_s", bufs=2)
                    for cc in range(2):
                        psq = p2("psq")
                        for ib in range(16):
                            nc.tensor.matmul(
                                psq[:],
                                wq_t[:, ib, :],
                                xt[ib][:, cc * 512:(cc + 1) * 512],
                                start=(ib == 0), stop=(ib == 15))
                        nc.vector.tensor_copy(qt_s[:, cc * 512:(cc + 1) * 512],
                                              psq[:])
                    return qt_s

                def rope_q_half(h, qt_s, qrt, qc, cc):
                    """one 512-half of q-rope + c'-scaled copy, on DVE"""
                    hs = slice(cc * 512, (cc + 1) * 512)
                    t1 = ph.tile([128, 512], F32, tag="qtmp", bufs=2)
                    t2 = ph.tile([128, 512], F32, tag="qtmp", bufs=2)
                    psr2 = p2("psr_q")
                    nc.tensor.matmul(psr2[:], consts["pmrot"][:], qt_s[:, hs])
                    nc.vector.tensor_mul(t1[:], psr2[:], consts["sintab"][:, hs])
                    nc.vector.tensor_mul(t2[:], qt_s[:, hs].bitcast(F32),
                                         consts["costab"][:, hs])
                    nc.vector.tensor_add(qrt[:, hs], t1[:], t2[:])
                    nc.vector.tensor_scalar_mul(qc[:, hs], qrt[:, hs].bitcast(F32),
                                                consts["sqrtc"][:, h:h + 1])

                def make_etiles(h):
                    ets = []
                    for mb in range(NB):
                        qlo = _stripe_qlo(mb)
                        et = ph.tile([128, L - qlo], BF16, tag=f"esc{mb}", bufs=2,
                                     name=f"esc_h{mb}")
                        ets.append(et)
                    etiles[h] = ets

                def score_group(h, qrt, qc, mb, qs, npair):
                    """npair chunks (1 or 2) of stripe mb starting at qs; one
                    fused exp over the pair."""
                    g = h // 4
                    qlo = _stripe_qlo(mb)
                    kb = slice(mb * 128, (mb + 1) * 128)
                    ets = etiles[h]
                    raw = ph.tile([128, 512], F32, tag="raw", bufs=3)
                    for j in range(npair):
                        cqs = qs + 256 * j
                        cqe = cqs + 256
                        psA = sa("psA")
                        psB = sb("psB")
                        nc.tensor.matmul(psA[:, 0:256], krt[g][0:64, kb],
                                         qrt[0:64, cqs:cqe])
                        nc.tensor.matmul(psA[:, 256:512], kswap[g][0:64, kb],
                                         qc[0:64, cqs:cqe])
                        nc.tensor.matmul(psB[:, 0:256], krt[g][64:128, kb],
                                         qrt[64:128, cqs:cqe])
                        nc.tensor.matmul(psB[:, 256:512], kswap[g][64:128, kb],
                                         qc[64:128, cqs:cqe])
                        bs = ph.tile([128, 512], F32, tag="bs", bufs=3)
                        nc.scalar.copy(bs[:], psB[:])
                        tp = ph.tile([128, 512], F32, tag="tprod", bufs=3)
                        nc.vector.tensor_mul(tp[:], psA[:], bs[:])
                        nc.gpsimd.tensor_add(raw[:, 256 * j:256 * (j + 1)],
                                             tp[:, 0:256], tp[:, 256:512])
                    w = 256 * npair
                    esl = ets[mb][:, qs - qlo: qs - qlo + w]
                    nc.scalar.activation(esl, raw[:, 0:w], Act.Exp,
                                         bias=consts["maskb"][:, mb:mb + 1],
                                         scale=consts["alpha"][:, h:h + 1])
                    if qs == qlo:
                        # causal mask on the diagonal 256 cols:
                        # keep where (qlo + col) - (128*mb + part) >= 0
                        nc.gpsimd.affine_select(
                            ets[mb][:, 0:256], ets[mb][:, 0:256],
                            pattern=[[1, 256]], compare_op=AluOp.is_ge,
                            fill=0.0, base=qlo - 128 * mb,
                            channel_multiplier=-1)

                def attnv_half(h, c, use_sasb=True):
                    g = h // 4
                    ets = etiles[h]
                    ps_o, ps_rs = sa("ps_o"), sb("ps_rs")
                    mbs = [mb for mb in range(NB) if 128 * mb < 512 * (c + 1)]
                    for i, mb in enumerate(mbs):
                        qlo = _stripe_qlo(mb)
                        os_ = max(512 * c, 128 * mb)
                        oe = 512 * (c + 1)
                        esl = ets[mb][:, os_ - qlo: oe - qlo]
                        st, sp = (i == 0), (i == len(mbs) - 1)
                        nc.tensor.matmul(ps_o[:, os_ - 512 * c: oe - 512 * c],
                                         vblk[g * NB + mb][:], esl,
                                         start=st, stop=sp)
                        nc.tensor.matmul(ps_rs[:, os_ - 512 * c: oe - 512 * c],
                                         consts["onesb"][:], esl,
                                         start=st, stop=sp)
                    rcp = ph.tile([128, 512], F32, tag="rcp", bufs=2)
                    nc.vector.reciprocal_approx_fast(rcp[:], ps_rs[:])
                    nc.vector.tensor_mul(outtn[h][:, c * 512:(c + 1) * 512],
                                         ps_o[:], rcp[:])

                # chunk groups (mb, qs, npair): a = groups with qs < 512 of
                # stripes 0-3 (cover attnv c=0), b = the rest
                a_set = [(0, 0, 2), (1, 0, 2), (2, 256, 1), (3, 256, 1)]
                b_set = [(0, 512, 2), (1, 512, 2), (2, 512, 2), (3, 512, 2),
                         (4, 512, 2), (5, 512, 2), (6, 768, 1), (7, 768, 1)]

                qrts = {0: (qrt0, qc0)}
                wq_dma(1)
                for h in range(NH):
                    make_etiles(h)
                    qrt, qc = qrts[h]
                    wq_dma(h + 2)
                    # Qproj(h+1) first: no dependency on head h's chains, so
                    # it covers the tail of head h-1's vector pipeline.
                    qt_n = qproj(h + 1) if h + 1 < NH else None
                    if h > 0:
                        attnv_half(h - 1, 1)
                    for g_ in a_set:
                        score_group(h, qrt, qc, *g_)
                    if qt_n is not None:
                        qrt_n = ph.tile([128, L], F32R, tag="qrt", bufs=2,
                                        name=f"qrt{h+1}")
                        qc_n = ph.tile([128, L], F32R, tag="qc", bufs=2,
                                       name=f"qc{h+1}")
                        qrts[h + 1] = (qrt_n, qc_n)
                        rope_q_half(h + 1, qt_n, qrt_n, qc_n, 0)
                    if h == NH - 1:
                        nc.sync.dma_start(wo_c0[:], wo[:, :, 0:512])
                    for i, g_ in enumerate(b_set):
                        if i == 1:
                            attnv_half(h, 0)
                        if i == 2 and qt_n is not None:
                            rope_q_half(h + 1, qt_n, qrt_n, qc_n, 1)
                        score_group(h, qrt, qc, *g_)
                    qrts.pop(h)

                # first epilogue block (heads 0-6) covers head 7's E tail,
                # then attnv(7,1); its hh=7 matmul lands in the epilogue.
                psy0 = p2("psy")
                for hh in range(NH - 1):
                    nc.tensor.matmul(psy0[:],
                                     outtn[hh][:, 0:128], wo_c0[:, hh, :],
                                     start=(hh == 0), stop=False)
                attnv_half(NH - 1, 1, use_sasb=True)

            # ---------------- epilogue: Wo projection ----------------
            with tc.tile_pool(name="ep", bufs=1) as pe:
                def ytile_out(psy, dc, lb):
                    yt = pe.tile([128, 512], F32, tag="ytile", bufs=3)
                    nc.scalar.copy(yt[:], psy[:])
                    nc.sync.dma_start(
                        y[lb * 128:(lb + 1) * 128, dc * 512:(dc + 1) * 512],
                        yt[:])

                # finish the lb=0 block started before attnv(7,1): lb=1's
                # heads 0-6 cover the outtn[7] norm latency.
                psy1 = p2("psy")
                for hh in range(NH - 1):
                    nc.tensor.matmul(psy1[:],
                                     outtn[hh][:, 128:256], wo_c0[:, hh, :],
                                     start=(hh == 0), stop=False)
                nc.tensor.matmul(psy0[:], outtn[NH - 1][:, 0:128],
                                 wo_c0[:, NH - 1, :], start=False, stop=True)
                ytile_out(psy0, 0, 0)
                nc.tensor.matmul(psy1[:], outtn[NH - 1][:, 128:256],
                                 wo_c0[:, NH - 1, :], start=False, stop=True)
                ytile_out(psy1, 0, 1)

                wo_ts = {0: wo_c0}
                for dc in range(4):
                    if dc + 1 < 4:
                        wo_n = pe.tile([128, NH, 512], BF16, tag="wo_c", bufs=2,
                                       name=f"wo_c{dc+1}")
                        nc.sync.dma_start(
                            wo_n[:], wo[:, :, (dc + 1) * 512:(dc + 2) * 512])
                        wo_ts[dc + 1] = wo_n
                    wo_t = wo_ts.pop(dc)
                    for lb in range(2 if dc == 0 else 0, NB):
                        psy = p2("psy")
                        for hh in range(NH):
                            nc.tensor.matmul(
                                psy[:],
                                outtn[hh][:, lb * 128:(lb + 1) * 128],
                                wo_t[:, hh, :],
                                start=(hh == 0), stop=(hh == NH - 1))
                        ytile_out(psy, dc, lb)

    nc.compile()
    return nc


def _host_prep(x, Wq, Wk, Wv, Wo, q_param, log_scale, cos, sin, mask):
    """Build the 8 per-core input maps."""
    x = np.asarray(x, np.float32)
    Wq = np.asarray(Wq, np.float32)
    Wk = np.asarray(Wk, np.float32)
    Wv = np.asarray(Wv, np.float32)
    Wo = np.asarray(Wo, np.float32)
    cos = np.asarray(cos, np.float32)[0]      # [L, H, 64]
    sin = np.asarray(sin, np.float32)[0]
    qp = np.asarray(q_param, np.float32).reshape(H)
    ls = np.asarray(log_scale, np.float32).reshape(H)
    mask = np.asarray(mask)

    p64 = np.arange(128) % 64
    # rope tables are identical across heads: use head 0
    cos_p = np.ascontiguousarray(cos[:, 0, :][:, p64].T)   # [128, L]
    sin_p = np.ascontiguousarray(sin[:, 0, :][:, p64].T)

    PM = np.zeros((128, 128), np.float32)
    for dp in range(128):
        base, r = (dp // 64) * 64, dp % 64
        if r < 32:
            PM[base + r + 32, dp] = -1.0
        else:
            PM[base + r - 32, dp] = 1.0
    SW = np.zeros((128, 128), np.float32)
    for dp in range(128):
        SW[(dp + 64) % 128, dp] = 1.0
    ONES = np.ones((128, 128), ml_dtypes.bfloat16)
    IDENT = np.eye(128, dtype=ml_dtypes.bfloat16)

    in_maps = []
    for core in range(8):
        b, g2 = core // 2, core % 2
        heads = list(range(g2 * NH, (g2 + 1) * NH))

        xhv = x[b].astype(ml_dtypes.bfloat16)

        wq_c = Wq[:, g2 * NH * 128:(g2 + 1) * NH * 128]
        wk_c = Wk[:, g2 * NKV * 128:(g2 + 1) * NKV * 128]
        wv_c = Wv[:, g2 * NKV * 128:(g2 + 1) * NKV * 128]
        wo_c = Wo[g2 * NH * 128:(g2 + 1) * NH * 128, :]

        wq_p = wq_c.reshape(16, 128, NH, 128).transpose(2, 1, 0, 3).copy()
        wk_p = wk_c.reshape(16, 128, NKV * 128).transpose(1, 0, 2).copy()
        wv_p = wv_c.reshape(16, 128, NKV * 128).transpose(1, 0, 2).copy()
        wo_p = wo_c.reshape(NH, 128, D).transpose(1, 0, 2).astype(ml_dtypes.bfloat16)

        mb = np.where(mask[b].reshape(NB, 128).T.astype(bool), 0.0, -1e9)
        mb = mb.astype(np.float32)

        cpr = -2.0 * np.tanh(qp[heads])               # per-head c'
        sq = np.sqrt(np.abs(cpr))
        sqc = np.tile(sq[None, :], (128, 1))
        sqc[:64, :] *= np.sign(cpr)[None, :]
        alp = np.tile((np.exp(ls[heads]) / HD)[None, :], (128, 1))

        in_maps.append({
            "xh": xhv,
            "wq": wq_p.astype(ml_dtypes.bfloat16),
            "wk": wk_p.astype(ml_dtypes.bfloat16),
            "wv": wv_p.astype(ml_dtypes.bfloat16), "wo": wo_p,
            "costab": cos_p, "sintab": sin_p,
            "maskb": mb, "sqrtc": sqc.astype(np.float32),
            "alpha": alp.astype(np.float32),
            "pmrot": PM, "pmswap": SW, "onesb": ONES, "identb": IDENT,
        })
    return in_maps


def kernel(**inputs):
    if "nc" not in _CACHED:
        _CACHED["nc"] = build_program()
    nc = _CACHED["nc"]
    in_maps = _host_prep(**inputs)
    res = run_bass_kernel_spmd(nc, in_maps, list(range(8))).results
    out = np.empty((B, L, D), np.float32)
    for b in range(B):
        out[b] = res[2 * b]["y"] + res[2 * b + 1]["y"]
    return out


# revision 4
# speedup vs baseline: 1.0372x; 1.0068x over previous
"""BivectorRotarySelfAttention TRN2 kernel, v3.

Sharding: 8 cores = 4 batches x 2 head-halves; host sums the two head-half
partial y's per batch.

v3 vs v2:
 - Score PSUM rings are 3-deep (SA/SB [128,512]x3) so the PE runs ~3 chunks
   ahead of the vector chain; all other PSUM users are [128,512] halves in a
   shared 2-deep ring (P2). 6+6+4 KB = 16 KB exactly.
 - Engine rebalance: rope t2/add and all raw-combines on Pool, 2 of 20 bs
   copies on DVE, rest on ACT.
 - Qproj+rope for head h+1 runs mid-scores(h); attnv halves are interleaved
   into the chunk stream so PE never waits on the exp chain.
 - V projection trails K by 4 ib-steps to match wv DMA arrival; wk DMA is
   issued before everything except nothing (first), consts after wv.
 - Epilogue prefetches wo and accumulates head 7 last.
"""
import sys
if '/opt/trn_rl_repo' not in sys.path:
    sys.path.insert(0, '/opt/trn_rl_repo')

import numpy as np
import ml_dtypes

import concourse.bass as bass
import concourse.mybir as mybir
import concourse.tile as tile
from concourse import bacc
from concourse.bass_utils import run_bass_kernel_spmd

F32 = mybir.dt.float32
F32R = mybir.dt.float32r
BF16 = mybir.dt.bfloat16

B, L, D, H, HKV = 4, 1024, 2048, 16, 4
HD = D // H            # 128
HD2 = HD // 2          # 64
NH = 8                 # heads per core
NKV = 2                # kv heads per core
NB = L // 128          # 8 blocks of 128
AluOp = mybir.AluOpType
Act = mybir.ActivationFunctionType

_CACHED = {}


def _stripe_qlo(mb):
    # stripe mb covers q in [qlo, L) in 256-wide chunks; odd stripes start one
    # 128-block early (the extra region is causal-masked to zero).
    return 128 * (mb - (mb % 2))


def build_program():
    nc = bacc.Bacc("TRN2", target_bir_lowering=False, debug=False)

    # ---- dram params (per-core shapes) ----
    xh = nc.declare_dram_parameter("xh", [L, D], BF16, isOutput=False)
    wq = nc.declare_dram_parameter("wq", [NH, 128, 16, 128], BF16, isOutput=False)
    wk = nc.declare_dram_parameter("wk", [128, 16, NKV * 128], BF16, isOutput=False)
    wv = nc.declare_dram_parameter("wv", [128, 16, NKV * 128], BF16, isOutput=False)
    wo = nc.declare_dram_parameter("wo", [128, NH, D], BF16, isOutput=False)
    costab = nc.declare_dram_parameter("costab", [128, L], F32, isOutput=False)
    sintab = nc.declare_dram_parameter("sintab", [128, L], F32, isOutput=False)
    maskb = nc.declare_dram_parameter("maskb", [128, NB], F32, isOutput=False)
    sqrtc = nc.declare_dram_parameter("sqrtc", [128, NH], F32, isOutput=False)
    alpha = nc.declare_dram_parameter("alpha", [128, NH], F32, isOutput=False)
    pmrot = nc.declare_dram_parameter("pmrot", [128, 128], F32R, isOutput=False)
    pmswap = nc.declare_dram_parameter("pmswap", [128, 128], F32R, isOutput=False)
    onesb = nc.declare_dram_parameter("onesb", [128, 128], BF16, isOutput=False)
    identb = nc.declare_dram_parameter("identb", [128, 128], BF16, isOutput=False)
    y = nc.declare_dram_parameter("y", [L, D], F32, isOutput=True)

    with tile.TileContext(nc) as tc:
        with (
            tc.tile_pool(name="persist", bufs=1) as pp,
            tc.tile_pool(name="psum", bufs=1, space="PSUM") as psp,
        ):
            # ---- persistent SBUF ----
            xt = [pp.tile([128, L], BF16, tag=f"xt{ib}", name=f"xt{ib}")
                  for ib in range(16)]
            krt = [pp.tile([128, L], F32R, tag=f"krt{g}", name=f"krt{g}")
                   for g in range(NKV)]
            kswap = [pp.tile([128, L], F32R, tag=f"ksw{g}", name=f"ksw{g}")
                     for g in range(NKV)]
            vblk = [pp.tile([128, 128], BF16, tag=f"vb{i}", name=f"vb{i}")
                    for i in range(NKV * NB)]
            outtn = [pp.tile([128, L], BF16, tag=f"ot{h}", name=f"ot{h}")
                     for h in range(NH)]
            wo_c0 = pp.tile([128, NH, 512], BF16, tag="wo_c0", name="wo_c0")
            consts = {}

            def sa(name):
                return psp.tile([128, 512], F32, tag="SA", bufs=3, name=name)

            def sb(name):
                return psp.tile([128, 512], F32, tag="SB", bufs=3, name=name)

            def p2(name, shape=None, dtype=F32):
                return psp.tile(shape or [128, 512], dtype, tag="P2", bufs=2,
                                name=name)

            # ---------------- prologue ----------------
            with tc.tile_pool(name="pro", bufs=1) as ppro:
                wk_t = ppro.tile([128, 16, NKV * 128], BF16, tag="wk")
                wv_t = ppro.tile([128, 16, NKV * 128], BF16, tag="wv")

                # DMA order: wk, 2 x-block pairs, wv, 2 pairs, consts, rest.
                nc.sync.dma_start(wk_t[:], wk[:])
                def xdma(ib):
                    nc.sync.dma_start_transpose(
                        xt[ib][:], xh[:, ib * 128:(ib + 1) * 128])

                wq_t0 = ppro.tile([128, 16, 128], BF16, tag="wq0", name="wq_t0")
                xdma(0)
                xdma(1)
                nc.sync.dma_start(wv_t[:], wv[:])
                for ib in range(2, 16):
                    xdma(ib)
                nc.sync.dma_start(wq_t0[:], wq[0])
                for nm, src, dt_ in [("pmrot", pmrot, F32R),
                                     ("costab", costab, F32),
                                     ("sintab", sintab, F32),
                                     ("identb", identb, BF16),
                                     ("pmswap", pmswap, F32R),
                                     ("onesb", onesb, BF16),
                                     ("maskb", maskb, F32),
                                     ("sqrtc", sqrtc, F32),
                                     ("alpha", alpha, F32)]:
                    t = pp.tile(list(src.shape), dt_, tag=nm, name=nm)
                    nc.sync.dma_start(t[:], src[:])
                    consts[nm] = t

                # K/V projection accumulators: [128,512] halves.
                psk = [[sa("psk0a"), sa("psk0b")], [sb("psk1a"), sb("psk1b")]]
                psv = [[sa("psv0a"), sb("psv0b")], [p2("psv1a"), p2("psv1b")]]

                VOFF = 2
                for step in range(16 + VOFF):
                    if step < 16:
                        ib = step
                        for g in range(NKV):
                            for c in range(2):
                                nc.tensor.matmul(
                                    psk[g][c][:],
                                    wk_t[:, ib, g * 128:(g + 1) * 128],
                                    xt[ib][:, c * 512:(c + 1) * 512],
                                    start=(ib == 0), stop=(ib == 15))
                    if step >= VOFF:
                        ib = step - VOFF
                        for g in range(NKV):
                            for c in range(2):
                                nc.tensor.matmul(
                                    psv[g][c][:],
                                    wv_t[:, ib, g * 128:(g + 1) * 128],
                                    xt[ib][:, c * 512:(c + 1) * 512],
                                    start=(ib == 0), stop=(ib == 15))

                # PSUM->SBUF copies: v-g1 first (frees the P2 slots that
                # Qproj(0) needs), then interleaved kt/vt.
                vt_s = [ppro.tile([128, L], BF16, tag=f"vt_s{g}", name=f"vt{g}")
                        for g in range(NKV)]
                kt_s = [ppro.tile([128, L], F32R, tag=f"kt_s{g}", name=f"kt{g}")
                        for g in range(NKV)]
                for c in range(2):
                    nc.scalar.copy(vt_s[1][:, c * 512:(c + 1) * 512], psv[1][c][:])
                for g in range(NKV):
                    for c in range(2):
                        nc.scalar.copy(kt_s[g][:, c * 512:(c + 1) * 512],
                                       psk[g][c][:])
                for c in range(2):
                    nc.scalar.copy(vt_s[0][:, c * 512:(c + 1) * 512], psv[0][c][:])

                # Qproj(0): no dependency on the copies above except P2 slots;
                # covers the kt/vt copy chain on PE.
                qt_s0 = ppro.tile([128, L], F32R, tag="qt_s0", name="qt_s0")
                for cc in range(2):
                    psq = p2("psq0")
                    for ib in range(16):
                        nc.tensor.matmul(psq[:], wq_t0[:, ib, :],
                                         xt[ib][:, cc * 512:(cc + 1) * 512],
                                         start=(ib == 0), stop=(ib == 15))
                    nc.vector.tensor_copy(qt_s0[:, cc * 512:(cc + 1) * 512],
                                          psq[:])

                # rope-q0 chain starts immediately (DVE halves); the k-rope /
                # v-transpose / k-swap PE work below covers it.
                qrt0 = pp.tile([128, L], F32R, tag="qrt0", name="qrt0")
                qc0 = pp.tile([128, L], F32R, tag="qc0", name="qc0")
                for cc in range(2):
                    hs = slice(cc * 512, (cc + 1) * 512)
                    t1q = ppro.tile([128, 512], F32, tag="rq0", bufs=2)
                    t2q = ppro.tile([128, 512], F32, tag="rq0", bufs=2)
                    psr2 = p2("psr_q0")
                    nc.tensor.matmul(psr2[:], consts["pmrot"][:], qt_s0[:, hs])
                    nc.vector.tensor_mul(t1q[:], psr2[:], consts["sintab"][:, hs])
                    nc.vector.tensor_mul(t2q[:], qt_s0[:, hs].bitcast(F32),
                                         consts["costab"][:, hs])
                    nc.vector.tensor_add(qrt0[:, hs], t1q[:], t2q[:])
                    nc.vector.tensor_scalar_mul(qc0[:, hs],
                                                qrt0[:, hs].bitcast(F32),
                                                consts["sqrtc"][:, 0:1])

                # k rope rotate-half part (SA slots freed by the kt copies)
                for g in range(NKV):
                    t1 = ppro.tile([128, L], F32, tag="rtmp", bufs=4)
                    t2 = ppro.tile([128, L], F32, tag="rtmp", bufs=4)
                    for c in range(2):
                        psr = sa(f"psr_k{c}")
                        nc.tensor.matmul(psr[:],
                                         consts["pmrot"][:],
                                         kt_s[g][:, c * 512:(c + 1) * 512])
                        nc.vector.tensor_mul(
                            t1[:, c * 512:(c + 1) * 512], psr[:],
                            consts["sintab"][:, c * 512:(c + 1) * 512])
                    nc.gpsimd.tensor_mul(t2[:], kt_s[g][:].bitcast(F32),
                                         consts["costab"][:])
                    nc.vector.tensor_add(krt[g][:], t1[:], t2[:])

                # v transpose to [m, d] blocks
                for g in range(NKV):
                    for mb in range(NB):
                        pv = p2("pv", shape=[128, 128], dtype=BF16)
                        nc.tensor.transpose(pv[:], vt_s[g][:, mb * 128:(mb + 1) * 128],
                                            consts["identb"][:])
                        nc.vector.tensor_copy(vblk[g * NB + mb][:], pv[:])

                # k swap perms (krt chains complete under the ops above)
                for g in range(NKV):
                    for c in range(2):
                        psw = sb(f"psw_k{g}{c}")
                        nc.tensor.matmul(psw[:],
                                         consts["pmswap"][:],
                                         krt[g][:, c * 512:(c + 1) * 512])
                        nc.scalar.copy(kswap[g][:, c * 512:(c + 1) * 512],
                                       psw[:])

            # ---------------- head loop ----------------
            with tc.tile_pool(name="hl", bufs=1) as ph:
                etiles = {}

                wq_tiles = {}

                def wq_dma(h):
                    if h < NH:
                        wq_t = ph.tile([128, 16, 128], BF16, tag="wq_h", bufs=2,
                                       name=f"wq_t{h}")
                        nc.sync.dma_start(wq_t[:], wq[h])
                        wq_tiles[h] = wq_t

                def qproj(h):
                    wq_t = wq_tiles.pop(h)
                    qt_s = ph.tile([128, L], F32R, tag="qt_s", bufs=2)
                    for cc in range(2):
                        psq = p2("psq")
                        for ib in range(16):
                            nc.tensor.matmul(
                                psq[:],
                                wq_t[:, ib, :],
                                xt[ib][:, cc * 512:(cc + 1) * 512],
                                start=(ib == 0), stop=(ib == 15))
                        nc.vector.tensor_copy(qt_s[:, cc * 512:(cc + 1) * 512],
                                              psq[:])
                    return qt_s

                def rope_q_half(h, qt_s, qrt, qc, cc):
                    """one 512-half of q-rope + c'-scaled copy, on DVE"""
                    hs = slice(cc * 512, (cc + 1) * 512)
                    t1 = ph.tile([128, 512], F32, tag="qtmp", bufs=2)
                    t2 = ph.tile([128, 512], F32, tag="qtmp", bufs=2)
                    psr2 = p2("psr_q")
                    nc.tensor.matmul(psr2[:], consts["pmrot"][:], qt_s[:, hs])
                    nc.vector.tensor_mul(t1[:], psr2[:], consts["sintab"][:, hs])
                    nc.vector.tensor_mul(t2[:], qt_s[:, hs].bitcast(F32),
                                         consts["costab"][:, hs])
                    nc.vector.tensor_add(qrt[:, hs], t1[:], t2[:])
                    nc.vector.tensor_scalar_mul(qc[:, hs], qrt[:, hs].bitcast(F32),
                                                consts["sqrtc"][:, h:h + 1])

                def make_etiles(h):
                    ets = []
                    for mb in range(NB):
                        qlo = _stripe_qlo(mb)
                        et = ph.tile([128, L - qlo], BF16, tag=f"esc{mb}", bufs=2,
                                     name=f"esc_h{mb}")
                        ets.append(et)
                    etiles[h] = ets

                def score_group(h, qrt, qc, mb, qs, npair, bs_dve=False):
                    """npair chunks (1 or 2) of stripe mb starting at qs; one
                    fused exp over the pair."""
                    g = h // 4
                    qlo = _stripe_qlo(mb)
                    kb = slice(mb * 128, (mb + 1) * 128)
                    ets = etiles[h]
                    raw = ph.tile([128, 512], F32, tag="raw", bufs=3)
                    for j in range(npair):
                        cqs = qs + 256 * j
                        cqe = cqs + 256
                        psA = sa("psA")
                        psB = sb("psB")
                        nc.tensor.matmul(psA[:, 0:256], krt[g][0:64, kb],
                                         qrt[0:64, cqs:cqe])
                        nc.tensor.matmul(psA[:, 256:512], kswap[g][0:64, kb],
                                         qc[0:64, cqs:cqe])
                        nc.tensor.matmul(psB[:, 0:256], krt[g][64:128, kb],
                                         qrt[64:128, cqs:cqe])
                        nc.tensor.matmul(psB[:, 256:512], kswap[g][64:128, kb],
                                         qc[64:128, cqs:cqe])
                        bs = ph.tile([128, 512], F32, tag="bs", bufs=3)
                        if bs_dve and j == 0:
                            nc.vector.tensor_copy(bs[:], psB[:])
                        else:
                            nc.scalar.copy(bs[:], psB[:])
                        tp = ph.tile([128, 512], F32, tag="tprod", bufs=3)
                        nc.vector.tensor_mul(tp[:], psA[:], bs[:])
                        nc.gpsimd.tensor_add(raw[:, 256 * j:256 * (j + 1)],
                                             tp[:, 0:256], tp[:, 256:512])
                    w = 256 * npair
                    esl = ets[mb][:, qs - qlo: qs - qlo + w]
                    nc.scalar.activation(esl, raw[:, 0:w], Act.Exp,
                                         bias=consts["maskb"][:, mb:mb + 1],
                                         scale=consts["alpha"][:, h:h + 1])
                    if qs == qlo:
                        # causal mask on the diagonal 256 cols:
                        # keep where (qlo + col) - (128*mb + part) >= 0
                        nc.gpsimd.affine_select(
                            ets[mb][:, 0:256], ets[mb][:, 0:256],
                            pattern=[[1, 256]], compare_op=AluOp.is_ge,
                            fill=0.0, base=qlo - 128 * mb,
                            channel_multiplier=-1)

                def attnv_half(h, c, use_sasb=True):
                    g = h // 4
                    ets = etiles[h]
                    ps_o, ps_rs = sa("ps_o"), sb("ps_rs")
                    mbs = [mb for mb in range(NB) if 128 * mb < 512 * (c + 1)]
                    for i, mb in enumerate(mbs):
                        qlo = _stripe_qlo(mb)
                        os_ = max(512 * c, 128 * mb)
                        oe = 512 * (c + 1)
                        esl = ets[mb][:, os_ - qlo: oe - qlo]
                        st, sp = (i == 0), (i == len(mbs) - 1)
                        nc.tensor.matmul(ps_o[:, os_ - 512 * c: oe - 512 * c],
                                         vblk[g * NB + mb][:], esl,
                                         start=st, stop=sp)
                        nc.tensor.matmul(ps_rs[:, os_ - 512 * c: oe - 512 * c],
                                         consts["onesb"][:], esl,
                                         start=st, stop=sp)
                    rcp = ph.tile([128, 512], F32, tag="rcp", bufs=2)
                    nc.vector.reciprocal_approx_fast(rcp[:], ps_rs[:])
                    nc.vector.tensor_mul(outtn[h][:, c * 512:(c + 1) * 512],
                                         ps_o[:], rcp[:])

                # chunk groups (mb, qs, npair): a = groups with qs < 512 of
                # stripes 0-3 (cover attnv c=0), b = the rest
                a_set = [(0, 0, 2), (1, 0, 2), (2, 256, 1), (3, 256, 1)]
                b_set = [(0, 512, 2), (1, 512, 2), (2, 512, 2), (3, 512, 2),
                         (4, 512, 2), (5, 512, 2), (6, 768, 1), (7, 768, 1)]

                qrts = {0: (qrt0, qc0)}
                wq_dma(1)
                for h in range(NH):
                    make_etiles(h)
                    qrt, qc = qrts[h]
                    wq_dma(h + 2)
                    # Qproj(h+1) first: no dependency on head h's chains, so
                    # it covers the tail of head h-1's vector pipeline.
                    qt_n = qproj(h + 1) if h + 1 < NH else None
                    if h > 0:
                        attnv_half(h - 1, 1)
                    for g_ in a_set:
                        score_group(h, qrt, qc, *g_)
                    if qt_n is not None:
                        qrt_n = ph.tile([128, L], F32R, tag="qrt", bufs=2,
                                        name=f"qrt{h+1}")
                        qc_n = ph.tile([128, L], F32R, tag="qc", bufs=2,
                                       name=f"qc{h+1}")
                        qrts[h + 1] = (qrt_n, qc_n)
                        rope_q_half(h + 1, qt_n, qrt_n, qc_n, 0)
                    if h == NH - 1:
                        nc.sync.dma_start(wo_c0[:], wo[:, :, 0:512])
                    for i, g_ in enumerate(b_set):
                        if i == 1:
                            attnv_half(h, 0)
                        if i == 2 and qt_n is not None:
                            rope_q_half(h + 1, qt_n, qrt_n, qc_n, 1)
                        score_group(h, qrt, qc, *g_)
                    qrts.pop(h)

                # first epilogue block (heads 0-6) covers head 7's E tail,
                # then attnv(7,1); its hh=7 matmul lands in the epilogue.
                psy0 = p2("psy")
                for hh in range(NH - 1):
                    nc.tensor.matmul(psy0[:],
                                     outtn[hh][:, 0:128], wo_c0[:, hh, :],
                                     start=(hh == 0), stop=False)
                attnv_half(NH - 1, 1, use_sasb=True)

            # ---------------- epilogue: Wo projection ----------------
            with tc.tile_pool(name="ep", bufs=1) as pe:
                def ytile_out(psy, dc, lb):
                    yt = pe.tile([128, 512], F32, tag="ytile", bufs=3)
                    nc.scalar.copy(yt[:], psy[:])
                    nc.sync.dma_start(
                        y[lb * 128:(lb + 1) * 128, dc * 512:(dc + 1) * 512],
                        yt[:])

                # finish the lb=0 block started before attnv(7,1): lb=1's
                # heads 0-6 cover the outtn[7] norm latency.
                psy1 = p2("psy")
                for hh in range(NH - 1):
                    nc.tensor.matmul(psy1[:],
                                     outtn[hh][:, 128:256], wo_c0[:, hh, :],
                                     start=(hh == 0), stop=False)
                nc.tensor.matmul(psy0[:], outtn[NH - 1][:, 0:128],
                                 wo_c0[:, NH - 1, :], start=False, stop=True)
                ytile_out(psy0, 0, 0)
                nc.tensor.matmul(psy1[:], outtn[NH - 1][:, 128:256],
                                 wo_c0[:, NH - 1, :], start=False, stop=True)
                ytile_out(psy1, 0, 1)

                wo_ts = {0: wo_c0}
                for dc in range(4):
                    if dc + 1 < 4:
                        wo_n = pe.tile([128, NH, 512], BF16, tag="wo_c", bufs=2,
                                       name=f"wo_c{dc+1}")
                        nc.sync.dma_start(
                            wo_n[:], wo[:, :, (dc + 1) * 512:(dc + 2) * 512])
                        wo_ts[dc + 1] = wo_n
                    wo_t = wo_ts.pop(dc)
                    for lb in range(2 if dc == 0 else 0, NB):
                        psy = p2("psy")
                        for hh in range(NH):
                            nc.tensor.matmul(
                                psy[:],
                                outtn[hh][:, lb * 128:(lb + 1) * 128],
                                wo_t[:, hh, :],
                                start=(hh == 0), stop=(hh == NH - 1))
                        ytile_out(psy, dc, lb)

    nc.compile()
    return nc


def _host_prep(x, Wq, Wk, Wv, Wo, q_param, log_scale, cos, sin, mask):
    """Build the 8 per-core input maps."""
    x = np.asarray(x, np.float32)
    Wq = np.asarray(Wq, np.float32)
    Wk = np.asarray(Wk, np.float32)
    Wv = np.asarray(Wv, np.float32)
    Wo = np.asarray(Wo, np.float32)
    cos = np.asarray(cos, np.float32)[0]      # [L, H, 64]
    sin = np.asarray(sin, np.float32)[0]
    qp = np.asarray(q_param, np.float32).reshape(H)
    ls = np.asarray(log_scale, np.float32).reshape(H)
    mask = np.asarray(mask)

    p64 = np.arange(128) % 64
    # rope tables are identical across heads: use head 0
    cos_p = np.ascontiguousarray(cos[:, 0, :][:, p64].T)   # [128, L]
    sin_p = np.ascontiguousarray(sin[:, 0, :][:, p64].T)

    PM = np.zeros((128, 128), np.float32)
    for dp in range(128):
        base, r = (dp // 64) * 64, dp % 64
        if r < 32:
            PM[base + r + 32, dp] = -1.0
        else:
            PM[base + r - 32, dp] = 1.0
    SW = np.zeros((128, 128), np.float32)
    for dp in range(128):
        SW[(dp + 64) % 128, dp] = 1.0
    ONES = np.ones((128, 128), ml_dtypes.bfloat16)
    IDENT = np.eye(128, dtype=ml_dtypes.bfloat16)

    in_maps = []
    for core in range(8):
        b, g2 = core // 2, core % 2
        heads = list(range(g2 * NH, (g2 + 1) * NH))

        xhv = x[b].astype(ml_dtypes.bfloat16)

        wq_c = Wq[:, g2 * NH * 128:(g2 + 1) * NH * 128]
        wk_c = Wk[:, g2 * NKV * 128:(g2 + 1) * NKV * 128]
        wv_c = Wv[:, g2 * NKV * 128:(g2 + 1) * NKV * 128]
        wo_c = Wo[g2 * NH * 128:(g2 + 1) * NH * 128, :]

        wq_p = wq_c.reshape(16, 128, NH, 128).transpose(2, 1, 0, 3).copy()
        wk_p = wk_c.reshape(16, 128, NKV * 128).transpose(1, 0, 2).copy()
        wv_p = wv_c.reshape(16, 128, NKV * 128).transpose(1, 0, 2).copy()
        wo_p = wo_c.reshape(NH, 128, D).transpose(1, 0, 2).astype(ml_dtypes.bfloat16)

        mb = np.where(mask[b].reshape(NB, 128).T.astype(bool), 0.0, -1e9)
        mb = mb.astype(np.float32)

        cpr = -2.0 * np.tanh(qp[heads])               # per-head c'
        sq = np.sqrt(np.abs(cpr))
        sqc = np.tile(sq[None, :], (128, 1))
        sqc[:64, :] *= np.sign(cpr)[None, :]
        alp = np.tile((np.exp(ls[heads]) / HD)[None, :], (128, 1))

        in_maps.append({
            "xh": xhv,
            "wq": wq_p.astype(ml_dtypes.bfloat16),
            "wk": wk_p.astype(ml_dtypes.bfloat16),
            "wv": wv_p.astype(ml_dtypes.bfloat16), "wo": wo_p,
            "costab": cos_p, "sintab": sin_p,
            "maskb": mb, "sqrtc": sqc.astype(np.float32),
            "alpha": alp.astype(np.float32),
            "pmrot": PM, "pmswap": SW, "onesb": ONES, "identb": IDENT,
        })
    return in_maps


def kernel(**inputs):
    if "nc" not in _CACHED:
        _CACHED["nc"] = build_program()
    nc = _CACHED["nc"]
    in_maps = _host_prep(**inputs)
    res = run_bass_kernel_spmd(nc, in_maps, list(range(8))).results
    out = np.empty((B, L, D), np.float32)
    for b in range(B):
        out[b] = res[2 * b]["y"] + res[2 * b + 1]["y"]
    return out


# revision 5
# speedup vs baseline: 1.0539x; 1.0162x over previous
"""BivectorRotarySelfAttention TRN2 kernel, v3.

Sharding: 8 cores = 4 batches x 2 head-halves; host sums the two head-half
partial y's per batch.

v3 vs v2:
 - Score PSUM rings are 3-deep (SA/SB [128,512]x3) so the PE runs ~3 chunks
   ahead of the vector chain; all other PSUM users are [128,512] halves in a
   shared 2-deep ring (P2). 6+6+4 KB = 16 KB exactly.
 - Engine rebalance: rope t2/add and all raw-combines on Pool, 2 of 20 bs
   copies on DVE, rest on ACT.
 - Qproj+rope for head h+1 runs mid-scores(h); attnv halves are interleaved
   into the chunk stream so PE never waits on the exp chain.
 - V projection trails K by 4 ib-steps to match wv DMA arrival; wk DMA is
   issued before everything except nothing (first), consts after wv.
 - Epilogue prefetches wo and accumulates head 7 last.
"""
import sys
if '/opt/trn_rl_repo' not in sys.path:
    sys.path.insert(0, '/opt/trn_rl_repo')

import numpy as np
import ml_dtypes

import concourse.bass as bass
import concourse.mybir as mybir
import concourse.tile as tile
from concourse import bacc
from concourse.bass_utils import run_bass_kernel_spmd

F32 = mybir.dt.float32
F32R = mybir.dt.float32r
BF16 = mybir.dt.bfloat16

B, L, D, H, HKV = 4, 1024, 2048, 16, 4
HD = D // H            # 128
HD2 = HD // 2          # 64
NH = 8                 # heads per core
NKV = 2                # kv heads per core
NB = L // 128          # 8 blocks of 128
AluOp = mybir.AluOpType
Act = mybir.ActivationFunctionType

_CACHED = {}


def _stripe_qlo(mb):
    # stripe mb covers q in [qlo, L) in 256-wide chunks; odd stripes start one
    # 128-block early (the extra region is causal-masked to zero).
    return 128 * (mb - (mb % 2))


def build_program():
    nc = bacc.Bacc("TRN2", target_bir_lowering=False, debug=False)

    # ---- dram params (per-core shapes) ----
    xh = nc.declare_dram_parameter("xh", [L, D], BF16, isOutput=False)
    wq = nc.declare_dram_parameter("wq", [NH, 128, 16, 128], BF16, isOutput=False)
    wk = nc.declare_dram_parameter("wk", [128, 16, NKV * 128], BF16, isOutput=False)
    wv = nc.declare_dram_parameter("wv", [128, 16, NKV * 128], BF16, isOutput=False)
    wo = nc.declare_dram_parameter("wo", [128, NH, D], BF16, isOutput=False)
    costab = nc.declare_dram_parameter("costab", [128, L], F32, isOutput=False)
    sintab = nc.declare_dram_parameter("sintab", [128, L], F32, isOutput=False)
    maskb = nc.declare_dram_parameter("maskb", [128, NB], F32, isOutput=False)
    sqrtc = nc.declare_dram_parameter("sqrtc", [128, NH], F32, isOutput=False)
    alpha = nc.declare_dram_parameter("alpha", [128, NH], F32, isOutput=False)
    pmrot = nc.declare_dram_parameter("pmrot", [128, 128], F32R, isOutput=False)
    pmswap = nc.declare_dram_parameter("pmswap", [128, 128], F32R, isOutput=False)
    onesb = nc.declare_dram_parameter("onesb", [128, 128], BF16, isOutput=False)
    identb = nc.declare_dram_parameter("identb", [128, 128], BF16, isOutput=False)
    y = nc.declare_dram_parameter("y", [L, D], F32, isOutput=True)

    with tile.TileContext(nc) as tc:
        with (
            tc.tile_pool(name="persist", bufs=1) as pp,
            tc.tile_pool(name="psum", bufs=1, space="PSUM") as psp,
        ):
            # ---- persistent SBUF ----
            xt = [pp.tile([128, L], BF16, tag=f"xt{ib}", name=f"xt{ib}")
                  for ib in range(16)]
            krt = [pp.tile([128, L], F32R, tag=f"krt{g}", name=f"krt{g}")
                   for g in range(NKV)]
            kswap = [pp.tile([128, L], F32R, tag=f"ksw{g}", name=f"ksw{g}")
                     for g in range(NKV)]
            vblk = [pp.tile([128, 128], BF16, tag=f"vb{i}", name=f"vb{i}")
                    for i in range(NKV * NB)]
            outtn = [pp.tile([128, L], BF16, tag=f"ot{h}", name=f"ot{h}")
                     for h in range(NH)]
            wo_c0 = pp.tile([128, NH, 512], BF16, tag="wo_c0", name="wo_c0")
            consts = {}

            def sa(name):
                return psp.tile([128, 512], F32, tag="SA", bufs=3, name=name)

            def sb(name):
                return psp.tile([128, 512], F32, tag="SB", bufs=3, name=name)

            def p2(name, shape=None, dtype=F32):
                return psp.tile(shape or [128, 512], dtype, tag="P2", bufs=2,
                                name=name)

            # ---------------- prologue ----------------
            with tc.tile_pool(name="pro", bufs=1) as ppro:
                wk_t = ppro.tile([128, 16, NKV * 128], BF16, tag="wk")
                wv_t = ppro.tile([128, 16, NKV * 128], BF16, tag="wv")

                # DMA order: wk (halved), x0, wv, rest of x, consts.
                nc.sync.dma_start(wk_t[:, 0:8], wk[:, 0:8])
                nc.sync.dma_start(wk_t[:, 8:16], wk[:, 8:16])
                def xdma(ib):
                    nc.sync.dma_start_transpose(
                        xt[ib][:], xh[:, ib * 128:(ib + 1) * 128])

                wq_t0 = ppro.tile([128, 16, 128], BF16, tag="wq0", name="wq_t0")
                xdma(0)
                nc.sync.dma_start(wv_t[:], wv[:])
                for ib in range(1, 16):
                    xdma(ib)
                nc.sync.dma_start(wq_t0[:], wq[0])
                for nm, src, dt_ in [("pmrot", pmrot, F32R),
                                     ("costab", costab, F32),
                                     ("sintab", sintab, F32),
                                     ("identb", identb, BF16),
                                     ("pmswap", pmswap, F32R),
                                     ("onesb", onesb, BF16),
                                     ("maskb", maskb, F32),
                                     ("sqrtc", sqrtc, F32),
                                     ("alpha", alpha, F32)]:
                    t = pp.tile(list(src.shape), dt_, tag=nm, name=nm)
                    nc.sync.dma_start(t[:], src[:])
                    consts[nm] = t

                # K/V projection accumulators: [128,512] halves.
                psk = [[sa("psk0a"), sa("psk0b")], [sb("psk1a"), sb("psk1b")]]
                psv = [[sa("psv0a"), sb("psv0b")], [p2("psv1a"), p2("psv1b")]]

                VOFF = 3
                for step in range(16 + VOFF):
                    if step < 16:
                        ib = step
                        for g in range(NKV):
                            for c in range(2):
                                nc.tensor.matmul(
                                    psk[g][c][:],
                                    wk_t[:, ib, g * 128:(g + 1) * 128],
                                    xt[ib][:, c * 512:(c + 1) * 512],
                                    start=(ib == 0), stop=(ib == 15))
                    if step >= VOFF:
                        ib = step - VOFF
                        for g in range(NKV):
                            for c in range(2):
                                nc.tensor.matmul(
                                    psv[g][c][:],
                                    wv_t[:, ib, g * 128:(g + 1) * 128],
                                    xt[ib][:, c * 512:(c + 1) * 512],
                                    start=(ib == 0), stop=(ib == 15))

                # PSUM->SBUF copies: v-g1 first (frees the P2 slots that
                # Qproj(0) needs), then interleaved kt/vt.
                vt_s = [ppro.tile([128, L], BF16, tag=f"vt_s{g}", name=f"vt{g}")
                        for g in range(NKV)]
                kt_s = [ppro.tile([128, L], F32R, tag=f"kt_s{g}", name=f"kt{g}")
                        for g in range(NKV)]
                for c in range(2):
                    nc.scalar.copy(vt_s[1][:, c * 512:(c + 1) * 512], psv[1][c][:])
                for g in range(NKV):
                    for c in range(2):
                        nc.scalar.copy(kt_s[g][:, c * 512:(c + 1) * 512],
                                       psk[g][c][:])
                for c in range(2):
                    nc.scalar.copy(vt_s[0][:, c * 512:(c + 1) * 512], psv[0][c][:])

                # Qproj(0): no dependency on the copies above except P2 slots;
                # covers the kt/vt copy chain on PE.
                qt_s0 = ppro.tile([128, L], F32R, tag="qt_s0", name="qt_s0")
                for cc in range(2):
                    psq = p2("psq0")
                    for ib in range(16):
                        nc.tensor.matmul(psq[:], wq_t0[:, ib, :],
                                         xt[ib][:, cc * 512:(cc + 1) * 512],
                                         start=(ib == 0), stop=(ib == 15))
                    nc.vector.tensor_copy(qt_s0[:, cc * 512:(cc + 1) * 512],
                                          psq[:])

                # rope-q0 chain starts immediately (DVE halves); the k-rope /
                # v-transpose / k-swap PE work below covers it.
                qrt0 = pp.tile([128, L], F32R, tag="qrt0", name="qrt0")
                qc0 = pp.tile([128, L], F32R, tag="qc0", name="qc0")
                for cc in range(2):
                    hs = slice(cc * 512, (cc + 1) * 512)
                    t1q = ppro.tile([128, 512], F32, tag="rq0", bufs=2)
                    t2q = ppro.tile([128, 512], F32, tag="rq0", bufs=2)
                    psr2 = p2("psr_q0")
                    nc.tensor.matmul(psr2[:], consts["pmrot"][:], qt_s0[:, hs])
                    nc.vector.tensor_mul(t1q[:], psr2[:], consts["sintab"][:, hs])
                    nc.vector.tensor_mul(t2q[:], qt_s0[:, hs].bitcast(F32),
                                         consts["costab"][:, hs])
                    nc.vector.tensor_add(qrt0[:, hs], t1q[:], t2q[:])
                    nc.vector.tensor_scalar_mul(qc0[:, hs],
                                                qrt0[:, hs].bitcast(F32),
                                                consts["sqrtc"][:, 0:1])

                # k rope rotate-half part (SA slots freed by the kt copies)
                for g in range(NKV):
                    t1 = ppro.tile([128, L], F32, tag="rtmp", bufs=4)
                    t2 = ppro.tile([128, L], F32, tag="rtmp", bufs=4)
                    for c in range(2):
                        psr = sa(f"psr_k{c}")
                        nc.tensor.matmul(psr[:],
                                         consts["pmrot"][:],
                                         kt_s[g][:, c * 512:(c + 1) * 512])
                        nc.vector.tensor_mul(
                            t1[:, c * 512:(c + 1) * 512], psr[:],
                            consts["sintab"][:, c * 512:(c + 1) * 512])
                    nc.gpsimd.tensor_mul(t2[:], kt_s[g][:].bitcast(F32),
                                         consts["costab"][:])
                    nc.vector.tensor_add(krt[g][:], t1[:], t2[:])

                # v transpose to [m, d] blocks
                for g in range(NKV):
                    for mb in range(NB):
                        pv = p2("pv", shape=[128, 128], dtype=BF16)
                        nc.tensor.transpose(pv[:], vt_s[g][:, mb * 128:(mb + 1) * 128],
                                            consts["identb"][:])
                        nc.vector.tensor_copy(vblk[g * NB + mb][:], pv[:])

                # k swap perms (krt chains complete under the ops above)
                for g in range(NKV):
                    for c in range(2):
                        psw = sb(f"psw_k{g}{c}")
                        nc.tensor.matmul(psw[:],
                                         consts["pmswap"][:],
                                         krt[g][:, c * 512:(c + 1) * 512])
                        nc.scalar.copy(kswap[g][:, c * 512:(c + 1) * 512],
                                       psw[:])

            # ---------------- head loop ----------------
            with tc.tile_pool(name="hl", bufs=1) as ph:
                etiles = {}

                wq_tiles = {}

                def wq_dma(h):
                    if h < NH:
                        wq_t = ph.tile([128, 16, 128], BF16, tag="wq_h", bufs=2,
                                       name=f"wq_t{h}")
                        nc.sync.dma_start(wq_t[:], wq[h])
                        wq_tiles[h] = wq_t

                def qproj(h):
                    wq_t = wq_tiles.pop(h)
                    qt_s = ph.tile([128, L], F32R, tag="qt_s", bufs=2)
                    for cc in range(2):
                        psq = p2("psq")
                        for ib in range(16):
                            nc.tensor.matmul(
                                psq[:],
                                wq_t[:, ib, :],
                                xt[ib][:, cc * 512:(cc + 1) * 512],
                                start=(ib == 0), stop=(ib == 15))
                        nc.vector.tensor_copy(qt_s[:, cc * 512:(cc + 1) * 512],
                                              psq[:])
                    return qt_s

                def rope_q_half(h, qt_s, qrt, qc, cc):
                    """one 512-half of q-rope + c'-scaled copy, on DVE"""
                    hs = slice(cc * 512, (cc + 1) * 512)
                    t1 = ph.tile([128, 512], F32, tag="qtmp", bufs=2)
                    t2 = ph.tile([128, 512], F32, tag="qtmp", bufs=2)
                    psr2 = p2("psr_q")
                    nc.tensor.matmul(psr2[:], consts["pmrot"][:], qt_s[:, hs])
                    nc.vector.tensor_mul(t1[:], psr2[:], consts["sintab"][:, hs])
                    nc.vector.tensor_mul(t2[:], qt_s[:, hs].bitcast(F32),
                                         consts["costab"][:, hs])
                    nc.vector.tensor_add(qrt[:, hs], t1[:], t2[:])
                    nc.vector.tensor_scalar_mul(qc[:, hs], qrt[:, hs].bitcast(F32),
                                                consts["sqrtc"][:, h:h + 1])

                def make_etiles(h):
                    ets = []
                    for mb in range(NB):
                        qlo = _stripe_qlo(mb)
                        et = ph.tile([128, L - qlo], BF16, tag=f"esc{mb}", bufs=2,
                                     name=f"esc_h{mb}")
                        ets.append(et)
                    etiles[h] = ets

                def score_group(h, qrt, qc, mb, qs, npair, bs_dve=False):
                    """npair chunks (1 or 2) of stripe mb starting at qs; one
                    fused exp over the pair."""
                    g = h // 4
                    qlo = _stripe_qlo(mb)
                    kb = slice(mb * 128, (mb + 1) * 128)
                    ets = etiles[h]
                    raw = ph.tile([128, 512], F32, tag="raw", bufs=3)
                    for j in range(npair):
                        cqs = qs + 256 * j
                        cqe = cqs + 256
                        psA = sa("psA")
                        psB = sb("psB")
                        nc.tensor.matmul(psA[:, 0:256], krt[g][0:64, kb],
                                         qrt[0:64, cqs:cqe])
                        nc.tensor.matmul(psA[:, 256:512], kswap[g][0:64, kb],
                                         qc[0:64, cqs:cqe])
                        nc.tensor.matmul(psB[:, 0:256], krt[g][64:128, kb],
                                         qrt[64:128, cqs:cqe])
                        nc.tensor.matmul(psB[:, 256:512], kswap[g][64:128, kb],
                                         qc[64:128, cqs:cqe])
                        bs = ph.tile([128, 512], F32, tag="bs", bufs=3)
                        if bs_dve and j == 0:
                            nc.vector.tensor_copy(bs[:], psB[:])
                        else:
                            nc.scalar.copy(bs[:], psB[:])
                        tp = ph.tile([128, 512], F32, tag="tprod", bufs=3)
                        nc.vector.tensor_mul(tp[:], psA[:], bs[:])
                        nc.gpsimd.tensor_add(raw[:, 256 * j:256 * (j + 1)],
                                             tp[:, 0:256], tp[:, 256:512])
                    w = 256 * npair
                    esl = ets[mb][:, qs - qlo: qs - qlo + w]
                    nc.scalar.activation(esl, raw[:, 0:w], Act.Exp,
                                         bias=consts["maskb"][:, mb:mb + 1],
                                         scale=consts["alpha"][:, h:h + 1])
                    if qs == qlo:
                        # causal mask on the diagonal 256 cols:
                        # keep where (qlo + col) - (128*mb + part) >= 0
                        nc.gpsimd.affine_select(
                            ets[mb][:, 0:256], ets[mb][:, 0:256],
                            pattern=[[1, 256]], compare_op=AluOp.is_ge,
                            fill=0.0, base=qlo - 128 * mb,
                            channel_multiplier=-1)

                def attnv_half(h, c, use_sasb=True):
                    g = h // 4
                    ets = etiles[h]
                    ps_o, ps_rs = sa("ps_o"), sb("ps_rs")
                    mbs = [mb for mb in range(NB) if 128 * mb < 512 * (c + 1)]
                    for i, mb in enumerate(mbs):
                        qlo = _stripe_qlo(mb)
                        os_ = max(512 * c, 128 * mb)
                        oe = 512 * (c + 1)
                        esl = ets[mb][:, os_ - qlo: oe - qlo]
                        st, sp = (i == 0), (i == len(mbs) - 1)
                        nc.tensor.matmul(ps_o[:, os_ - 512 * c: oe - 512 * c],
                                         vblk[g * NB + mb][:], esl,
                                         start=st, stop=sp)
                        nc.tensor.matmul(ps_rs[:, os_ - 512 * c: oe - 512 * c],
                                         consts["onesb"][:], esl,
                                         start=st, stop=sp)
                    rcp = ph.tile([128, 512], F32, tag="rcp", bufs=2)
                    nc.vector.reciprocal_approx_fast(rcp[:], ps_rs[:])
                    nc.vector.tensor_mul(outtn[h][:, c * 512:(c + 1) * 512],
                                         ps_o[:], rcp[:])

                # chunk groups (mb, qs, npair): a = groups with qs < 512 of
                # stripes 0-3 (cover attnv c=0), b = the rest
                a_set = [(0, 0, 2), (1, 0, 2), (2, 256, 1), (3, 256, 1)]
                b_set = [(0, 512, 2), (1, 512, 2), (2, 512, 2), (3, 512, 2),
                         (4, 512, 2), (5, 512, 2), (6, 768, 1), (7, 768, 1)]

                qrts = {0: (qrt0, qc0)}
                wq_dma(1)
                for h in range(NH):
                    make_etiles(h)
                    qrt, qc = qrts[h]
                    wq_dma(h + 2)
                    # Qproj(h+1) first: no dependency on head h's chains, so
                    # it covers the tail of head h-1's vector pipeline.
                    qt_n = qproj(h + 1) if h + 1 < NH else None
                    if h > 0:
                        attnv_half(h - 1, 1)
                    for g_ in a_set:
                        score_group(h, qrt, qc, *g_)
                    if qt_n is not None:
                        qrt_n = ph.tile([128, L], F32R, tag="qrt", bufs=2,
                                        name=f"qrt{h+1}")
                        qc_n = ph.tile([128, L], F32R, tag="qc", bufs=2,
                                       name=f"qc{h+1}")
                        qrts[h + 1] = (qrt_n, qc_n)
                        rope_q_half(h + 1, qt_n, qrt_n, qc_n, 0)
                    if h == NH - 1:
                        nc.sync.dma_start(wo_c0[:], wo[:, :, 0:512])
                    for i, g_ in enumerate(b_set):
                        if i == 1:
                            attnv_half(h, 0)
                        if i == 2 and qt_n is not None:
                            rope_q_half(h + 1, qt_n, qrt_n, qc_n, 1)
                        score_group(h, qrt, qc, *g_)
                    qrts.pop(h)

                # first epilogue block (heads 0-6) covers head 7's E tail,
                # then attnv(7,1); its hh=7 matmul lands in the epilogue.
                psy0 = p2("psy")
                for hh in range(NH - 1):
                    nc.tensor.matmul(psy0[:],
                                     outtn[hh][:, 0:128], wo_c0[:, hh, :],
                                     start=(hh == 0), stop=False)
                attnv_half(NH - 1, 1, use_sasb=True)

            # ---------------- epilogue: Wo projection ----------------
            with tc.tile_pool(name="ep", bufs=1) as pe:
                def ytile_out(psy, dc, lb):
                    yt = pe.tile([128, 512], F32, tag="ytile", bufs=3)
                    nc.scalar.copy(yt[:], psy[:])
                    nc.sync.dma_start(
                        y[lb * 128:(lb + 1) * 128, dc * 512:(dc + 1) * 512],
                        yt[:])

                # finish the lb=0 block started before attnv(7,1): lb=1's
                # heads 0-6 cover the outtn[7] norm latency.
                psy1 = p2("psy")
                for hh in range(NH - 1):
                    nc.tensor.matmul(psy1[:],
                                     outtn[hh][:, 128:256], wo_c0[:, hh, :],
                                     start=(hh == 0), stop=False)
                nc.tensor.matmul(psy0[:], outtn[NH - 1][:, 0:128],
                                 wo_c0[:, NH - 1, :], start=False, stop=True)
                ytile_out(psy0, 0, 0)
                nc.tensor.matmul(psy1[:], outtn[NH - 1][:, 128:256],
                                 wo_c0[:, NH - 1, :], start=False, stop=True)
                ytile_out(psy1, 0, 1)

                wo_ts = {0: wo_c0}
                for dc in range(4):
                    if dc + 1 < 4:
                        wo_n = pe.tile([128, NH, 512], BF16, tag="wo_c", bufs=2,
                                       name=f"wo_c{dc+1}")
                        nc.sync.dma_start(
                            wo_n[:], wo[:, :, (dc + 1) * 512:(dc + 2) * 512])
                        wo_ts[dc + 1] = wo_n
                    wo_t = wo_ts.pop(dc)
                    for lb in range(2 if dc == 0 else 0, NB):
                        psy = p2("psy")
                        for hh in range(NH):
                            nc.tensor.matmul(
                                psy[:],
                                outtn[hh][:, lb * 128:(lb + 1) * 128],
                                wo_t[:, hh, :],
                                start=(hh == 0), stop=(hh == NH - 1))
                        ytile_out(psy, dc, lb)

    nc.compile()
    return nc


def _host_prep(x, Wq, Wk, Wv, Wo, q_param, log_scale, cos, sin, mask):
    """Build the 8 per-core input maps."""
    x = np.asarray(x, np.float32)
    Wq = np.asarray(Wq, np.float32)
    Wk = np.asarray(Wk, np.float32)
    Wv = np.asarray(Wv, np.float32)
    Wo = np.asarray(Wo, np.float32)
    cos = np.asarray(cos, np.float32)[0]      # [L, H, 64]
    sin = np.asarray(sin, np.float32)[0]
    qp = np.asarray(q_param, np.float32).reshape(H)
    ls = np.asarray(log_scale, np.float32).reshape(H)
    mask = np.asarray(mask)

    p64 = np.arange(128) % 64
    # rope tables are identical across heads: use head 0
    cos_p = np.ascontiguousarray(cos[:, 0, :][:, p64].T)   # [128, L]
    sin_p = np.ascontiguousarray(sin[:, 0, :][:, p64].T)

    PM = np.zeros((128, 128), np.float32)
    for dp in range(128):
        base, r = (dp // 64) * 64, dp % 64
        if r < 32:
            PM[base + r + 32, dp] = -1.0
        else:
            PM[base + r - 32, dp] = 1.0
    SW = np.zeros((128, 128), np.float32)
    for dp in range(128):
        SW[(dp + 64) % 128, dp] = 1.0
    ONES = np.ones((128, 128), ml_dtypes.bfloat16)
    IDENT = np.eye(128, dtype=ml_dtypes.bfloat16)

    in_maps = []
    for core in range(8):
        b, g2 = core // 2, core % 2
        heads = list(range(g2 * NH, (g2 + 1) * NH))

        xhv = x[b].astype(ml_dtypes.bfloat16)

        wq_c = Wq[:, g2 * NH * 128:(g2 + 1) * NH * 128]
        wk_c = Wk[:, g2 * NKV * 128:(g2 + 1) * NKV * 128]
        wv_c = Wv[:, g2 * NKV * 128:(g2 + 1) * NKV * 128]
        wo_c = Wo[g2 * NH * 128:(g2 + 1) * NH * 128, :]

        wq_p = wq_c.reshape(16, 128, NH, 128).transpose(2, 1, 0, 3).copy()
        wk_p = wk_c.reshape(16, 128, NKV * 128).transpose(1, 0, 2).copy()
        wv_p = wv_c.reshape(16, 128, NKV * 128).transpose(1, 0, 2).copy()
        wo_p = wo_c.reshape(NH, 128, D).transpose(1, 0, 2).astype(ml_dtypes.bfloat16)

        mb = np.where(mask[b].reshape(NB, 128).T.astype(bool), 0.0, -1e9)
        mb = mb.astype(np.float32)

        cpr = -2.0 * np.tanh(qp[heads])               # per-head c'
        sq = np.sqrt(np.abs(cpr))
        sqc = np.tile(sq[None, :], (128, 1))
        sqc[:64, :] *= np.sign(cpr)[None, :]
        alp = np.tile((np.exp(ls[heads]) / HD)[None, :], (128, 1))

        in_maps.append({
            "xh": xhv,
            "wq": wq_p.astype(ml_dtypes.bfloat16),
            "wk": wk_p.astype(ml_dtypes.bfloat16),
            "wv": wv_p.astype(ml_dtypes.bfloat16), "wo": wo_p,
            "costab": cos_p, "sintab": sin_p,
            "maskb": mb, "sqrtc": sqc.astype(np.float32),
            "alpha": alp.astype(np.float32),
            "pmrot": PM, "pmswap": SW, "onesb": ONES, "identb": IDENT,
        })
    return in_maps


def kernel(**inputs):
    if "nc" not in _CACHED:
        _CACHED["nc"] = build_program()
    nc = _CACHED["nc"]
    in_maps = _host_prep(**inputs)
    res = run_bass_kernel_spmd(nc, in_maps, list(range(8))).results
    out = np.empty((B, L, D), np.float32)
    for b in range(B):
        out[b] = res[2 * b]["y"] + res[2 * b + 1]["y"]
    return out


# revision 6
# speedup vs baseline: 1.0759x; 1.0208x over previous
"""BivectorRotarySelfAttention TRN2 kernel, v3.

Sharding: 8 cores = 4 batches x 2 head-halves; host sums the two head-half
partial y's per batch.

v3 vs v2:
 - Score PSUM rings are 3-deep (SA/SB [128,512]x3) so the PE runs ~3 chunks
   ahead of the vector chain; all other PSUM users are [128,512] halves in a
   shared 2-deep ring (P2). 6+6+4 KB = 16 KB exactly.
 - Engine rebalance: rope t2/add and all raw-combines on Pool, 2 of 20 bs
   copies on DVE, rest on ACT.
 - Qproj+rope for head h+1 runs mid-scores(h); attnv halves are interleaved
   into the chunk stream so PE never waits on the exp chain.
 - V projection trails K by 4 ib-steps to match wv DMA arrival; wk DMA is
   issued before everything except nothing (first), consts after wv.
 - Epilogue prefetches wo and accumulates head 7 last.
"""
import sys
if '/opt/trn_rl_repo' not in sys.path:
    sys.path.insert(0, '/opt/trn_rl_repo')

import numpy as np
import ml_dtypes

import concourse.bass as bass
import concourse.mybir as mybir
import concourse.tile as tile
from concourse import bacc
from concourse.bass_utils import run_bass_kernel_spmd

F32 = mybir.dt.float32
F32R = mybir.dt.float32r
BF16 = mybir.dt.bfloat16

B, L, D, H, HKV = 4, 1024, 2048, 16, 4
HD = D // H            # 128
HD2 = HD // 2          # 64
NH = 8                 # heads per core
NKV = 2                # kv heads per core
NB = L // 128          # 8 blocks of 128
AluOp = mybir.AluOpType
Act = mybir.ActivationFunctionType

_CACHED = {}


def _stripe_qlo(mb):
    # stripe mb covers q in [qlo, L) in 256-wide chunks; odd stripes start one
    # 128-block early (the extra region is causal-masked to zero).
    return 128 * (mb - (mb % 2))


def build_program():
    nc = bacc.Bacc("TRN2", target_bir_lowering=False, debug=False)

    # ---- dram params (per-core shapes) ----
    xh = nc.declare_dram_parameter("xh", [L, D], BF16, isOutput=False)
    wq = nc.declare_dram_parameter("wq", [NH, 128, 16, 128], BF16, isOutput=False)
    wk = nc.declare_dram_parameter("wk", [128, 16, NKV * 128], BF16, isOutput=False)
    wv = nc.declare_dram_parameter("wv", [128, 16, NKV * 128], BF16, isOutput=False)
    wo = nc.declare_dram_parameter("wo", [128, NH, D], BF16, isOutput=False)
    costab = nc.declare_dram_parameter("costab", [128, L], F32, isOutput=False)
    sintab = nc.declare_dram_parameter("sintab", [128, L], F32, isOutput=False)
    maskb = nc.declare_dram_parameter("maskb", [128, NB], F32, isOutput=False)
    sqrtc = nc.declare_dram_parameter("sqrtc", [128, NH], F32, isOutput=False)
    alpha = nc.declare_dram_parameter("alpha", [128, NH], F32, isOutput=False)
    pmrot = nc.declare_dram_parameter("pmrot", [128, 128], F32R, isOutput=False)
    pmswap = nc.declare_dram_parameter("pmswap", [128, 128], F32R, isOutput=False)
    onesb = nc.declare_dram_parameter("onesb", [128, 128], BF16, isOutput=False)
    identb = nc.declare_dram_parameter("identb", [128, 128], BF16, isOutput=False)
    y = nc.declare_dram_parameter("y", [L, D], F32, isOutput=True)

    with tile.TileContext(nc) as tc:
        with (
            tc.tile_pool(name="persist", bufs=1) as pp,
            tc.tile_pool(name="psum", bufs=1, space="PSUM") as psp,
        ):
            # ---- persistent SBUF ----
            xt = [pp.tile([128, L], BF16, tag=f"xt{ib}", name=f"xt{ib}")
                  for ib in range(16)]
            krt = [pp.tile([128, L], F32R, tag=f"krt{g}", name=f"krt{g}")
                   for g in range(NKV)]
            kswap = [pp.tile([128, L], F32R, tag=f"ksw{g}", name=f"ksw{g}")
                     for g in range(NKV)]
            vblk = [pp.tile([128, 128], BF16, tag=f"vb{i}", name=f"vb{i}")
                    for i in range(NKV * NB)]
            outtn = [pp.tile([128, L], BF16, tag=f"ot{h}", name=f"ot{h}")
                     for h in range(NH)]
            wo_c0 = pp.tile([128, NH, 512], BF16, tag="wo_c0", name="wo_c0")
            consts = {}

            def sa(name):
                return psp.tile([128, 512], F32, tag="SA", bufs=3, name=name)

            def sb(name):
                return psp.tile([128, 512], F32, tag="SB", bufs=3, name=name)

            def p2(name, shape=None, dtype=F32):
                return psp.tile(shape or [128, 512], dtype, tag="P2", bufs=2,
                                name=name)

            # ---------------- prologue ----------------
            with tc.tile_pool(name="pro", bufs=1) as ppro:
                wk_t = ppro.tile([128, 16, NKV * 128], BF16, tag="wk")
                wv_t = ppro.tile([128, 16, NKV * 128], BF16, tag="wv")

                # DMA order: wk (halved), x0, wv, rest of x, consts.
                nc.sync.dma_start(wk_t[:, 0:8], wk[:, 0:8])
                nc.sync.dma_start(wk_t[:, 8:16], wk[:, 8:16])
                def xdma(ib, eng=None):
                    (eng or nc.sync).dma_start_transpose(
                        xt[ib][:], xh[:, ib * 128:(ib + 1) * 128])

                wq_t0 = ppro.tile([128, 16, 128], BF16, tag="wq0", name="wq_t0")
                xdma(0)
                nc.sync.dma_start(wv_t[:], wv[:])
                for ib in range(1, 16):
                    xdma(ib)
                nc.sync.dma_start(wq_t0[:], wq[0])
                for nm, src, dt_ in [("pmrot", pmrot, F32R),
                                     ("costab", costab, F32),
                                     ("sintab", sintab, F32),
                                     ("identb", identb, BF16),
                                     ("pmswap", pmswap, F32R),
                                     ("onesb", onesb, BF16),
                                     ("maskb", maskb, F32),
                                     ("sqrtc", sqrtc, F32),
                                     ("alpha", alpha, F32)]:
                    t = pp.tile(list(src.shape), dt_, tag=nm, name=nm)
                    nc.sync.dma_start(t[:], src[:])
                    consts[nm] = t

                # K/V projection accumulators: [128,512] halves.
                psk = [[sa("psk0a"), sa("psk0b")], [sb("psk1a"), sb("psk1b")]]
                psv = [[sa("psv0a"), sb("psv0b")], [p2("psv1a"), p2("psv1b")]]

                VOFF = 3
                for step in range(16 + VOFF):
                    if step < 16:
                        ib = step
                        for g in range(NKV):
                            for c in range(2):
                                nc.tensor.matmul(
                                    psk[g][c][:],
                                    wk_t[:, ib, g * 128:(g + 1) * 128],
                                    xt[ib][:, c * 512:(c + 1) * 512],
                                    start=(ib == 0), stop=(ib == 15))
                    if step >= VOFF:
                        ib = step - VOFF
                        for g in range(NKV):
                            for c in range(2):
                                nc.tensor.matmul(
                                    psv[g][c][:],
                                    wv_t[:, ib, g * 128:(g + 1) * 128],
                                    xt[ib][:, c * 512:(c + 1) * 512],
                                    start=(ib == 0), stop=(ib == 15))

                # PSUM->SBUF copies: v-g1 first (frees the P2 slots that
                # Qproj(0) needs), then interleaved kt/vt.
                vt_s = [ppro.tile([128, L], BF16, tag=f"vt_s{g}", name=f"vt{g}")
                        for g in range(NKV)]
                kt_s = [ppro.tile([128, L], F32R, tag=f"kt_s{g}", name=f"kt{g}")
                        for g in range(NKV)]
                for c in range(2):
                    nc.scalar.copy(vt_s[1][:, c * 512:(c + 1) * 512], psv[1][c][:])
                for g in range(NKV):
                    for c in range(2):
                        nc.scalar.copy(kt_s[g][:, c * 512:(c + 1) * 512],
                                       psk[g][c][:])
                for c in range(2):
                    nc.scalar.copy(vt_s[0][:, c * 512:(c + 1) * 512], psv[0][c][:])

                # Qproj(0): no dependency on the copies above except P2 slots;
                # covers the kt/vt copy chain on PE.
                qt_s0 = ppro.tile([128, L], F32R, tag="qt_s0", name="qt_s0")
                for cc in range(2):
                    psq = p2("psq0")
                    for ib in range(16):
                        nc.tensor.matmul(psq[:], wq_t0[:, ib, :],
                                         xt[ib][:, cc * 512:(cc + 1) * 512],
                                         start=(ib == 0), stop=(ib == 15))
                    nc.vector.tensor_copy(qt_s0[:, cc * 512:(cc + 1) * 512],
                                          psq[:])

                # rope-q0 chain starts immediately (DVE halves); the k-rope /
                # v-transpose / k-swap PE work below covers it.
                qrt0 = pp.tile([128, L], F32R, tag="qrt0", name="qrt0")
                qc0 = pp.tile([128, L], F32R, tag="qc0", name="qc0")
                for cc in range(2):
                    hs = slice(cc * 512, (cc + 1) * 512)
                    t1q = ppro.tile([128, 512], F32, tag="rq0", bufs=2)
                    t2q = ppro.tile([128, 512], F32, tag="rq0", bufs=2)
                    psr2 = p2("psr_q0")
                    nc.tensor.matmul(psr2[:], consts["pmrot"][:], qt_s0[:, hs])
                    nc.vector.tensor_mul(t1q[:], psr2[:], consts["sintab"][:, hs])
                    nc.vector.tensor_mul(t2q[:], qt_s0[:, hs].bitcast(F32),
                                         consts["costab"][:, hs])
                    nc.vector.tensor_add(qrt0[:, hs], t1q[:], t2q[:])
                    nc.vector.tensor_scalar_mul(qc0[:, hs],
                                                qrt0[:, hs].bitcast(F32),
                                                consts["sqrtc"][:, 0:1])

                # k rope rotate-half part (SA slots freed by the kt copies)
                for g in range(NKV):
                    t1 = ppro.tile([128, L], F32, tag="rtmp", bufs=4)
                    t2 = ppro.tile([128, L], F32, tag="rtmp", bufs=4)
                    for c in range(2):
                        psr = sa(f"psr_k{c}")
                        nc.tensor.matmul(psr[:],
                                         consts["pmrot"][:],
                                         kt_s[g][:, c * 512:(c + 1) * 512])
                        nc.vector.tensor_mul(
                            t1[:, c * 512:(c + 1) * 512], psr[:],
                            consts["sintab"][:, c * 512:(c + 1) * 512])
                    nc.gpsimd.tensor_mul(t2[:], kt_s[g][:].bitcast(F32),
                                         consts["costab"][:])
                    nc.vector.tensor_add(krt[g][:], t1[:], t2[:])

                # v transpose to [m, d] blocks
                for g in range(NKV):
                    for mb in range(NB):
                        pv = p2("pv", shape=[128, 128], dtype=BF16)
                        nc.tensor.transpose(pv[:], vt_s[g][:, mb * 128:(mb + 1) * 128],
                                            consts["identb"][:])
                        nc.vector.tensor_copy(vblk[g * NB + mb][:], pv[:])

                # k swap perms (krt chains complete under the ops above)
                for g in range(NKV):
                    for c in range(2):
                        psw = sb(f"psw_k{g}{c}")
                        nc.tensor.matmul(psw[:],
                                         consts["pmswap"][:],
                                         krt[g][:, c * 512:(c + 1) * 512])
                        nc.scalar.copy(kswap[g][:, c * 512:(c + 1) * 512],
                                       psw[:])

            # ---------------- head loop ----------------
            with tc.tile_pool(name="hl", bufs=1) as ph:
                etiles = {}

                wq_tiles = {}

                def wq_dma(h):
                    if h < NH:
                        wq_t = ph.tile([128, 16, 128], BF16, tag="wq_h", bufs=2,
                                       name=f"wq_t{h}")
                        nc.sync.dma_start(wq_t[:], wq[h])
                        wq_tiles[h] = wq_t

                def qproj(h):
                    wq_t = wq_tiles.pop(h)
                    qt_s = ph.tile([128, L], F32R, tag="qt_s", bufs=2)
                    for cc in range(2):
                        psq = p2("psq")
                        for ib in range(16):
                            nc.tensor.matmul(
                                psq[:],
                                wq_t[:, ib, :],
                                xt[ib][:, cc * 512:(cc + 1) * 512],
                                start=(ib == 0), stop=(ib == 15))
                        nc.vector.tensor_copy(qt_s[:, cc * 512:(cc + 1) * 512],
                                              psq[:])
                    return qt_s

                def rope_q_half(h, qt_s, qrt, qc, cc):
                    """one 512-half of q-rope + c'-scaled copy, on DVE"""
                    hs = slice(cc * 512, (cc + 1) * 512)
                    t1 = ph.tile([128, 512], F32, tag="qtmp", bufs=2)
                    t2 = ph.tile([128, 512], F32, tag="qtmp", bufs=2)
                    psr2 = p2("psr_q")
                    nc.tensor.matmul(psr2[:], consts["pmrot"][:], qt_s[:, hs])
                    nc.vector.tensor_mul(t1[:], psr2[:], consts["sintab"][:, hs])
                    nc.vector.tensor_mul(t2[:], qt_s[:, hs].bitcast(F32),
                                         consts["costab"][:, hs])
                    nc.vector.tensor_add(qrt[:, hs], t1[:], t2[:])
                    nc.vector.tensor_scalar_mul(qc[:, hs], qrt[:, hs].bitcast(F32),
                                                consts["sqrtc"][:, h:h + 1])

                def make_etiles(h):
                    ets = []
                    for mb in range(NB):
                        qlo = _stripe_qlo(mb)
                        et = ph.tile([128, L - qlo], BF16, tag=f"esc{mb}", bufs=2,
                                     name=f"esc_h{mb}")
                        ets.append(et)
                    etiles[h] = ets

                def score_group(h, qrt, qc, mb, qs, npair, bs_dve=False):
                    """npair chunks (1 or 2) of stripe mb starting at qs; one
                    fused exp over the pair."""
                    g = h // 4
                    qlo = _stripe_qlo(mb)
                    kb = slice(mb * 128, (mb + 1) * 128)
                    ets = etiles[h]
                    raw = ph.tile([128, 512], F32, tag="raw", bufs=4)
                    for j in range(npair):
                        cqs = qs + 256 * j
                        cqe = cqs + 256
                        psA = sa("psA")
                        psB = sb("psB")
                        nc.tensor.matmul(psA[:, 0:256], krt[g][0:64, kb],
                                         qrt[0:64, cqs:cqe])
                        nc.tensor.matmul(psA[:, 256:512], kswap[g][0:64, kb],
                                         qc[0:64, cqs:cqe])
                        nc.tensor.matmul(psB[:, 0:256], krt[g][64:128, kb],
                                         qrt[64:128, cqs:cqe])
                        nc.tensor.matmul(psB[:, 256:512], kswap[g][64:128, kb],
                                         qc[64:128, cqs:cqe])
                        bs = ph.tile([128, 512], F32, tag="bs", bufs=4)
                        if bs_dve and j == 0:
                            nc.vector.tensor_copy(bs[:], psB[:])
                        else:
                            nc.scalar.copy(bs[:], psB[:])
                        tp = ph.tile([128, 512], F32, tag="tprod", bufs=4)
                        nc.vector.tensor_mul(tp[:], psA[:], bs[:])
                        nc.gpsimd.tensor_add(raw[:, 256 * j:256 * (j + 1)],
                                             tp[:, 0:256], tp[:, 256:512])
                    w = 256 * npair
                    esl = ets[mb][:, qs - qlo: qs - qlo + w]
                    nc.scalar.activation(esl, raw[:, 0:w], Act.Exp,
                                         bias=consts["maskb"][:, mb:mb + 1],
                                         scale=consts["alpha"][:, h:h + 1])
                    if qs == qlo:
                        # causal mask on the diagonal 256 cols:
                        # keep where (qlo + col) - (128*mb + part) >= 0
                        nc.gpsimd.affine_select(
                            ets[mb][:, 0:256], ets[mb][:, 0:256],
                            pattern=[[1, 256]], compare_op=AluOp.is_ge,
                            fill=0.0, base=qlo - 128 * mb,
                            channel_multiplier=-1)

                def attnv_half(h, c, use_sasb=True):
                    g = h // 4
                    ets = etiles[h]
                    ps_o, ps_rs = sa("ps_o"), sb("ps_rs")
                    mbs = [mb for mb in range(NB) if 128 * mb < 512 * (c + 1)]
                    for i, mb in enumerate(mbs):
                        qlo = _stripe_qlo(mb)
                        os_ = max(512 * c, 128 * mb)
                        oe = 512 * (c + 1)
                        esl = ets[mb][:, os_ - qlo: oe - qlo]
                        st, sp = (i == 0), (i == len(mbs) - 1)
                        nc.tensor.matmul(ps_o[:, os_ - 512 * c: oe - 512 * c],
                                         vblk[g * NB + mb][:], esl,
                                         start=st, stop=sp)
                        nc.tensor.matmul(ps_rs[:, os_ - 512 * c: oe - 512 * c],
                                         consts["onesb"][:], esl,
                                         start=st, stop=sp)
                    rcp = ph.tile([128, 512], F32, tag="rcp", bufs=3)
                    nc.vector.reciprocal_approx_fast(rcp[:], ps_rs[:])
                    nc.vector.tensor_mul(outtn[h][:, c * 512:(c + 1) * 512],
                                         ps_o[:], rcp[:])

                # chunk groups (mb, qs, npair): a = groups with qs < 512 of
                # stripes 0-3 (cover attnv c=0), b = the rest
                a_set = [(0, 0, 2), (1, 0, 2), (2, 256, 1), (3, 256, 1)]
                b_set = [(0, 512, 2), (1, 512, 2), (2, 512, 2), (3, 512, 2),
                         (4, 512, 2), (5, 512, 2), (6, 768, 1), (7, 768, 1)]

                qrts = {0: (qrt0, qc0)}
                wq_dma(1)
                for h in range(NH):
                    make_etiles(h)
                    qrt, qc = qrts[h]
                    wq_dma(h + 2)
                    # Qproj(h+1) first: no dependency on head h's chains, so
                    # it covers the tail of head h-1's vector pipeline.
                    qt_n = qproj(h + 1) if h + 1 < NH else None
                    if h > 0:
                        attnv_half(h - 1, 1)
                    for g_ in a_set:
                        score_group(h, qrt, qc, *g_)
                    if qt_n is not None:
                        qrt_n = ph.tile([128, L], F32R, tag="qrt", bufs=2,
                                        name=f"qrt{h+1}")
                        qc_n = ph.tile([128, L], F32R, tag="qc", bufs=2,
                                       name=f"qc{h+1}")
                        qrts[h + 1] = (qrt_n, qc_n)
                        rope_q_half(h + 1, qt_n, qrt_n, qc_n, 0)
                    if h == NH - 1:
                        nc.sync.dma_start(wo_c0[:], wo[:, :, 0:512])
                    for i, g_ in enumerate(b_set):
                        if i == 1:
                            attnv_half(h, 0)
                        if i == 2 and qt_n is not None:
                            rope_q_half(h + 1, qt_n, qrt_n, qc_n, 1)
                        score_group(h, qrt, qc, *g_)
                    qrts.pop(h)

                # first epilogue blocks (heads 0-6) cover head 7's E tail,
                # attnv(7,1), and the outtn[7] norm; their hh=7 matmuls land
                # in the epilogue. psy2/psy3 borrow the idle SA/SB rings.
                psy0 = p2("psy")
                for hh in range(NH - 1):
                    nc.tensor.matmul(psy0[:],
                                     outtn[hh][:, 0:128], wo_c0[:, hh, :],
                                     start=(hh == 0), stop=False)
                attnv_half(NH - 1, 1, use_sasb=True)
                psy23 = [sa("psy2"), sb("psy3")]
                for j, psyx in enumerate(psy23):
                    for hh in range(NH - 1):
                        nc.tensor.matmul(psyx[:],
                                         outtn[hh][:, (2 + j) * 128:(3 + j) * 128],
                                         wo_c0[:, hh, :],
                                         start=(hh == 0), stop=False)

            # ---------------- epilogue: Wo projection ----------------
            with tc.tile_pool(name="ep", bufs=1) as pe:
                def ytile_out(psy, dc, lb):
                    yt = pe.tile([128, 512], F32, tag="ytile", bufs=3)
                    nc.scalar.copy(yt[:], psy[:])
                    nc.sync.dma_start(
                        y[lb * 128:(lb + 1) * 128, dc * 512:(dc + 1) * 512],
                        yt[:])

                # finish the pre-opened blocks: lb=1's heads 0-6 cover the
                # outtn[7] norm latency, then close lb=0..3.
                psy1 = p2("psy")
                for hh in range(NH - 1):
                    nc.tensor.matmul(psy1[:],
                                     outtn[hh][:, 128:256], wo_c0[:, hh, :],
                                     start=(hh == 0), stop=False)
                for psyx, lb in [(psy0, 0), (psy1, 1), (psy23[0], 2),
                                 (psy23[1], 3)]:
                    nc.tensor.matmul(psyx[:], outtn[NH - 1][:, lb * 128:(lb + 1) * 128],
                                     wo_c0[:, NH - 1, :], start=False, stop=True)
                    ytile_out(psyx, 0, lb)

                wo_ts = {0: wo_c0}
                for dc in range(4):
                    if dc + 1 < 4:
                        wo_n = pe.tile([128, NH, 512], BF16, tag="wo_c", bufs=2,
                                       name=f"wo_c{dc+1}")
                        nc.sync.dma_start(
                            wo_n[:], wo[:, :, (dc + 1) * 512:(dc + 2) * 512])
                        wo_ts[dc + 1] = wo_n
                    wo_t = wo_ts.pop(dc)
                    for lb in range(4 if dc == 0 else 0, NB):
                        psy = p2("psy")
                        for hh in range(NH):
                            nc.tensor.matmul(
                                psy[:],
                                outtn[hh][:, lb * 128:(lb + 1) * 128],
                                wo_t[:, hh, :],
                                start=(hh == 0), stop=(hh == NH - 1))
                        ytile_out(psy, dc, lb)

    nc.compile()
    return nc


def _host_prep(x, Wq, Wk, Wv, Wo, q_param, log_scale, cos, sin, mask):
    """Build the 8 per-core input maps."""
    x = np.asarray(x, np.float32)
    Wq = np.asarray(Wq, np.float32)
    Wk = np.asarray(Wk, np.float32)
    Wv = np.asarray(Wv, np.float32)
    Wo = np.asarray(Wo, np.float32)
    cos = np.asarray(cos, np.float32)[0]      # [L, H, 64]
    sin = np.asarray(sin, np.float32)[0]
    qp = np.asarray(q_param, np.float32).reshape(H)
    ls = np.asarray(log_scale, np.float32).reshape(H)
    mask = np.asarray(mask)

    p64 = np.arange(128) % 64
    # rope tables are identical across heads: use head 0
    cos_p = np.ascontiguousarray(cos[:, 0, :][:, p64].T)   # [128, L]
    sin_p = np.ascontiguousarray(sin[:, 0, :][:, p64].T)

    PM = np.zeros((128, 128), np.float32)
    for dp in range(128):
        base, r = (dp // 64) * 64, dp % 64
        if r < 32:
            PM[base + r + 32, dp] = -1.0
        else:
            PM[base + r - 32, dp] = 1.0
    SW = np.zeros((128, 128), np.float32)
    for dp in range(128):
        SW[(dp + 64) % 128, dp] = 1.0
    ONES = np.ones((128, 128), ml_dtypes.bfloat16)
    IDENT = np.eye(128, dtype=ml_dtypes.bfloat16)

    in_maps = []
    for core in range(8):
        b, g2 = core // 2, core % 2
        heads = list(range(g2 * NH, (g2 + 1) * NH))

        xhv = x[b].astype(ml_dtypes.bfloat16)

        wq_c = Wq[:, g2 * NH * 128:(g2 + 1) * NH * 128]
        wk_c = Wk[:, g2 * NKV * 128:(g2 + 1) * NKV * 128]
        wv_c = Wv[:, g2 * NKV * 128:(g2 + 1) * NKV * 128]
        wo_c = Wo[g2 * NH * 128:(g2 + 1) * NH * 128, :]

        wq_p = wq_c.reshape(16, 128, NH, 128).transpose(2, 1, 0, 3).copy()
        wk_p = wk_c.reshape(16, 128, NKV * 128).transpose(1, 0, 2).copy()
        wv_p = wv_c.reshape(16, 128, NKV * 128).transpose(1, 0, 2).copy()
        wo_p = wo_c.reshape(NH, 128, D).transpose(1, 0, 2).astype(ml_dtypes.bfloat16)

        mb = np.where(mask[b].reshape(NB, 128).T.astype(bool), 0.0, -1e9)
        mb = mb.astype(np.float32)

        cpr = -2.0 * np.tanh(qp[heads])               # per-head c'
        sq = np.sqrt(np.abs(cpr))
        sqc = np.tile(sq[None, :], (128, 1))
        sqc[:64, :] *= np.sign(cpr)[None, :]
        alp = np.tile((np.exp(ls[heads]) / HD)[None, :], (128, 1))

        in_maps.append({
            "xh": xhv,
            "wq": wq_p.astype(ml_dtypes.bfloat16),
            "wk": wk_p.astype(ml_dtypes.bfloat16),
            "wv": wv_p.astype(ml_dtypes.bfloat16), "wo": wo_p,
            "costab": cos_p, "sintab": sin_p,
            "maskb": mb, "sqrtc": sqc.astype(np.float32),
            "alpha": alp.astype(np.float32),
            "pmrot": PM, "pmswap": SW, "onesb": ONES, "identb": IDENT,
        })
    return in_maps


def kernel(**inputs):
    if "nc" not in _CACHED:
        _CACHED["nc"] = build_program()
    nc = _CACHED["nc"]
    in_maps = _host_prep(**inputs)
    res = run_bass_kernel_spmd(nc, in_maps, list(range(8))).results
    out = np.empty((B, L, D), np.float32)
    for b in range(B):
        out[b] = res[2 * b]["y"] + res[2 * b + 1]["y"]
    return out


# revision 7
# speedup vs baseline: 1.0903x; 1.0134x over previous
"""BivectorRotarySelfAttention TRN2 kernel, v3.

Sharding: 8 cores = 4 batches x 2 head-halves; host sums the two head-half
partial y's per batch.

v3 vs v2:
 - Score PSUM rings are 3-deep (SA/SB [128,512]x3) so the PE runs ~3 chunks
   ahead of the vector chain; all other PSUM users are [128,512] halves in a
   shared 2-deep ring (P2). 6+6+4 KB = 16 KB exactly.
 - Engine rebalance: rope t2/add and all raw-combines on Pool, 2 of 20 bs
   copies on DVE, rest on ACT.
 - Qproj+rope for head h+1 runs mid-scores(h); attnv halves are interleaved
   into the chunk stream so PE never waits on the exp chain.
 - V projection trails K by 4 ib-steps to match wv DMA arrival; wk DMA is
   issued before everything except nothing (first), consts after wv.
 - Epilogue prefetches wo and accumulates head 7 last.
"""
import sys
if '/opt/trn_rl_repo' not in sys.path:
    sys.path.insert(0, '/opt/trn_rl_repo')

import numpy as np
import ml_dtypes

import concourse.bass as bass
import concourse.mybir as mybir
import concourse.tile as tile
from concourse import bacc
from concourse.bass_utils import run_bass_kernel_spmd

F32 = mybir.dt.float32
F32R = mybir.dt.float32r
BF16 = mybir.dt.bfloat16

B, L, D, H, HKV = 4, 1024, 2048, 16, 4
HD = D // H            # 128
HD2 = HD // 2          # 64
NH = 8                 # heads per core
NKV = 2                # kv heads per core
NB = L // 128          # 8 blocks of 128
AluOp = mybir.AluOpType
Act = mybir.ActivationFunctionType

_CACHED = {}


def _stripe_qlo(mb):
    # stripe mb covers q in [qlo, L) in 256-wide chunks; odd stripes start one
    # 128-block early (the extra region is causal-masked to zero).
    return 128 * (mb - (mb % 2))


def build_program():
    nc = bacc.Bacc("TRN2", target_bir_lowering=False, debug=False)

    # ---- dram params (per-core shapes) ----
    xh = nc.declare_dram_parameter("xh", [D, L], BF16, isOutput=False)
    wq = nc.declare_dram_parameter("wq", [NH, 128, 16, 128], BF16, isOutput=False)
    wk = nc.declare_dram_parameter("wk", [128, 16, NKV * 128], BF16, isOutput=False)
    wv = nc.declare_dram_parameter("wv", [128, 16, NKV * 128], BF16, isOutput=False)
    wo = nc.declare_dram_parameter("wo", [128, NH, D], BF16, isOutput=False)
    costab = nc.declare_dram_parameter("costab", [128, L], F32, isOutput=False)
    sintab = nc.declare_dram_parameter("sintab", [128, L], F32, isOutput=False)
    maskb = nc.declare_dram_parameter("maskb", [128, NB], F32, isOutput=False)
    sqrtc = nc.declare_dram_parameter("sqrtc", [128, NH], F32, isOutput=False)
    alpha = nc.declare_dram_parameter("alpha", [128, NH], F32, isOutput=False)
    pmrot = nc.declare_dram_parameter("pmrot", [128, 128], F32R, isOutput=False)
    pmswap = nc.declare_dram_parameter("pmswap", [128, 128], F32R, isOutput=False)
    onesb = nc.declare_dram_parameter("onesb", [128, 128], BF16, isOutput=False)
    identb = nc.declare_dram_parameter("identb", [128, 128], BF16, isOutput=False)
    y = nc.declare_dram_parameter("y", [L, D], F32, isOutput=True)

    with tile.TileContext(nc) as tc:
        with (
            tc.tile_pool(name="persist", bufs=1) as pp,
            tc.tile_pool(name="psum", bufs=1, space="PSUM") as psp,
        ):
            # ---- persistent SBUF ----
            xt = [pp.tile([128, L], BF16, tag=f"xt{ib}", name=f"xt{ib}")
                  for ib in range(16)]
            krt = [pp.tile([128, L], F32R, tag=f"krt{g}", name=f"krt{g}")
                   for g in range(NKV)]
            kswap = [pp.tile([128, L], F32R, tag=f"ksw{g}", name=f"ksw{g}")
                     for g in range(NKV)]
            vblk = [pp.tile([128, 128], BF16, tag=f"vb{i}", name=f"vb{i}")
                    for i in range(NKV * NB)]
            outtn = [pp.tile([128, L], BF16, tag=f"ot{h}", name=f"ot{h}")
                     for h in range(NH)]
            wo_c0 = pp.tile([128, NH, 512], BF16, tag="wo_c0", name="wo_c0")
            consts = {}

            def sa(name):
                return psp.tile([128, 512], F32, tag="SA", bufs=3, name=name)

            def sb(name):
                return psp.tile([128, 512], F32, tag="SB", bufs=3, name=name)

            def p2(name, shape=None, dtype=F32):
                return psp.tile(shape or [128, 512], dtype, tag="P2", bufs=2,
                                name=name)

            # ---------------- prologue ----------------
            with tc.tile_pool(name="pro", bufs=1) as ppro:
                wk_t = ppro.tile([128, 16, NKV * 128], BF16, tag="wk")
                wv_t = ppro.tile([128, 16, NKV * 128], BF16, tag="wv")

                # DMA order: wk (halved), x0, wv, rest of x, consts.
                nc.sync.dma_start(wk_t[:, 0:8], wk[:, 0:8])
                nc.sync.dma_start(wk_t[:, 8:16], wk[:, 8:16])
                def xdma(ib):
                    nc.sync.dma_start(
                        xt[ib][:], xh[ib * 128:(ib + 1) * 128, :])

                wq_t0 = ppro.tile([128, 16, 128], BF16, tag="wq0", name="wq_t0")
                xdma(0)
                nc.sync.dma_start(wv_t[:], wv[:])
                for ib in range(1, 16):
                    xdma(ib)
                nc.sync.dma_start(wq_t0[:], wq[0])
                for nm, src, dt_ in [("pmrot", pmrot, F32R),
                                     ("costab", costab, F32),
                                     ("sintab", sintab, F32),
                                     ("identb", identb, BF16),
                                     ("pmswap", pmswap, F32R),
                                     ("onesb", onesb, BF16),
                                     ("maskb", maskb, F32),
                                     ("sqrtc", sqrtc, F32),
                                     ("alpha", alpha, F32)]:
                    t = pp.tile(list(src.shape), dt_, tag=nm, name=nm)
                    nc.sync.dma_start(t[:], src[:])
                    consts[nm] = t

                # K/V projection accumulators: [128,512] halves.
                psk = [[sa("psk0a"), sa("psk0b")], [sb("psk1a"), sb("psk1b")]]
                psv = [[sa("psv0a"), sb("psv0b")], [p2("psv1a"), p2("psv1b")]]

                VOFF = 3
                for step in range(16 + VOFF):
                    if step < 16:
                        ib = step
                        for g in range(NKV):
                            for c in range(2):
                                nc.tensor.matmul(
                                    psk[g][c][:],
                                    wk_t[:, ib, g * 128:(g + 1) * 128],
                                    xt[ib][:, c * 512:(c + 1) * 512],
                                    start=(ib == 0), stop=(ib == 15))
                    if step >= VOFF:
                        ib = step - VOFF
                        for g in range(NKV):
                            for c in range(2):
                                nc.tensor.matmul(
                                    psv[g][c][:],
                                    wv_t[:, ib, g * 128:(g + 1) * 128],
                                    xt[ib][:, c * 512:(c + 1) * 512],
                                    start=(ib == 0), stop=(ib == 15))

                # PSUM->SBUF copies: v-g1 first (frees the P2 slots that
                # Qproj(0) needs), then interleaved kt/vt.
                vt_s = [ppro.tile([128, L], BF16, tag=f"vt_s{g}", name=f"vt{g}")
                        for g in range(NKV)]
                kt_s = [ppro.tile([128, L], F32R, tag=f"kt_s{g}", name=f"kt{g}")
                        for g in range(NKV)]
                for c in range(2):
                    nc.scalar.copy(vt_s[1][:, c * 512:(c + 1) * 512], psv[1][c][:])
                for g in range(NKV):
                    for c in range(2):
                        nc.scalar.copy(kt_s[g][:, c * 512:(c + 1) * 512],
                                       psk[g][c][:])
                for c in range(2):
                    nc.scalar.copy(vt_s[0][:, c * 512:(c + 1) * 512], psv[0][c][:])

                # Qproj(0): no dependency on the copies above except P2 slots;
                # covers the kt/vt copy chain on PE.
                qt_s0 = ppro.tile([128, L], F32R, tag="qt_s0", name="qt_s0")
                for cc in range(2):
                    psq = p2("psq0")
                    for ib in range(16):
                        nc.tensor.matmul(psq[:], wq_t0[:, ib, :],
                                         xt[ib][:, cc * 512:(cc + 1) * 512],
                                         start=(ib == 0), stop=(ib == 15))
                    nc.vector.tensor_copy(qt_s0[:, cc * 512:(cc + 1) * 512],
                                          psq[:])

                # rope-q0 chain starts immediately (DVE halves); the k-rope /
                # v-transpose / k-swap PE work below covers it.
                qrt0 = pp.tile([128, L], F32R, tag="qrt0", name="qrt0")
                qc0 = pp.tile([128, L], F32R, tag="qc0", name="qc0")
                for cc in range(2):
                    hs = slice(cc * 512, (cc + 1) * 512)
                    t1q = ppro.tile([128, 512], F32, tag="rq0", bufs=2)
                    t2q = ppro.tile([128, 512], F32, tag="rq0", bufs=2)
                    psr2 = p2("psr_q0")
                    nc.tensor.matmul(psr2[:], consts["pmrot"][:], qt_s0[:, hs])
                    nc.vector.tensor_mul(t1q[:], psr2[:], consts["sintab"][:, hs])
                    nc.vector.tensor_mul(t2q[:], qt_s0[:, hs].bitcast(F32),
                                         consts["costab"][:, hs])
                    nc.vector.tensor_add(qrt0[:, hs], t1q[:], t2q[:])
                    nc.vector.tensor_scalar_mul(qc0[:, hs],
                                                qrt0[:, hs].bitcast(F32),
                                                consts["sqrtc"][:, 0:1])

                # k rope rotate-half part (SA slots freed by the kt copies)
                for g in range(NKV):
                    t1 = ppro.tile([128, L], F32, tag="rtmp", bufs=4)
                    t2 = ppro.tile([128, L], F32, tag="rtmp", bufs=4)
                    for c in range(2):
                        psr = sa(f"psr_k{c}")
                        nc.tensor.matmul(psr[:],
                                         consts["pmrot"][:],
                                         kt_s[g][:, c * 512:(c + 1) * 512])
                        nc.vector.tensor_mul(
                            t1[:, c * 512:(c + 1) * 512], psr[:],
                            consts["sintab"][:, c * 512:(c + 1) * 512])
                    nc.gpsimd.tensor_mul(t2[:], kt_s[g][:].bitcast(F32),
                                         consts["costab"][:])
                    nc.vector.tensor_add(krt[g][:], t1[:], t2[:])

                # v transpose to [m, d] blocks
                for g in range(NKV):
                    for mb in range(NB):
                        pv = p2("pv", shape=[128, 128], dtype=BF16)
                        nc.tensor.transpose(pv[:], vt_s[g][:, mb * 128:(mb + 1) * 128],
                                            consts["identb"][:])
                        nc.vector.tensor_copy(vblk[g * NB + mb][:], pv[:])

                # k swap perms (krt chains complete under the ops above)
                for g in range(NKV):
                    for c in range(2):
                        psw = sb(f"psw_k{g}{c}")
                        nc.tensor.matmul(psw[:],
                                         consts["pmswap"][:],
                                         krt[g][:, c * 512:(c + 1) * 512])
                        nc.scalar.copy(kswap[g][:, c * 512:(c + 1) * 512],
                                       psw[:])

            # ---------------- head loop ----------------
            with tc.tile_pool(name="hl", bufs=1) as ph:
                etiles = {}

                wq_tiles = {}

                def wq_dma(h):
                    if h < NH:
                        wq_t = ph.tile([128, 16, 128], BF16, tag="wq_h", bufs=2,
                                       name=f"wq_t{h}")
                        nc.sync.dma_start(wq_t[:], wq[h])
                        wq_tiles[h] = wq_t

                def qproj(h):
                    wq_t = wq_tiles.pop(h)
                    qt_s = ph.tile([128, L], F32R, tag="qt_s", bufs=2)
                    for cc in range(2):
                        psq = p2("psq")
                        for ib in range(16):
                            nc.tensor.matmul(
                                psq[:],
                                wq_t[:, ib, :],
                                xt[ib][:, cc * 512:(cc + 1) * 512],
                                start=(ib == 0), stop=(ib == 15))
                        nc.vector.tensor_copy(qt_s[:, cc * 512:(cc + 1) * 512],
                                              psq[:])
                    return qt_s

                def rope_q_half(h, qt_s, qrt, qc, cc):
                    """one 512-half of q-rope + c'-scaled copy, on DVE"""
                    hs = slice(cc * 512, (cc + 1) * 512)
                    t1 = ph.tile([128, 512], F32, tag="qtmp", bufs=2)
                    t2 = ph.tile([128, 512], F32, tag="qtmp", bufs=2)
                    psr2 = p2("psr_q")
                    nc.tensor.matmul(psr2[:], consts["pmrot"][:], qt_s[:, hs])
                    nc.vector.tensor_mul(t1[:], psr2[:], consts["sintab"][:, hs])
                    nc.vector.tensor_mul(t2[:], qt_s[:, hs].bitcast(F32),
                                         consts["costab"][:, hs])
                    nc.vector.tensor_add(qrt[:, hs], t1[:], t2[:])
                    nc.vector.tensor_scalar_mul(qc[:, hs], qrt[:, hs].bitcast(F32),
                                                consts["sqrtc"][:, h:h + 1])

                def make_etiles(h):
                    ets = []
                    for mb in range(NB):
                        qlo = _stripe_qlo(mb)
                        et = ph.tile([128, L - qlo], BF16, tag=f"esc{mb}", bufs=2,
                                     name=f"esc_h{mb}")
                        ets.append(et)
                    etiles[h] = ets

                def score_group(h, qrt, qc, mb, qs, npair, bs_dve=False):
                    """npair chunks (1 or 2) of stripe mb starting at qs; one
                    fused exp over the pair."""
                    g = h // 4
                    qlo = _stripe_qlo(mb)
                    kb = slice(mb * 128, (mb + 1) * 128)
                    ets = etiles[h]
                    raw = ph.tile([128, 512], F32, tag="raw", bufs=4)
                    for j in range(npair):
                        cqs = qs + 256 * j
                        cqe = cqs + 256
                        psA = sa("psA")
                        psB = sb("psB")
                        nc.tensor.matmul(psA[:, 0:256], krt[g][0:64, kb],
                                         qrt[0:64, cqs:cqe])
                        nc.tensor.matmul(psA[:, 256:512], kswap[g][0:64, kb],
                                         qc[0:64, cqs:cqe])
                        nc.tensor.matmul(psB[:, 0:256], krt[g][64:128, kb],
                                         qrt[64:128, cqs:cqe])
                        nc.tensor.matmul(psB[:, 256:512], kswap[g][64:128, kb],
                                         qc[64:128, cqs:cqe])
                        bs = ph.tile([128, 512], F32, tag="bs", bufs=4)
                        if bs_dve and j == 0:
                            nc.vector.tensor_copy(bs[:], psB[:])
                        else:
                            nc.scalar.copy(bs[:], psB[:])
                        tp = ph.tile([128, 512], F32, tag="tprod", bufs=4)
                        nc.vector.tensor_mul(tp[:], psA[:], bs[:])
                        nc.gpsimd.tensor_add(raw[:, 256 * j:256 * (j + 1)],
                                             tp[:, 0:256], tp[:, 256:512])
                    w = 256 * npair
                    esl = ets[mb][:, qs - qlo: qs - qlo + w]
                    nc.scalar.activation(esl, raw[:, 0:w], Act.Exp,
                                         bias=consts["maskb"][:, mb:mb + 1],
                                         scale=consts["alpha"][:, h:h + 1])
                    if qs == qlo:
                        # causal mask on the diagonal 256 cols:
                        # keep where (qlo + col) - (128*mb + part) >= 0
                        nc.gpsimd.affine_select(
                            ets[mb][:, 0:256], ets[mb][:, 0:256],
                            pattern=[[1, 256]], compare_op=AluOp.is_ge,
                            fill=0.0, base=qlo - 128 * mb,
                            channel_multiplier=-1)

                def attnv_half(h, c, use_sasb=True):
                    g = h // 4
                    ets = etiles[h]
                    ps_o, ps_rs = sa("ps_o"), sb("ps_rs")
                    mbs = [mb for mb in range(NB) if 128 * mb < 512 * (c + 1)]
                    for i, mb in enumerate(mbs):
                        qlo = _stripe_qlo(mb)
                        os_ = max(512 * c, 128 * mb)
                        oe = 512 * (c + 1)
                        esl = ets[mb][:, os_ - qlo: oe - qlo]
                        st, sp = (i == 0), (i == len(mbs) - 1)
                        nc.tensor.matmul(ps_o[:, os_ - 512 * c: oe - 512 * c],
                                         vblk[g * NB + mb][:], esl,
                                         start=st, stop=sp)
                        nc.tensor.matmul(ps_rs[:, os_ - 512 * c: oe - 512 * c],
                                         consts["onesb"][:], esl,
                                         start=st, stop=sp)
                    rcp = ph.tile([128, 512], F32, tag="rcp", bufs=3)
                    nc.vector.reciprocal_approx_fast(rcp[:], ps_rs[:])
                    nc.vector.tensor_mul(outtn[h][:, c * 512:(c + 1) * 512],
                                         ps_o[:], rcp[:])

                # chunk groups (mb, qs, npair): a = groups with qs < 512 of
                # stripes 0-3 (cover attnv c=0), b = the rest
                a_set = [(0, 0, 2), (1, 0, 2), (2, 256, 1), (3, 256, 1)]
                b_set = [(0, 512, 2), (1, 512, 2), (2, 512, 2), (3, 512, 2),
                         (4, 512, 2), (5, 512, 2), (6, 768, 1), (7, 768, 1)]

                qrts = {0: (qrt0, qc0)}
                wq_dma(1)
                for h in range(NH):
                    make_etiles(h)
                    qrt, qc = qrts[h]
                    wq_dma(h + 2)
                    # Qproj(h+1) first: no dependency on head h's chains, so
                    # it covers the tail of head h-1's vector pipeline.
                    qt_n = qproj(h + 1) if h + 1 < NH else None
                    if h > 0:
                        attnv_half(h - 1, 1)
                    for g_ in a_set:
                        score_group(h, qrt, qc, *g_)
                    if qt_n is not None:
                        qrt_n = ph.tile([128, L], F32R, tag="qrt", bufs=2,
                                        name=f"qrt{h+1}")
                        qc_n = ph.tile([128, L], F32R, tag="qc", bufs=2,
                                       name=f"qc{h+1}")
                        qrts[h + 1] = (qrt_n, qc_n)
                        rope_q_half(h + 1, qt_n, qrt_n, qc_n, 0)
                    if h == NH - 1:
                        nc.sync.dma_start(wo_c0[:], wo[:, :, 0:512])
                    for i, g_ in enumerate(b_set):
                        if i == 1:
                            attnv_half(h, 0)
                        if i == 2 and qt_n is not None:
                            rope_q_half(h + 1, qt_n, qrt_n, qc_n, 1)
                        score_group(h, qrt, qc, *g_)
                    qrts.pop(h)

                # first epilogue blocks (heads 0-6) cover head 7's E tail,
                # attnv(7,1), and the outtn[7] norm; their hh=7 matmuls land
                # in the epilogue. psy2/psy3 borrow the idle SA/SB rings.
                psy0 = p2("psy")
                for hh in range(NH - 1):
                    nc.tensor.matmul(psy0[:],
                                     outtn[hh][:, 0:128], wo_c0[:, hh, :],
                                     start=(hh == 0), stop=False)
                attnv_half(NH - 1, 1, use_sasb=True)
                psy23 = [sa("psy2"), sb("psy3")]
                for j, psyx in enumerate(psy23):
                    for hh in range(NH - 1):
                        nc.tensor.matmul(psyx[:],
                                         outtn[hh][:, (2 + j) * 128:(3 + j) * 128],
                                         wo_c0[:, hh, :],
                                         start=(hh == 0), stop=False)

            # ---------------- epilogue: Wo projection ----------------
            with tc.tile_pool(name="ep", bufs=1) as pe:
                def ytile_out(psy, dc, lb):
                    yt = pe.tile([128, 512], F32, tag="ytile", bufs=3)
                    nc.scalar.copy(yt[:], psy[:])
                    nc.sync.dma_start(
                        y[lb * 128:(lb + 1) * 128, dc * 512:(dc + 1) * 512],
                        yt[:])

                # finish the pre-opened blocks: lb=1's heads 0-6 cover the
                # outtn[7] norm latency, then close lb=0..3.
                psy1 = p2("psy")
                for hh in range(NH - 1):
                    nc.tensor.matmul(psy1[:],
                                     outtn[hh][:, 128:256], wo_c0[:, hh, :],
                                     start=(hh == 0), stop=False)
                for psyx, lb in [(psy0, 0), (psy1, 1), (psy23[0], 2),
                                 (psy23[1], 3)]:
                    nc.tensor.matmul(psyx[:], outtn[NH - 1][:, lb * 128:(lb + 1) * 128],
                                     wo_c0[:, NH - 1, :], start=False, stop=True)
                    ytile_out(psyx, 0, lb)

                wo_ts = {0: wo_c0}
                for dc in range(4):
                    if dc + 1 < 4:
                        wo_n = pe.tile([128, NH, 512], BF16, tag="wo_c", bufs=2,
                                       name=f"wo_c{dc+1}")
                        nc.sync.dma_start(
                            wo_n[:], wo[:, :, (dc + 1) * 512:(dc + 2) * 512])
                        wo_ts[dc + 1] = wo_n
                    wo_t = wo_ts.pop(dc)
                    for lb in range(4 if dc == 0 else 0, NB):
                        psy = p2("psy")
                        for hh in range(NH):
                            nc.tensor.matmul(
                                psy[:],
                                outtn[hh][:, lb * 128:(lb + 1) * 128],
                                wo_t[:, hh, :],
                                start=(hh == 0), stop=(hh == NH - 1))
                        ytile_out(psy, dc, lb)

    nc.compile()
    return nc


def _host_prep(x, Wq, Wk, Wv, Wo, q_param, log_scale, cos, sin, mask):
    """Build the 8 per-core input maps."""
    x = np.asarray(x, np.float32)
    Wq = np.asarray(Wq, np.float32)
    Wk = np.asarray(Wk, np.float32)
    Wv = np.asarray(Wv, np.float32)
    Wo = np.asarray(Wo, np.float32)
    cos = np.asarray(cos, np.float32)[0]      # [L, H, 64]
    sin = np.asarray(sin, np.float32)[0]
    qp = np.asarray(q_param, np.float32).reshape(H)
    ls = np.asarray(log_scale, np.float32).reshape(H)
    mask = np.asarray(mask)

    p64 = np.arange(128) % 64
    # rope tables are identical across heads: use head 0
    cos_p = np.ascontiguousarray(cos[:, 0, :][:, p64].T)   # [128, L]
    sin_p = np.ascontiguousarray(sin[:, 0, :][:, p64].T)

    PM = np.zeros((128, 128), np.float32)
    for dp in range(128):
        base, r = (dp // 64) * 64, dp % 64
        if r < 32:
            PM[base + r + 32, dp] = -1.0
        else:
            PM[base + r - 32, dp] = 1.0
    SW = np.zeros((128, 128), np.float32)
    for dp in range(128):
        SW[(dp + 64) % 128, dp] = 1.0
    ONES = np.ones((128, 128), ml_dtypes.bfloat16)
    IDENT = np.eye(128, dtype=ml_dtypes.bfloat16)

    in_maps = []
    for core in range(8):
        b, g2 = core // 2, core % 2
        heads = list(range(g2 * NH, (g2 + 1) * NH))

        xhv = np.ascontiguousarray(x[b].T).astype(ml_dtypes.bfloat16)

        wq_c = Wq[:, g2 * NH * 128:(g2 + 1) * NH * 128]
        wk_c = Wk[:, g2 * NKV * 128:(g2 + 1) * NKV * 128]
        wv_c = Wv[:, g2 * NKV * 128:(g2 + 1) * NKV * 128]
        wo_c = Wo[g2 * NH * 128:(g2 + 1) * NH * 128, :]

        wq_p = wq_c.reshape(16, 128, NH, 128).transpose(2, 1, 0, 3).copy()
        wk_p = wk_c.reshape(16, 128, NKV * 128).transpose(1, 0, 2).copy()
        wv_p = wv_c.reshape(16, 128, NKV * 128).transpose(1, 0, 2).copy()
        wo_p = wo_c.reshape(NH, 128, D).transpose(1, 0, 2).astype(ml_dtypes.bfloat16)

        mb = np.where(mask[b].reshape(NB, 128).T.astype(bool), 0.0, -1e9)
        mb = mb.astype(np.float32)

        cpr = -2.0 * np.tanh(qp[heads])               # per-head c'
        sq = np.sqrt(np.abs(cpr))
        sqc = np.tile(sq[None, :], (128, 1))
        sqc[:64, :] *= np.sign(cpr)[None, :]
        alp = np.tile((np.exp(ls[heads]) / HD)[None, :], (128, 1))

        in_maps.append({
            "xh": xhv,
            "wq": wq_p.astype(ml_dtypes.bfloat16),
            "wk": wk_p.astype(ml_dtypes.bfloat16),
            "wv": wv_p.astype(ml_dtypes.bfloat16), "wo": wo_p,
            "costab": cos_p, "sintab": sin_p,
            "maskb": mb, "sqrtc": sqc.astype(np.float32),
            "alpha": alp.astype(np.float32),
            "pmrot": PM, "pmswap": SW, "onesb": ONES, "identb": IDENT,
        })
    return in_maps


def kernel(**inputs):
    if "nc" not in _CACHED:
        _CACHED["nc"] = build_program()
    nc = _CACHED["nc"]
    in_maps = _host_prep(**inputs)
    res = run_bass_kernel_spmd(nc, in_maps, list(range(8))).results
    out = np.empty((B, L, D), np.float32)
    for b in range(B):
        out[b] = res[2 * b]["y"] + res[2 * b + 1]["y"]
    return out


# revision 8
# speedup vs baseline: 1.0950x; 1.0043x over previous
"""BivectorRotarySelfAttention TRN2 kernel, v3.

Sharding: 8 cores = 4 batches x 2 head-halves; host sums the two head-half
partial y's per batch.

v3 vs v2:
 - Score PSUM rings are 3-deep (SA/SB [128,512]x3) so the PE runs ~3 chunks
   ahead of the vector chain; all other PSUM users are [128,512] halves in a
   shared 2-deep ring (P2). 6+6+4 KB = 16 KB exactly.
 - Engine rebalance: rope t2/add and all raw-combines on Pool, 2 of 20 bs
   copies on DVE, rest on ACT.
 - Qproj+rope for head h+1 runs mid-scores(h); attnv halves are interleaved
   into the chunk stream so PE never waits on the exp chain.
 - V projection trails K by 4 ib-steps to match wv DMA arrival; wk DMA is
   issued before everything except nothing (first), consts after wv.
 - Epilogue prefetches wo and accumulates head 7 last.
"""
import sys
if '/opt/trn_rl_repo' not in sys.path:
    sys.path.insert(0, '/opt/trn_rl_repo')

import numpy as np
import ml_dtypes

import concourse.bass as bass
import concourse.mybir as mybir
import concourse.tile as tile
from concourse import bacc
from concourse.bass_utils import run_bass_kernel_spmd

F32 = mybir.dt.float32
F32R = mybir.dt.float32r
BF16 = mybir.dt.bfloat16

B, L, D, H, HKV = 4, 1024, 2048, 16, 4
HD = D // H            # 128
HD2 = HD // 2          # 64
NH = 8                 # heads per core
NKV = 2                # kv heads per core
NB = L // 128          # 8 blocks of 128
AluOp = mybir.AluOpType
Act = mybir.ActivationFunctionType

_CACHED = {}


def _stripe_qlo(mb):
    # stripe mb covers q in [qlo, L) in 256-wide chunks; odd stripes start one
    # 128-block early (the extra region is causal-masked to zero).
    return 128 * (mb - (mb % 2))


def build_program():
    nc = bacc.Bacc("TRN2", target_bir_lowering=False, debug=False)

    # ---- dram params (per-core shapes) ----
    xh = nc.declare_dram_parameter("xh", [D, L], BF16, isOutput=False)
    wq = nc.declare_dram_parameter("wq", [NH, 128, 16, 128], BF16, isOutput=False)
    wk = nc.declare_dram_parameter("wk", [128, 16, NKV * 128], BF16, isOutput=False)
    wv = nc.declare_dram_parameter("wv", [128, 16, NKV * 128], BF16, isOutput=False)
    wo = nc.declare_dram_parameter("wo", [128, NH, D], BF16, isOutput=False)
    costab = nc.declare_dram_parameter("costab", [128, L], F32, isOutput=False)
    sintab = nc.declare_dram_parameter("sintab", [128, L], F32, isOutput=False)
    maskb = nc.declare_dram_parameter("maskb", [128, NB], F32, isOutput=False)
    sqrtc = nc.declare_dram_parameter("sqrtc", [128, NH], F32, isOutput=False)
    alpha = nc.declare_dram_parameter("alpha", [128, NH], F32, isOutput=False)
    pmrot = nc.declare_dram_parameter("pmrot", [128, 128], F32R, isOutput=False)
    pmswap = nc.declare_dram_parameter("pmswap", [128, 128], F32R, isOutput=False)
    onesb = nc.declare_dram_parameter("onesb", [128, 128], BF16, isOutput=False)
    identb = nc.declare_dram_parameter("identb", [128, 128], BF16, isOutput=False)
    y = nc.declare_dram_parameter("y", [L, D], F32, isOutput=True)

    with tile.TileContext(nc) as tc:
        with (
            tc.tile_pool(name="persist", bufs=1) as pp,
            tc.tile_pool(name="psum", bufs=1, space="PSUM") as psp,
        ):
            # ---- persistent SBUF ----
            xt = [pp.tile([128, L], BF16, tag=f"xt{ib}", name=f"xt{ib}")
                  for ib in range(16)]
            krt = [pp.tile([128, L], F32R, tag=f"krt{g}", name=f"krt{g}")
                   for g in range(NKV)]
            kswap = [pp.tile([128, L], F32R, tag=f"ksw{g}", name=f"ksw{g}")
                     for g in range(NKV)]
            vblk = [pp.tile([128, 128], BF16, tag=f"vb{i}", name=f"vb{i}")
                    for i in range(NKV * NB)]
            outtn = [pp.tile([128, L], BF16, tag=f"ot{h}", name=f"ot{h}")
                     for h in range(NH)]
            wo_c0 = pp.tile([128, NH, 512], BF16, tag="wo_c0", name="wo_c0")
            consts = {}

            def sa(name):
                return psp.tile([128, 512], F32, tag="SA", bufs=3, name=name)

            def sb(name):
                return psp.tile([128, 512], F32, tag="SB", bufs=3, name=name)

            def p2(name, shape=None, dtype=F32):
                return psp.tile(shape or [128, 512], dtype, tag="P2", bufs=2,
                                name=name)

            # ---------------- prologue ----------------
            with tc.tile_pool(name="pro", bufs=1) as ppro:
                wk_t = ppro.tile([128, 16, NKV * 128], BF16, tag="wk")
                wv_t = ppro.tile([128, 16, NKV * 128], BF16, tag="wv")

                # DMA order: wk (halved), x0, wv, rest of x, consts.
                def xdma(ib):
                    nc.sync.dma_start(
                        xt[ib][:], xh[ib * 128:(ib + 1) * 128, :])

                wq_t0 = ppro.tile([128, 16, 128], BF16, tag="wq0", name="wq_t0")
                nc.sync.dma_start(wk_t[:, 0:8], wk[:, 0:8])
                xdma(0)
                nc.sync.dma_start(wk_t[:, 8:16], wk[:, 8:16])
                nc.sync.dma_start(wv_t[:], wv[:])
                for ib in range(1, 16):
                    xdma(ib)
                nc.sync.dma_start(wq_t0[:], wq[0])
                for nm, src, dt_ in [("pmrot", pmrot, F32R),
                                     ("costab", costab, F32),
                                     ("sintab", sintab, F32),
                                     ("identb", identb, BF16),
                                     ("pmswap", pmswap, F32R),
                                     ("onesb", onesb, BF16),
                                     ("maskb", maskb, F32),
                                     ("sqrtc", sqrtc, F32),
                                     ("alpha", alpha, F32)]:
                    t = pp.tile(list(src.shape), dt_, tag=nm, name=nm)
                    nc.sync.dma_start(t[:], src[:])
                    consts[nm] = t

                # K/V projection accumulators: [128,512] halves.
                psk = [[sa("psk0a"), sa("psk0b")], [sb("psk1a"), sb("psk1b")]]
                psv = [[sa("psv0a"), sb("psv0b")], [p2("psv1a"), p2("psv1b")]]

                VOFF = 3
                for step in range(16 + VOFF):
                    if step < 16:
                        ib = step
                        for g in range(NKV):
                            for c in range(2):
                                nc.tensor.matmul(
                                    psk[g][c][:],
                                    wk_t[:, ib, g * 128:(g + 1) * 128],
                                    xt[ib][:, c * 512:(c + 1) * 512],
                                    start=(ib == 0), stop=(ib == 15))
                    if step >= VOFF:
                        ib = step - VOFF
                        for g in range(NKV):
                            for c in range(2):
                                nc.tensor.matmul(
                                    psv[g][c][:],
                                    wv_t[:, ib, g * 128:(g + 1) * 128],
                                    xt[ib][:, c * 512:(c + 1) * 512],
                                    start=(ib == 0), stop=(ib == 15))

                # PSUM->SBUF copies: v-g1 first (frees the P2 slots that
                # Qproj(0) needs), then interleaved kt/vt.
                vt_s = [ppro.tile([128, L], BF16, tag=f"vt_s{g}", name=f"vt{g}")
                        for g in range(NKV)]
                kt_s = [ppro.tile([128, L], F32R, tag=f"kt_s{g}", name=f"kt{g}")
                        for g in range(NKV)]
                for c in range(2):
                    nc.scalar.copy(vt_s[1][:, c * 512:(c + 1) * 512], psv[1][c][:])
                for g in range(NKV):
                    for c in range(2):
                        nc.scalar.copy(kt_s[g][:, c * 512:(c + 1) * 512],
                                       psk[g][c][:])
                for c in range(2):
                    nc.scalar.copy(vt_s[0][:, c * 512:(c + 1) * 512], psv[0][c][:])

                # Qproj(0): no dependency on the copies above except P2 slots;
                # covers the kt/vt copy chain on PE.
                qt_s0 = ppro.tile([128, L], F32R, tag="qt_s0", name="qt_s0")
                for cc in range(2):
                    psq = p2("psq0")
                    for ib in range(16):
                        nc.tensor.matmul(psq[:], wq_t0[:, ib, :],
                                         xt[ib][:, cc * 512:(cc + 1) * 512],
                                         start=(ib == 0), stop=(ib == 15))
                    nc.vector.tensor_copy(qt_s0[:, cc * 512:(cc + 1) * 512],
                                          psq[:])

                # rope-q0 chain starts immediately (DVE halves); the k-rope /
                # v-transpose / k-swap PE work below covers it.
                qrt0 = pp.tile([128, L], F32R, tag="qrt0", name="qrt0")
                qc0 = pp.tile([128, L], F32R, tag="qc0", name="qc0")
                for cc in range(2):
                    hs = slice(cc * 512, (cc + 1) * 512)
                    t1q = ppro.tile([128, 512], F32, tag="rq0", bufs=2)
                    t2q = ppro.tile([128, 512], F32, tag="rq0", bufs=2)
                    psr2 = p2("psr_q0")
                    nc.tensor.matmul(psr2[:], consts["pmrot"][:], qt_s0[:, hs])
                    nc.vector.tensor_mul(t1q[:], psr2[:], consts["sintab"][:, hs])
                    nc.vector.tensor_mul(t2q[:], qt_s0[:, hs].bitcast(F32),
                                         consts["costab"][:, hs])
                    nc.vector.tensor_add(qrt0[:, hs], t1q[:], t2q[:])
                    nc.vector.tensor_scalar_mul(qc0[:, hs],
                                                qrt0[:, hs].bitcast(F32),
                                                consts["sqrtc"][:, 0:1])

                # k rope rotate-half part (SA slots freed by the kt copies)
                for g in range(NKV):
                    t1 = ppro.tile([128, L], F32, tag="rtmp", bufs=4)
                    t2 = ppro.tile([128, L], F32, tag="rtmp", bufs=4)
                    for c in range(2):
                        psr = sa(f"psr_k{c}")
                        nc.tensor.matmul(psr[:],
                                         consts["pmrot"][:],
                                         kt_s[g][:, c * 512:(c + 1) * 512])
                        nc.vector.tensor_mul(
                            t1[:, c * 512:(c + 1) * 512], psr[:],
                            consts["sintab"][:, c * 512:(c + 1) * 512])
                    nc.gpsimd.tensor_mul(t2[:], kt_s[g][:].bitcast(F32),
                                         consts["costab"][:])
                    nc.vector.tensor_add(krt[g][:], t1[:], t2[:])

                # v transpose to [m, d] blocks
                for g in range(NKV):
                    for mb in range(NB):
                        pv = p2("pv", shape=[128, 128], dtype=BF16)
                        nc.tensor.transpose(pv[:], vt_s[g][:, mb * 128:(mb + 1) * 128],
                                            consts["identb"][:])
                        nc.vector.tensor_copy(vblk[g * NB + mb][:], pv[:])

                # k swap perms (krt chains complete under the ops above)
                for g in range(NKV):
                    for c in range(2):
                        psw = sb(f"psw_k{g}{c}")
                        nc.tensor.matmul(psw[:],
                                         consts["pmswap"][:],
                                         krt[g][:, c * 512:(c + 1) * 512])
                        nc.scalar.copy(kswap[g][:, c * 512:(c + 1) * 512],
                                       psw[:])

            # ---------------- head loop ----------------
            with tc.tile_pool(name="hl", bufs=1) as ph:
                etiles = {}

                wq_tiles = {}

                def wq_dma(h):
                    if h < NH:
                        wq_t = ph.tile([128, 16, 128], BF16, tag="wq_h", bufs=2,
                                       name=f"wq_t{h}")
                        nc.sync.dma_start(wq_t[:], wq[h])
                        wq_tiles[h] = wq_t

                def qproj(h):
                    wq_t = wq_tiles.pop(h)
                    qt_s = ph.tile([128, L], F32R, tag="qt_s", bufs=2)
                    for cc in range(2):
                        psq = p2("psq")
                        for ib in range(16):
                            nc.tensor.matmul(
                                psq[:],
                                wq_t[:, ib, :],
                                xt[ib][:, cc * 512:(cc + 1) * 512],
                                start=(ib == 0), stop=(ib == 15))
                        nc.vector.tensor_copy(qt_s[:, cc * 512:(cc + 1) * 512],
                                              psq[:])
                    return qt_s

                def rope_q_half(h, qt_s, qrt, qc, cc):
                    """one 512-half of q-rope + c'-scaled copy, on DVE"""
                    hs = slice(cc * 512, (cc + 1) * 512)
                    t1 = ph.tile([128, 512], F32, tag="qtmp", bufs=2)
                    t2 = ph.tile([128, 512], F32, tag="qtmp", bufs=2)
                    psr2 = p2("psr_q")
                    nc.tensor.matmul(psr2[:], consts["pmrot"][:], qt_s[:, hs])
                    nc.vector.tensor_mul(t1[:], psr2[:], consts["sintab"][:, hs])
                    nc.vector.tensor_mul(t2[:], qt_s[:, hs].bitcast(F32),
                                         consts["costab"][:, hs])
                    nc.vector.tensor_add(qrt[:, hs], t1[:], t2[:])
                    nc.vector.tensor_scalar_mul(qc[:, hs], qrt[:, hs].bitcast(F32),
                                                consts["sqrtc"][:, h:h + 1])

                def make_etiles(h):
                    ets = []
                    for mb in range(NB):
                        qlo = _stripe_qlo(mb)
                        et = ph.tile([128, L - qlo], BF16, tag=f"esc{mb}", bufs=2,
                                     name=f"esc_h{mb}")
                        ets.append(et)
                    etiles[h] = ets

                def score_group(h, qrt, qc, mb, qs, npair, bs_dve=False):
                    """npair chunks (1 or 2) of stripe mb starting at qs; one
                    fused exp over the pair."""
                    g = h // 4
                    qlo = _stripe_qlo(mb)
                    kb = slice(mb * 128, (mb + 1) * 128)
                    ets = etiles[h]
                    raw = ph.tile([128, 512], F32, tag="raw", bufs=4)
                    for j in range(npair):
                        cqs = qs + 256 * j
                        cqe = cqs + 256
                        psA = sa("psA")
                        psB = sb("psB")
                        nc.tensor.matmul(psA[:, 0:256], krt[g][0:64, kb],
                                         qrt[0:64, cqs:cqe])
                        nc.tensor.matmul(psA[:, 256:512], kswap[g][0:64, kb],
                                         qc[0:64, cqs:cqe])
                        nc.tensor.matmul(psB[:, 0:256], krt[g][64:128, kb],
                                         qrt[64:128, cqs:cqe])
                        nc.tensor.matmul(psB[:, 256:512], kswap[g][64:128, kb],
                                         qc[64:128, cqs:cqe])
                        bs = ph.tile([128, 512], F32, tag="bs", bufs=4)
                        if bs_dve and j == 0:
                            nc.vector.tensor_copy(bs[:], psB[:])
                        else:
                            nc.scalar.copy(bs[:], psB[:])
                        tp = ph.tile([128, 512], F32, tag="tprod", bufs=4)
                        nc.vector.tensor_mul(tp[:], psA[:], bs[:])
                        nc.gpsimd.tensor_add(raw[:, 256 * j:256 * (j + 1)],
                                             tp[:, 0:256], tp[:, 256:512])
                    w = 256 * npair
                    esl = ets[mb][:, qs - qlo: qs - qlo + w]
                    nc.scalar.activation(esl, raw[:, 0:w], Act.Exp,
                                         bias=consts["maskb"][:, mb:mb + 1],
                                         scale=consts["alpha"][:, h:h + 1])
                    if qs == qlo:
                        # causal mask on the diagonal 256 cols:
                        # keep where (qlo + col) - (128*mb + part) >= 0
                        nc.gpsimd.affine_select(
                            ets[mb][:, 0:256], ets[mb][:, 0:256],
                            pattern=[[1, 256]], compare_op=AluOp.is_ge,
                            fill=0.0, base=qlo - 128 * mb,
                            channel_multiplier=-1)

                def attnv_half(h, c, use_sasb=True):
                    g = h // 4
                    ets = etiles[h]
                    ps_o, ps_rs = sa("ps_o"), sb("ps_rs")
                    mbs = [mb for mb in range(NB) if 128 * mb < 512 * (c + 1)]
                    for i, mb in enumerate(mbs):
                        qlo = _stripe_qlo(mb)
                        os_ = max(512 * c, 128 * mb)
                        oe = 512 * (c + 1)
                        esl = ets[mb][:, os_ - qlo: oe - qlo]
                        st, sp = (i == 0), (i == len(mbs) - 1)
                        nc.tensor.matmul(ps_o[:, os_ - 512 * c: oe - 512 * c],
                                         vblk[g * NB + mb][:], esl,
                                         start=st, stop=sp)
                        nc.tensor.matmul(ps_rs[:, os_ - 512 * c: oe - 512 * c],
                                         consts["onesb"][:], esl,
                                         start=st, stop=sp)
                    rcp = ph.tile([128, 512], F32, tag="rcp", bufs=3)
                    nc.vector.reciprocal_approx_fast(rcp[:], ps_rs[:])
                    nc.vector.tensor_mul(outtn[h][:, c * 512:(c + 1) * 512],
                                         ps_o[:], rcp[:])

                # chunk groups (mb, qs, npair): a = groups with qs < 512 of
                # stripes 0-3 (cover attnv c=0), b = the rest
                a_set = [(0, 0, 2), (1, 0, 2), (2, 256, 1), (3, 256, 1)]
                b_set = [(0, 512, 2), (1, 512, 2), (2, 512, 2), (3, 512, 2),
                         (4, 512, 2), (5, 512, 2), (6, 768, 1), (7, 768, 1)]

                qrts = {0: (qrt0, qc0)}
                wq_dma(1)
                for h in range(NH):
                    make_etiles(h)
                    qrt, qc = qrts[h]
                    wq_dma(h + 2)
                    # Qproj(h+1) first: no dependency on head h's chains, so
                    # it covers the tail of head h-1's vector pipeline.
                    qt_n = qproj(h + 1) if h + 1 < NH else None
                    if h > 0:
                        attnv_half(h - 1, 1)
                    for g_ in a_set:
                        score_group(h, qrt, qc, *g_)
                    if qt_n is not None:
                        qrt_n = ph.tile([128, L], F32R, tag="qrt", bufs=2,
                                        name=f"qrt{h+1}")
                        qc_n = ph.tile([128, L], F32R, tag="qc", bufs=2,
                                       name=f"qc{h+1}")
                        qrts[h + 1] = (qrt_n, qc_n)
                        rope_q_half(h + 1, qt_n, qrt_n, qc_n, 0)
                    if h == NH - 1:
                        nc.sync.dma_start(wo_c0[:], wo[:, :, 0:512])
                    for i, g_ in enumerate(b_set):
                        if i == 1:
                            attnv_half(h, 0)
                        if i == 2 and qt_n is not None:
                            rope_q_half(h + 1, qt_n, qrt_n, qc_n, 1)
                        score_group(h, qrt, qc, *g_)
                    qrts.pop(h)

                # first epilogue blocks (heads 0-6) cover head 7's E tail,
                # attnv(7,1), and the outtn[7] norm; their hh=7 matmuls land
                # in the epilogue. psy2/psy3 borrow the idle SA/SB rings.
                psy0 = p2("psy")
                for hh in range(NH - 1):
                    nc.tensor.matmul(psy0[:],
                                     outtn[hh][:, 0:128], wo_c0[:, hh, :],
                                     start=(hh == 0), stop=False)
                attnv_half(NH - 1, 1, use_sasb=True)
                psy23 = [sa("psy2"), sb("psy3")]
                for j, psyx in enumerate(psy23):
                    for hh in range(NH - 1):
                        nc.tensor.matmul(psyx[:],
                                         outtn[hh][:, (2 + j) * 128:(3 + j) * 128],
                                         wo_c0[:, hh, :],
                                         start=(hh == 0), stop=False)

            # ---------------- epilogue: Wo projection ----------------
            with tc.tile_pool(name="ep", bufs=1) as pe:
                def ytile_out(psy, dc, lb):
                    yt = pe.tile([128, 512], F32, tag="ytile", bufs=3)
                    nc.scalar.copy(yt[:], psy[:])
                    nc.sync.dma_start(
                        y[lb * 128:(lb + 1) * 128, dc * 512:(dc + 1) * 512],
                        yt[:])

                # finish the pre-opened blocks: lb=1's heads 0-6 cover the
                # outtn[7] norm latency, then close lb=0..3.
                psy1 = p2("psy")
                for hh in range(NH - 1):
                    nc.tensor.matmul(psy1[:],
                                     outtn[hh][:, 128:256], wo_c0[:, hh, :],
                                     start=(hh == 0), stop=False)
                for psyx, lb in [(psy0, 0), (psy1, 1), (psy23[0], 2),
                                 (psy23[1], 3)]:
                    nc.tensor.matmul(psyx[:], outtn[NH - 1][:, lb * 128:(lb + 1) * 128],
                                     wo_c0[:, NH - 1, :], start=False, stop=True)
                    ytile_out(psyx, 0, lb)

                wo_ts = {0: wo_c0}
                for dc in range(4):
                    if dc + 1 < 4:
                        wo_n = pe.tile([128, NH, 512], BF16, tag="wo_c", bufs=2,
                                       name=f"wo_c{dc+1}")
                        nc.sync.dma_start(
                            wo_n[:], wo[:, :, (dc + 1) * 512:(dc + 2) * 512])
                        wo_ts[dc + 1] = wo_n
                    wo_t = wo_ts.pop(dc)
                    for lb in range(4 if dc == 0 else 0, NB):
                        psy = (sa if lb % 2 == 0 else sb)("psy")
                        for hh in range(NH):
                            nc.tensor.matmul(
                                psy[:],
                                outtn[hh][:, lb * 128:(lb + 1) * 128],
                                wo_t[:, hh, :],
                                start=(hh == 0), stop=(hh == NH - 1))
                        if dc == 3 and lb == NB - 1:
                            for half in range(2):
                                yt = pe.tile([128, 256], F32, tag="ytl", bufs=2)
                                nc.scalar.copy(yt[:], psy[:, half * 256:(half + 1) * 256])
                                nc.sync.dma_start(
                                    y[lb * 128:(lb + 1) * 128,
                                      dc * 512 + half * 256:dc * 512 + (half + 1) * 256],
                                    yt[:])
                        else:
                            ytile_out(psy, dc, lb)

    nc.compile()
    return nc


def _host_prep(x, Wq, Wk, Wv, Wo, q_param, log_scale, cos, sin, mask):
    """Build the 8 per-core input maps."""
    x = np.asarray(x, np.float32)
    Wq = np.asarray(Wq, np.float32)
    Wk = np.asarray(Wk, np.float32)
    Wv = np.asarray(Wv, np.float32)
    Wo = np.asarray(Wo, np.float32)
    cos = np.asarray(cos, np.float32)[0]      # [L, H, 64]
    sin = np.asarray(sin, np.float32)[0]
    qp = np.asarray(q_param, np.float32).reshape(H)
    ls = np.asarray(log_scale, np.float32).reshape(H)
    mask = np.asarray(mask)

    p64 = np.arange(128) % 64
    # rope tables are identical across heads: use head 0
    cos_p = np.ascontiguousarray(cos[:, 0, :][:, p64].T)   # [128, L]
    sin_p = np.ascontiguousarray(sin[:, 0, :][:, p64].T)

    PM = np.zeros((128, 128), np.float32)
    for dp in range(128):
        base, r = (dp // 64) * 64, dp % 64
        if r < 32:
            PM[base + r + 32, dp] = -1.0
        else:
            PM[base + r - 32, dp] = 1.0
    SW = np.zeros((128, 128), np.float32)
    for dp in range(128):
        SW[(dp + 64) % 128, dp] = 1.0
    ONES = np.ones((128, 128), ml_dtypes.bfloat16)
    IDENT = np.eye(128, dtype=ml_dtypes.bfloat16)

    in_maps = []
    for core in range(8):
        b, g2 = core // 2, core % 2
        heads = list(range(g2 * NH, (g2 + 1) * NH))

        xhv = np.ascontiguousarray(x[b].T).astype(ml_dtypes.bfloat16)

        wq_c = Wq[:, g2 * NH * 128:(g2 + 1) * NH * 128]
        wk_c = Wk[:, g2 * NKV * 128:(g2 + 1) * NKV * 128]
        wv_c = Wv[:, g2 * NKV * 128:(g2 + 1) * NKV * 128]
        wo_c = Wo[g2 * NH * 128:(g2 + 1) * NH * 128, :]

        wq_p = wq_c.reshape(16, 128, NH, 128).transpose(2, 1, 0, 3).copy()
        wk_p = wk_c.reshape(16, 128, NKV * 128).transpose(1, 0, 2).copy()
        wv_p = wv_c.reshape(16, 128, NKV * 128).transpose(1, 0, 2).copy()
        wo_p = wo_c.reshape(NH, 128, D).transpose(1, 0, 2).astype(ml_dtypes.bfloat16)

        mb = np.where(mask[b].reshape(NB, 128).T.astype(bool), 0.0, -1e9)
        mb = mb.astype(np.float32)

        cpr = -2.0 * np.tanh(qp[heads])               # per-head c'
        sq = np.sqrt(np.abs(cpr))
        sqc = np.tile(sq[None, :], (128, 1))
        sqc[:64, :] *= np.sign(cpr)[None, :]
        alp = np.tile((np.exp(ls[heads]) / HD)[None, :], (128, 1))

        in_maps.append({
            "xh": xhv,
            "wq": wq_p.astype(ml_dtypes.bfloat16),
            "wk": wk_p.astype(ml_dtypes.bfloat16),
            "wv": wv_p.astype(ml_dtypes.bfloat16), "wo": wo_p,
            "costab": cos_p, "sintab": sin_p,
            "maskb": mb, "sqrtc": sqc.astype(np.float32),
            "alpha": alp.astype(np.float32),
            "pmrot": PM, "pmswap": SW, "onesb": ONES, "identb": IDENT,
        })
    return in_maps


def kernel(**inputs):
    if "nc" not in _CACHED:
        _CACHED["nc"] = build_program()
    nc = _CACHED["nc"]
    in_maps = _host_prep(**inputs)
    res = run_bass_kernel_spmd(nc, in_maps, list(range(8))).results
    out = np.empty((B, L, D), np.float32)
    for b in range(B):
        out[b] = res[2 * b]["y"] + res[2 * b + 1]["y"]
    return out


# revision 10
# speedup vs baseline: 1.0970x; 1.0018x over previous
"""BivectorRotarySelfAttention TRN2 kernel, v3.

Sharding: 8 cores = 4 batches x 2 head-halves; host sums the two head-half
partial y's per batch.

v3 vs v2:
 - Score PSUM rings are 3-deep (SA/SB [128,512]x3) so the PE runs ~3 chunks
   ahead of the vector chain; all other PSUM users are [128,512] halves in a
   shared 2-deep ring (P2). 6+6+4 KB = 16 KB exactly.
 - Engine rebalance: rope t2/add and all raw-combines on Pool, 2 of 20 bs
   copies on DVE, rest on ACT.
 - Qproj+rope for head h+1 runs mid-scores(h); attnv halves are interleaved
   into the chunk stream so PE never waits on the exp chain.
 - V projection trails K by 4 ib-steps to match wv DMA arrival; wk DMA is
   issued before everything except nothing (first), consts after wv.
 - Epilogue prefetches wo and accumulates head 7 last.
"""
import sys
if '/opt/trn_rl_repo' not in sys.path:
    sys.path.insert(0, '/opt/trn_rl_repo')

import numpy as np
import ml_dtypes

import concourse.bass as bass
import concourse.mybir as mybir
import concourse.tile as tile
from concourse import bacc
from concourse.bass_utils import run_bass_kernel_spmd

F32 = mybir.dt.float32
F32R = mybir.dt.float32r
BF16 = mybir.dt.bfloat16

B, L, D, H, HKV = 4, 1024, 2048, 16, 4
HD = D // H            # 128
HD2 = HD // 2          # 64
NH = 8                 # heads per core
NKV = 2                # kv heads per core
NB = L // 128          # 8 blocks of 128
AluOp = mybir.AluOpType
Act = mybir.ActivationFunctionType

_CACHED = {}


def _stripe_qlo(mb):
    # stripe mb covers q in [qlo, L) in 256-wide chunks; odd stripes start one
    # 128-block early (the extra region is causal-masked to zero).
    return 128 * (mb - (mb % 2))


def build_program():
    nc = bacc.Bacc("TRN2", target_bir_lowering=False, debug=False)

    # ---- dram params (per-core shapes) ----
    xh = nc.declare_dram_parameter("xh", [D, L], BF16, isOutput=False)
    wq = nc.declare_dram_parameter("wq", [NH, 128, 16, 128], BF16, isOutput=False)
    wk = nc.declare_dram_parameter("wk", [128, 16, NKV * 128], BF16, isOutput=False)
    wv = nc.declare_dram_parameter("wv", [128, 16, NKV * 128], BF16, isOutput=False)
    wo = nc.declare_dram_parameter("wo", [128, NH, D], BF16, isOutput=False)
    costab = nc.declare_dram_parameter("costab", [128, L], F32, isOutput=False)
    sintab = nc.declare_dram_parameter("sintab", [128, L], F32, isOutput=False)
    maskb = nc.declare_dram_parameter("maskb", [128, NB], F32, isOutput=False)
    sqrtc = nc.declare_dram_parameter("sqrtc", [128, NH], F32, isOutput=False)
    alpha = nc.declare_dram_parameter("alpha", [128, NH], F32, isOutput=False)
    pmrot = nc.declare_dram_parameter("pmrot", [128, 128], F32R, isOutput=False)
    pmswap = nc.declare_dram_parameter("pmswap", [128, 128], F32R, isOutput=False)
    onesb = nc.declare_dram_parameter("onesb", [128, 128], BF16, isOutput=False)
    identb = nc.declare_dram_parameter("identb", [128, 128], BF16, isOutput=False)
    y = nc.declare_dram_parameter("y", [L, D], F32, isOutput=True)

    with tile.TileContext(nc) as tc:
        with (
            tc.tile_pool(name="persist", bufs=1) as pp,
            tc.tile_pool(name="psum", bufs=1, space="PSUM") as psp,
        ):
            # ---- persistent SBUF ----
            xt = [pp.tile([128, L], BF16, tag=f"xt{ib}", name=f"xt{ib}")
                  for ib in range(16)]
            krt = [pp.tile([128, L], F32R, tag=f"krt{g}", name=f"krt{g}")
                   for g in range(NKV)]
            kswap = [pp.tile([128, L], F32R, tag=f"ksw{g}", name=f"ksw{g}")
                     for g in range(NKV)]
            vblk = [pp.tile([128, 128], BF16, tag=f"vb{i}", name=f"vb{i}")
                    for i in range(NKV * NB)]
            outtn = [pp.tile([128, L], BF16, tag=f"ot{h}", name=f"ot{h}")
                     for h in range(NH)]
            wo_c0 = pp.tile([128, NH, 512], BF16, tag="wo_c0", name="wo_c0")
            consts = {}

            def sa(name):
                return psp.tile([128, 512], F32, tag="SA", bufs=3, name=name)

            def sb(name):
                return psp.tile([128, 512], F32, tag="SB", bufs=3, name=name)

            def p2(name, shape=None, dtype=F32):
                return psp.tile(shape or [128, 512], dtype, tag="P2", bufs=2,
                                name=name)

            # ---------------- prologue ----------------
            with tc.tile_pool(name="pro", bufs=1) as ppro:
                wk_t = ppro.tile([128, 16, NKV * 128], BF16, tag="wk")
                wv_t = ppro.tile([128, 16, NKV * 128], BF16, tag="wv")

                # DMA order: wk (halved), x0, wv, rest of x, consts.
                def xdma(ib):
                    nc.sync.dma_start(
                        xt[ib][:], xh[ib * 128:(ib + 1) * 128, :])

                wq_t0 = ppro.tile([128, 16, 128], BF16, tag="wq0", name="wq_t0")
                nc.sync.dma_start(wk_t[:, 0:8], wk[:, 0:8])
                xdma(0)
                nc.sync.dma_start(wk_t[:, 8:16], wk[:, 8:16])
                nc.sync.dma_start(wv_t[:], wv[:])
                for ib in range(1, 16):
                    xdma(ib)
                nc.sync.dma_start(wq_t0[:], wq[0])
                for nm, src, dt_ in [("pmrot", pmrot, F32R),
                                     ("costab", costab, F32),
                                     ("sintab", sintab, F32),
                                     ("identb", identb, BF16),
                                     ("pmswap", pmswap, F32R),
                                     ("onesb", onesb, BF16),
                                     ("maskb", maskb, F32),
                                     ("sqrtc", sqrtc, F32),
                                     ("alpha", alpha, F32)]:
                    t = pp.tile(list(src.shape), dt_, tag=nm, name=nm)
                    nc.sync.dma_start(t[:], src[:])
                    consts[nm] = t

                # K/V projection accumulators: [128,512] halves.
                psk = [[sa("psk0a"), sa("psk0b")], [sb("psk1a"), sb("psk1b")]]
                psv = [[sa("psv0a"), sb("psv0b")], [p2("psv1a"), p2("psv1b")]]

                VOFF = 3
                for step in range(16 + VOFF):
                    if step < 16:
                        ib = step
                        for g in range(NKV):
                            for c in range(2):
                                nc.tensor.matmul(
                                    psk[g][c][:],
                                    wk_t[:, ib, g * 128:(g + 1) * 128],
                                    xt[ib][:, c * 512:(c + 1) * 512],
                                    start=(ib == 0), stop=(ib == 15))
                    if step >= VOFF:
                        ib = step - VOFF
                        for g in range(NKV):
                            for c in range(2):
                                nc.tensor.matmul(
                                    psv[g][c][:],
                                    wv_t[:, ib, g * 128:(g + 1) * 128],
                                    xt[ib][:, c * 512:(c + 1) * 512],
                                    start=(ib == 0), stop=(ib == 15))

                # PSUM->SBUF copies: v-g1 first (frees the P2 slots that
                # Qproj(0) needs), then interleaved kt/vt.
                vt_s = [ppro.tile([128, L], BF16, tag=f"vt_s{g}", name=f"vt{g}")
                        for g in range(NKV)]
                kt_s = [ppro.tile([128, L], F32R, tag=f"kt_s{g}", name=f"kt{g}")
                        for g in range(NKV)]
                for c in range(2):
                    nc.scalar.copy(vt_s[1][:, c * 512:(c + 1) * 512], psv[1][c][:])
                for g in range(NKV):
                    for c in range(2):
                        nc.scalar.copy(kt_s[g][:, c * 512:(c + 1) * 512],
                                       psk[g][c][:])
                for c in range(2):
                    nc.scalar.copy(vt_s[0][:, c * 512:(c + 1) * 512], psv[0][c][:])

                # Qproj(0): no dependency on the copies above except P2 slots;
                # covers the kt/vt copy chain on PE.
                qt_s0 = ppro.tile([128, L], F32R, tag="qt_s0", name="qt_s0")
                for cc in range(2):
                    psq = p2("psq0")
                    for ib in range(16):
                        nc.tensor.matmul(psq[:], wq_t0[:, ib, :],
                                         xt[ib][:, cc * 512:(cc + 1) * 512],
                                         start=(ib == 0), stop=(ib == 15))
                    nc.vector.tensor_copy(qt_s0[:, cc * 512:(cc + 1) * 512],
                                          psq[:])

                # rope-q0 chain starts immediately (DVE halves); the k-rope /
                # v-transpose / k-swap PE work below covers it.
                qrt0 = pp.tile([128, L], F32R, tag="qrt0", name="qrt0")
                qc0 = pp.tile([128, L], F32R, tag="qc0", name="qc0")
                for cc in range(2):
                    hs = slice(cc * 512, (cc + 1) * 512)
                    t1q = ppro.tile([128, 512], F32, tag="rq0", bufs=2)
                    t2q = ppro.tile([128, 512], F32, tag="rq0", bufs=2)
                    psr2 = p2("psr_q0")
                    nc.tensor.matmul(psr2[:], consts["pmrot"][:], qt_s0[:, hs])
                    nc.vector.tensor_mul(t1q[:], psr2[:], consts["sintab"][:, hs])
                    nc.vector.tensor_mul(t2q[:], qt_s0[:, hs].bitcast(F32),
                                         consts["costab"][:, hs])
                    nc.vector.tensor_add(qrt0[:, hs], t1q[:], t2q[:])
                    nc.vector.tensor_scalar_mul(qc0[:, hs],
                                                qrt0[:, hs].bitcast(F32),
                                                consts["sqrtc"][:, 0:1])

                # k rope rotate-half part (SA slots freed by the kt copies)
                for g in range(NKV):
                    t1 = ppro.tile([128, L], F32, tag="rtmp", bufs=4)
                    t2 = ppro.tile([128, L], F32, tag="rtmp", bufs=4)
                    for c in range(2):
                        psr = sa(f"psr_k{c}")
                        nc.tensor.matmul(psr[:],
                                         consts["pmrot"][:],
                                         kt_s[g][:, c * 512:(c + 1) * 512])
                        nc.vector.tensor_mul(
                            t1[:, c * 512:(c + 1) * 512], psr[:],
                            consts["sintab"][:, c * 512:(c + 1) * 512])
                    nc.gpsimd.tensor_mul(t2[:], kt_s[g][:].bitcast(F32),
                                         consts["costab"][:])
                    nc.vector.tensor_add(krt[g][:], t1[:], t2[:])

                # v transpose to [m, d] blocks
                for g in range(NKV):
                    for mb in range(NB):
                        pv = p2("pv", shape=[128, 128], dtype=BF16)
                        nc.tensor.transpose(pv[:], vt_s[g][:, mb * 128:(mb + 1) * 128],
                                            consts["identb"][:])
                        nc.vector.tensor_copy(vblk[g * NB + mb][:], pv[:])

                # k swap perms (krt chains complete under the ops above)
                for g in range(NKV):
                    for c in range(2):
                        psw = sb(f"psw_k{g}{c}")
                        nc.tensor.matmul(psw[:],
                                         consts["pmswap"][:],
                                         krt[g][:, c * 512:(c + 1) * 512])
                        nc.scalar.copy(kswap[g][:, c * 512:(c + 1) * 512],
                                       psw[:])

            # ---------------- head loop ----------------
            with tc.tile_pool(name="hl", bufs=1) as ph:
                etiles = {}

                wq_tiles = {}

                def wq_dma(h):
                    if h < NH:
                        wq_t = ph.tile([128, 16, 128], BF16, tag="wq_h", bufs=2,
                                       name=f"wq_t{h}")
                        nc.sync.dma_start(wq_t[:], wq[h])
                        wq_tiles[h] = wq_t

                def qproj(h):
                    wq_t = wq_tiles.pop(h)
                    qt_s = ph.tile([128, L], F32R, tag="qt_s", bufs=2)
                    for cc in range(2):
                        psq = p2("psq")
                        for ib in range(16):
                            nc.tensor.matmul(
                                psq[:],
                                wq_t[:, ib, :],
                                xt[ib][:, cc * 512:(cc + 1) * 512],
                                start=(ib == 0), stop=(ib == 15))
                        nc.vector.tensor_copy(qt_s[:, cc * 512:(cc + 1) * 512],
                                              psq[:])
                    return qt_s

                def rope_q_half(h, qt_s, qrt, qc, cc):
                    """one 512-half of q-rope + c'-scaled copy, on DVE"""
                    hs = slice(cc * 512, (cc + 1) * 512)
                    t1 = ph.tile([128, 512], F32, tag="qtmp", bufs=2)
                    t2 = ph.tile([128, 512], F32, tag="qtmp", bufs=2)
                    psr2 = p2("psr_q")
                    nc.tensor.matmul(psr2[:], consts["pmrot"][:], qt_s[:, hs])
                    nc.vector.tensor_mul(t1[:], psr2[:], consts["sintab"][:, hs])
                    nc.vector.tensor_mul(t2[:], qt_s[:, hs].bitcast(F32),
                                         consts["costab"][:, hs])
                    nc.vector.tensor_add(qrt[:, hs], t1[:], t2[:])
                    nc.vector.tensor_scalar_mul(qc[:, hs], qrt[:, hs].bitcast(F32),
                                                consts["sqrtc"][:, h:h + 1])

                def make_etiles(h):
                    ets = []
                    for mb in range(NB):
                        qlo = _stripe_qlo(mb)
                        et = ph.tile([128, L - qlo], BF16, tag=f"esc{mb}", bufs=2,
                                     name=f"esc_h{mb}")
                        ets.append(et)
                    etiles[h] = ets

                def score_group(h, qrt, qc, mb, qs, npair, bs_dve=False):
                    """npair chunks (1 or 2) of stripe mb starting at qs; one
                    fused exp over the pair."""
                    g = h // 4
                    qlo = _stripe_qlo(mb)
                    kb = slice(mb * 128, (mb + 1) * 128)
                    ets = etiles[h]
                    raw = ph.tile([128, 512], F32, tag="raw", bufs=4)
                    for j in range(npair):
                        cqs = qs + 256 * j
                        cqe = cqs + 256
                        psA = sa("psA")
                        psB = sb("psB")
                        nc.tensor.matmul(psA[:, 0:256], krt[g][0:64, kb],
                                         qrt[0:64, cqs:cqe])
                        nc.tensor.matmul(psA[:, 256:512], kswap[g][0:64, kb],
                                         qc[0:64, cqs:cqe])
                        nc.tensor.matmul(psB[:, 0:256], krt[g][64:128, kb],
                                         qrt[64:128, cqs:cqe])
                        nc.tensor.matmul(psB[:, 256:512], kswap[g][64:128, kb],
                                         qc[64:128, cqs:cqe])
                        bs = ph.tile([128, 512], F32, tag="bs", bufs=4)
                        if bs_dve and j == 0:
                            nc.vector.tensor_copy(bs[:], psB[:])
                        else:
                            nc.scalar.copy(bs[:], psB[:])
                        tp = ph.tile([128, 512], F32, tag="tprod", bufs=4)
                        nc.vector.tensor_mul(tp[:], psA[:], bs[:])
                        nc.gpsimd.tensor_add(raw[:, 256 * j:256 * (j + 1)],
                                             tp[:, 0:256], tp[:, 256:512])
                    w = 256 * npair
                    esl = ets[mb][:, qs - qlo: qs - qlo + w]
                    nc.scalar.activation(esl, raw[:, 0:w], Act.Exp,
                                         bias=consts["maskb"][:, mb:mb + 1],
                                         scale=consts["alpha"][:, h:h + 1])
                    if qs == qlo:
                        # causal mask on the diagonal 256 cols:
                        # keep where (qlo + col) - (128*mb + part) >= 0
                        nc.gpsimd.affine_select(
                            ets[mb][:, 0:256], ets[mb][:, 0:256],
                            pattern=[[1, 256]], compare_op=AluOp.is_ge,
                            fill=0.0, base=qlo - 128 * mb,
                            channel_multiplier=-1)

                def attnv_half(h, c, use_sasb=True):
                    g = h // 4
                    ets = etiles[h]
                    ps_o, ps_rs = sa("ps_o"), sb("ps_rs")
                    mbs = [mb for mb in range(NB) if 128 * mb < 512 * (c + 1)]
                    for i, mb in enumerate(mbs):
                        qlo = _stripe_qlo(mb)
                        os_ = max(512 * c, 128 * mb)
                        oe = 512 * (c + 1)
                        esl = ets[mb][:, os_ - qlo: oe - qlo]
                        st, sp = (i == 0), (i == len(mbs) - 1)
                        nc.tensor.matmul(ps_o[:, os_ - 512 * c: oe - 512 * c],
                                         vblk[g * NB + mb][:], esl,
                                         start=st, stop=sp)
                        nc.tensor.matmul(ps_rs[:, os_ - 512 * c: oe - 512 * c],
                                         consts["onesb"][:], esl,
                                         start=st, stop=sp)
                    rcp = ph.tile([128, 512], F32, tag="rcp", bufs=3)
                    nc.vector.reciprocal_approx_fast(rcp[:], ps_rs[:])
                    nc.vector.tensor_mul(outtn[h][:, c * 512:(c + 1) * 512],
                                         ps_o[:], rcp[:])

                # chunk groups (mb, qs, npair): a = groups with qs < 512 of
                # stripes 0-3 (cover attnv c=0), b = the rest
                a_set = [(0, 0, 2), (1, 0, 2), (2, 256, 1), (3, 256, 1)]
                b_set = [(0, 512, 2), (1, 512, 2), (2, 512, 2), (3, 512, 2),
                         (4, 512, 2), (5, 512, 2), (6, 768, 1), (7, 768, 1)]

                qrts = {0: (qrt0, qc0)}
                wq_dma(1)
                for h in range(NH):
                    make_etiles(h)
                    qrt, qc = qrts[h]
                    wq_dma(h + 2)
                    # Qproj(h+1) first: no dependency on head h's chains, so
                    # it covers the tail of head h-1's vector pipeline.
                    qt_n = qproj(h + 1) if h + 1 < NH else None
                    if h > 0:
                        attnv_half(h - 1, 1)
                    for g_ in a_set:
                        score_group(h, qrt, qc, *g_)
                    if qt_n is not None:
                        qrt_n = ph.tile([128, L], F32R, tag="qrt", bufs=2,
                                        name=f"qrt{h+1}")
                        qc_n = ph.tile([128, L], F32R, tag="qc", bufs=2,
                                       name=f"qc{h+1}")
                        qrts[h + 1] = (qrt_n, qc_n)
                        rope_q_half(h + 1, qt_n, qrt_n, qc_n, 0)
                    if h == NH - 1:
                        nc.sync.dma_start(wo_c0[:], wo[:, :, 0:512])
                    attnv0_at = 2 if h + 1 < NH else 4
                    for i, g_ in enumerate(b_set):
                        if i == attnv0_at:
                            attnv_half(h, 0)
                        if i == 2 and qt_n is not None:
                            rope_q_half(h + 1, qt_n, qrt_n, qc_n, 1)
                        score_group(h, qrt, qc, *g_)
                    qrts.pop(h)

                # first epilogue blocks (heads 0-6) cover head 7's E tail,
                # attnv(7,1), and the outtn[7] norm; their hh=7 matmuls land
                # in the epilogue. psy2/psy3 borrow the idle SA/SB rings.
                psy0 = p2("psy")
                for hh in range(NH - 1):
                    nc.tensor.matmul(psy0[:],
                                     outtn[hh][:, 0:128], wo_c0[:, hh, :],
                                     start=(hh == 0), stop=False)
                attnv_half(NH - 1, 1, use_sasb=True)
                psy23 = [sa("psy2"), sb("psy3")]
                for j, psyx in enumerate(psy23):
                    for hh in range(NH - 1):
                        nc.tensor.matmul(psyx[:],
                                         outtn[hh][:, (2 + j) * 128:(3 + j) * 128],
                                         wo_c0[:, hh, :],
                                         start=(hh == 0), stop=False)

            # ---------------- epilogue: Wo projection ----------------
            with tc.tile_pool(name="ep", bufs=1) as pe:
                def ytile_out(psy, dc, lb):
                    yt = pe.tile([128, 512], F32, tag="ytile", bufs=3)
                    nc.scalar.copy(yt[:], psy[:])
                    nc.sync.dma_start(
                        y[lb * 128:(lb + 1) * 128, dc * 512:(dc + 1) * 512],
                        yt[:])

                # finish the pre-opened blocks: lb=1's heads 0-6 cover the
                # outtn[7] norm latency, then close lb=0..3.
                psy1 = p2("psy")
                for hh in range(NH - 1):
                    nc.tensor.matmul(psy1[:],
                                     outtn[hh][:, 128:256], wo_c0[:, hh, :],
                                     start=(hh == 0), stop=False)
                for psyx, lb in [(psy0, 0), (psy1, 1), (psy23[0], 2),
                                 (psy23[1], 3)]:
                    nc.tensor.matmul(psyx[:], outtn[NH - 1][:, lb * 128:(lb + 1) * 128],
                                     wo_c0[:, NH - 1, :], start=False, stop=True)
                    ytile_out(psyx, 0, lb)

                wo_ts = {0: wo_c0}
                for dc in range(4):
                    if dc + 1 < 4:
                        wo_n = pe.tile([128, NH, 512], BF16, tag="wo_c", bufs=2,
                                       name=f"wo_c{dc+1}")
                        nc.sync.dma_start(
                            wo_n[:], wo[:, :, (dc + 1) * 512:(dc + 2) * 512])
                        wo_ts[dc + 1] = wo_n
                    wo_t = wo_ts.pop(dc)
                    for lb in range(4 if dc == 0 else 0, NB):
                        psy = (sa if lb % 2 == 0 else sb)("psy")
                        for hh in range(NH):
                            nc.tensor.matmul(
                                psy[:],
                                outtn[hh][:, lb * 128:(lb + 1) * 128],
                                wo_t[:, hh, :],
                                start=(hh == 0), stop=(hh == NH - 1))
                        if dc == 3 and lb == NB - 1:
                            for half in range(2):
                                yt = pe.tile([128, 256], F32, tag="ytl", bufs=2)
                                nc.scalar.copy(yt[:], psy[:, half * 256:(half + 1) * 256])
                                nc.sync.dma_start(
                                    y[lb * 128:(lb + 1) * 128,
                                      dc * 512 + half * 256:dc * 512 + (half + 1) * 256],
                                    yt[:])
                        else:
                            ytile_out(psy, dc, lb)

    nc.compile()
    return nc


def _host_prep(x, Wq, Wk, Wv, Wo, q_param, log_scale, cos, sin, mask):
    """Build the 8 per-core input maps."""
    x = np.asarray(x, np.float32)
    Wq = np.asarray(Wq, np.float32)
    Wk = np.asarray(Wk, np.float32)
    Wv = np.asarray(Wv, np.float32)
    Wo = np.asarray(Wo, np.float32)
    cos = np.asarray(cos, np.float32)[0]      # [L, H, 64]
    sin = np.asarray(sin, np.float32)[0]
    qp = np.asarray(q_param, np.float32).reshape(H)
    ls = np.asarray(log_scale, np.float32).reshape(H)
    mask = np.asarray(mask)

    p64 = np.arange(128) % 64
    # rope tables are identical across heads: use head 0
    cos_p = np.ascontiguousarray(cos[:, 0, :][:, p64].T)   # [128, L]
    sin_p = np.ascontiguousarray(sin[:, 0, :][:, p64].T)

    PM = np.zeros((128, 128), np.float32)
    for dp in range(128):
        base, r = (dp // 64) * 64, dp % 64
        if r < 32:
            PM[base + r + 32, dp] = -1.0
        else:
            PM[base + r - 32, dp] = 1.0
    SW = np.zeros((128, 128), np.float32)
    for dp in range(128):
        SW[(dp + 64) % 128, dp] = 1.0
    ONES = np.ones((128, 128), ml_dtypes.bfloat16)
    IDENT = np.eye(128, dtype=ml_dtypes.bfloat16)

    in_maps = []
    for core in range(8):
        b, g2 = core // 2, core % 2
        heads = list(range(g2 * NH, (g2 + 1) * NH))

        xhv = np.ascontiguousarray(x[b].T).astype(ml_dtypes.bfloat16)

        wq_c = Wq[:, g2 * NH * 128:(g2 + 1) * NH * 128]
        wk_c = Wk[:, g2 * NKV * 128:(g2 + 1) * NKV * 128]
        wv_c = Wv[:, g2 * NKV * 128:(g2 + 1) * NKV * 128]
        wo_c = Wo[g2 * NH * 128:(g2 + 1) * NH * 128, :]

        wq_p = wq_c.reshape(16, 128, NH, 128).transpose(2, 1, 0, 3).copy()
        wk_p = wk_c.reshape(16, 128, NKV * 128).transpose(1, 0, 2).copy()
        wv_p = wv_c.reshape(16, 128, NKV * 128).transpose(1, 0, 2).copy()
        wo_p = wo_c.reshape(NH, 128, D).transpose(1, 0, 2).astype(ml_dtypes.bfloat16)

        mb = np.where(mask[b].reshape(NB, 128).T.astype(bool), 0.0, -1e9)
        mb = mb.astype(np.float32)

        cpr = -2.0 * np.tanh(qp[heads])               # per-head c'
        sq = np.sqrt(np.abs(cpr))
        sqc = np.tile(sq[None, :], (128, 1))
        sqc[:64, :] *= np.sign(cpr)[None, :]
        alp = np.tile((np.exp(ls[heads]) / HD)[None, :], (128, 1))

        in_maps.append({
            "xh": xhv,
            "wq": wq_p.astype(ml_dtypes.bfloat16),
            "wk": wk_p.astype(ml_dtypes.bfloat16),
            "wv": wv_p.astype(ml_dtypes.bfloat16), "wo": wo_p,
            "costab": cos_p, "sintab": sin_p,
            "maskb": mb, "sqrtc": sqc.astype(np.float32),
            "alpha": alp.astype(np.float32),
            "pmrot": PM, "pmswap": SW, "onesb": ONES, "identb": IDENT,
        })
    return in_maps


def kernel(**inputs):
    if "nc" not in _CACHED:
        _CACHED["nc"] = build_program()
    nc = _CACHED["nc"]
    in_maps = _host_prep(**inputs)
    res = run_bass_kernel_spmd(nc, in_maps, list(range(8))).results
    out = np.empty((B, L, D), np.float32)
    for b in range(B):
        out[b] = res[2 * b]["y"] + res[2 * b + 1]["y"]
    return out


# revision 11
# speedup vs baseline: 1.0978x; 1.0007x over previous
"""BivectorRotarySelfAttention TRN2 kernel, v3.

Sharding: 8 cores = 4 batches x 2 head-halves; host sums the two head-half
partial y's per batch.

v3 vs v2:
 - Score PSUM rings are 3-deep (SA/SB [128,512]x3) so the PE runs ~3 chunks
   ahead of the vector chain; all other PSUM users are [128,512] halves in a
   shared 2-deep ring (P2). 6+6+4 KB = 16 KB exactly.
 - Engine rebalance: rope t2/add and all raw-combines on Pool, 2 of 20 bs
   copies on DVE, rest on ACT.
 - Qproj+rope for head h+1 runs mid-scores(h); attnv halves are interleaved
   into the chunk stream so PE never waits on the exp chain.
 - V projection trails K by 4 ib-steps to match wv DMA arrival; wk DMA is
   issued before everything except nothing (first), consts after wv.
 - Epilogue prefetches wo and accumulates head 7 last.
"""
import sys
if '/opt/trn_rl_repo' not in sys.path:
    sys.path.insert(0, '/opt/trn_rl_repo')

import numpy as np
import ml_dtypes

import concourse.bass as bass
import concourse.mybir as mybir
import concourse.tile as tile
from concourse import bacc
from concourse.bass_utils import run_bass_kernel_spmd

F32 = mybir.dt.float32
F32R = mybir.dt.float32r
BF16 = mybir.dt.bfloat16

B, L, D, H, HKV = 4, 1024, 2048, 16, 4
HD = D // H            # 128
HD2 = HD // 2          # 64
NH = 8                 # heads per core
NKV = 2                # kv heads per core
NB = L // 128          # 8 blocks of 128
AluOp = mybir.AluOpType
Act = mybir.ActivationFunctionType

_CACHED = {}


def _stripe_qlo(mb):
    # stripe mb covers q in [qlo, L) in 256-wide chunks; odd stripes start one
    # 128-block early (the extra region is causal-masked to zero).
    return 128 * (mb - (mb % 2))


def build_program():
    nc = bacc.Bacc("TRN2", target_bir_lowering=False, debug=False)

    # ---- dram params (per-core shapes) ----
    xh = nc.declare_dram_parameter("xh", [D, L], BF16, isOutput=False)
    wq = nc.declare_dram_parameter("wq", [NH, 128, 16, 128], BF16, isOutput=False)
    wk = nc.declare_dram_parameter("wk", [128, 16, NKV * 128], BF16, isOutput=False)
    wv = nc.declare_dram_parameter("wv", [128, 16, NKV * 128], BF16, isOutput=False)
    wo = nc.declare_dram_parameter("wo", [128, NH, D], BF16, isOutput=False)
    costab = nc.declare_dram_parameter("costab", [128, L], F32, isOutput=False)
    sintab = nc.declare_dram_parameter("sintab", [128, L], F32, isOutput=False)
    maskb = nc.declare_dram_parameter("maskb", [128, NB], F32, isOutput=False)
    sqrtc = nc.declare_dram_parameter("sqrtc", [128, NH], F32, isOutput=False)
    alpha = nc.declare_dram_parameter("alpha", [128, NH], F32, isOutput=False)
    pmrot = nc.declare_dram_parameter("pmrot", [128, 128], F32R, isOutput=False)
    pmswap = nc.declare_dram_parameter("pmswap", [128, 128], F32R, isOutput=False)
    onesb = nc.declare_dram_parameter("onesb", [128, 128], BF16, isOutput=False)
    identb = nc.declare_dram_parameter("identb", [128, 128], BF16, isOutput=False)
    y = nc.declare_dram_parameter("y", [L, D], F32, isOutput=True)

    with tile.TileContext(nc) as tc:
        with (
            tc.tile_pool(name="persist", bufs=1) as pp,
            tc.tile_pool(name="psum", bufs=1, space="PSUM") as psp,
        ):
            # ---- persistent SBUF ----
            xt = [pp.tile([128, L], BF16, tag=f"xt{ib}", name=f"xt{ib}")
                  for ib in range(16)]
            krt = [pp.tile([128, L], F32R, tag=f"krt{g}", name=f"krt{g}")
                   for g in range(NKV)]
            kswap = [pp.tile([128, L], F32R, tag=f"ksw{g}", name=f"ksw{g}")
                     for g in range(NKV)]
            vblk = [pp.tile([128, 128], BF16, tag=f"vb{i}", name=f"vb{i}")
                    for i in range(NKV * NB)]
            outtn = [pp.tile([128, L], BF16, tag=f"ot{h}", name=f"ot{h}")
                     for h in range(NH)]
            wo_c0 = pp.tile([128, NH, 512], BF16, tag="wo_c0", name="wo_c0")
            consts = {}

            def sa(name):
                return psp.tile([128, 512], F32, tag="SA", bufs=3, name=name)

            def sb(name):
                return psp.tile([128, 512], F32, tag="SB", bufs=3, name=name)

            def p2(name, shape=None, dtype=F32):
                return psp.tile(shape or [128, 512], dtype, tag="P2", bufs=2,
                                name=name)

            # ---------------- prologue ----------------
            with tc.tile_pool(name="pro", bufs=1) as ppro:
                wk_t = ppro.tile([128, 16, NKV * 128], BF16, tag="wk")
                wv_t = ppro.tile([128, 16, NKV * 128], BF16, tag="wv")

                # DMA order: wk (halved), x0, wv, rest of x, consts.
                def xdma(ib):
                    nc.sync.dma_start(
                        xt[ib][:], xh[ib * 128:(ib + 1) * 128, :])

                wq_t0 = ppro.tile([128, 16, 128], BF16, tag="wq0", name="wq_t0")
                nc.sync.dma_start(wk_t[:, 0:8], wk[:, 0:8])
                xdma(0)
                nc.sync.dma_start(wk_t[:, 8:16], wk[:, 8:16])
                nc.sync.dma_start(wv_t[:], wv[:])
                for ib in range(1, 16):
                    xdma(ib)
                nc.sync.dma_start(wq_t0[:], wq[0])
                for nm, src, dt_ in [("pmrot", pmrot, F32R),
                                     ("costab", costab, F32),
                                     ("sintab", sintab, F32),
                                     ("identb", identb, BF16),
                                     ("pmswap", pmswap, F32R),
                                     ("onesb", onesb, BF16),
                                     ("maskb", maskb, F32),
                                     ("sqrtc", sqrtc, F32),
                                     ("alpha", alpha, F32)]:
                    t = pp.tile(list(src.shape), dt_, tag=nm, name=nm)
                    nc.sync.dma_start(t[:], src[:])
                    consts[nm] = t

                # K/V projection accumulators: [128,512] halves.
                psk = [[sa("psk0a"), sa("psk0b")], [sb("psk1a"), sb("psk1b")]]
                psv = [[sa("psv0a"), sb("psv0b")], [p2("psv1a"), p2("psv1b")]]

                VOFF = 3
                for step in range(16 + VOFF):
                    if step < 16:
                        ib = step
                        for g in range(NKV):
                            for c in range(2):
                                nc.tensor.matmul(
                                    psk[g][c][:],
                                    wk_t[:, ib, g * 128:(g + 1) * 128],
                                    xt[ib][:, c * 512:(c + 1) * 512],
                                    start=(ib == 0), stop=(ib == 15))
                    if step >= VOFF:
                        ib = step - VOFF
                        for g in range(NKV):
                            for c in range(2):
                                nc.tensor.matmul(
                                    psv[g][c][:],
                                    wv_t[:, ib, g * 128:(g + 1) * 128],
                                    xt[ib][:, c * 512:(c + 1) * 512],
                                    start=(ib == 0), stop=(ib == 15))

                # PSUM->SBUF copies: v-g1 first (frees the P2 slots that
                # Qproj(0) needs), then interleaved kt/vt.
                vt_s = [ppro.tile([128, L], BF16, tag=f"vt_s{g}", name=f"vt{g}")
                        for g in range(NKV)]
                kt_s = [ppro.tile([128, L], F32R, tag=f"kt_s{g}", name=f"kt{g}")
                        for g in range(NKV)]
                for c in range(2):
                    nc.scalar.copy(vt_s[1][:, c * 512:(c + 1) * 512], psv[1][c][:])
                for g in range(NKV):
                    for c in range(2):
                        nc.scalar.copy(kt_s[g][:, c * 512:(c + 1) * 512],
                                       psk[g][c][:])
                for c in range(2):
                    nc.scalar.copy(vt_s[0][:, c * 512:(c + 1) * 512], psv[0][c][:])

                # Qproj(0): no dependency on the copies above except P2 slots;
                # covers the kt/vt copy chain on PE.
                qt_s0 = ppro.tile([128, L], F32R, tag="qt_s0", name="qt_s0")
                for cc in range(2):
                    psq = p2("psq0")
                    for ib in range(16):
                        nc.tensor.matmul(psq[:], wq_t0[:, ib, :],
                                         xt[ib][:, cc * 512:(cc + 1) * 512],
                                         start=(ib == 0), stop=(ib == 15))
                    nc.vector.tensor_copy(qt_s0[:, cc * 512:(cc + 1) * 512],
                                          psq[:])

                # rope-q0 chain starts immediately (DVE halves); the k-rope /
                # v-transpose / k-swap PE work below covers it.
                qrt0 = pp.tile([128, L], F32R, tag="qrt0", name="qrt0")
                qc0 = pp.tile([128, L], F32R, tag="qc0", name="qc0")
                for cc in range(2):
                    hs = slice(cc * 512, (cc + 1) * 512)
                    t1q = ppro.tile([128, 512], F32, tag="rq0", bufs=2)
                    t2q = ppro.tile([128, 512], F32, tag="rq0", bufs=2)
                    psr2 = p2("psr_q0")
                    nc.tensor.matmul(psr2[:], consts["pmrot"][:], qt_s0[:, hs])
                    nc.vector.tensor_mul(t1q[:], psr2[:], consts["sintab"][:, hs])
                    nc.vector.tensor_mul(t2q[:], qt_s0[:, hs].bitcast(F32),
                                         consts["costab"][:, hs])
                    nc.vector.tensor_add(qrt0[:, hs], t1q[:], t2q[:])
                    nc.vector.tensor_scalar_mul(qc0[:, hs],
                                                qrt0[:, hs].bitcast(F32),
                                                consts["sqrtc"][:, 0:1])

                # k rope rotate-half part (SA slots freed by the kt copies)
                for g in range(NKV):
                    t1 = ppro.tile([128, L], F32, tag="rtmp", bufs=4)
                    t2 = ppro.tile([128, L], F32, tag="rtmp", bufs=4)
                    for c in range(2):
                        psr = sa(f"psr_k{c}")
                        nc.tensor.matmul(psr[:],
                                         consts["pmrot"][:],
                                         kt_s[g][:, c * 512:(c + 1) * 512])
                        nc.vector.tensor_mul(
                            t1[:, c * 512:(c + 1) * 512], psr[:],
                            consts["sintab"][:, c * 512:(c + 1) * 512])
                    nc.gpsimd.tensor_mul(t2[:], kt_s[g][:].bitcast(F32),
                                         consts["costab"][:])
                    nc.vector.tensor_add(krt[g][:], t1[:], t2[:])

                # v transpose to [m, d] blocks
                for g in range(NKV):
                    for mb in range(NB):
                        pv = p2("pv", shape=[128, 128], dtype=BF16)
                        nc.tensor.transpose(pv[:], vt_s[g][:, mb * 128:(mb + 1) * 128],
                                            consts["identb"][:])
                        nc.vector.tensor_copy(vblk[g * NB + mb][:], pv[:])

                # k swap perms (krt chains complete under the ops above)
                for g in range(NKV):
                    for c in range(2):
                        psw = sb(f"psw_k{g}{c}")
                        nc.tensor.matmul(psw[:],
                                         consts["pmswap"][:],
                                         krt[g][:, c * 512:(c + 1) * 512])
                        nc.scalar.copy(kswap[g][:, c * 512:(c + 1) * 512],
                                       psw[:])

            # ---------------- head loop ----------------
            with tc.tile_pool(name="hl", bufs=1) as ph:
                etiles = {}

                wq_tiles = {}

                def wq_dma(h):
                    if h < NH:
                        wq_t = ph.tile([128, 16, 128], BF16, tag="wq_h", bufs=2,
                                       name=f"wq_t{h}")
                        nc.sync.dma_start(wq_t[:], wq[h])
                        wq_tiles[h] = wq_t

                def qproj(h):
                    wq_t = wq_tiles.pop(h)
                    qt_s = ph.tile([128, L], F32R, tag="qt_s", bufs=2)
                    for cc in range(2):
                        psq = p2("psq")
                        for ib in range(16):
                            nc.tensor.matmul(
                                psq[:],
                                wq_t[:, ib, :],
                                xt[ib][:, cc * 512:(cc + 1) * 512],
                                start=(ib == 0), stop=(ib == 15))
                        nc.vector.tensor_copy(qt_s[:, cc * 512:(cc + 1) * 512],
                                              psq[:])
                    return qt_s

                def rope_q_half(h, qt_s, qrt, qc, cc):
                    """one 512-half of q-rope + c'-scaled copy, on DVE"""
                    hs = slice(cc * 512, (cc + 1) * 512)
                    t1 = ph.tile([128, 512], F32, tag="qtmp", bufs=2)
                    t2 = ph.tile([128, 512], F32, tag="qtmp", bufs=2)
                    psr2 = p2("psr_q")
                    nc.tensor.matmul(psr2[:], consts["pmrot"][:], qt_s[:, hs])
                    nc.vector.tensor_mul(t1[:], psr2[:], consts["sintab"][:, hs])
                    nc.vector.tensor_mul(t2[:], qt_s[:, hs].bitcast(F32),
                                         consts["costab"][:, hs])
                    nc.vector.tensor_add(qrt[:, hs], t1[:], t2[:])
                    nc.vector.tensor_scalar_mul(qc[:, hs], qrt[:, hs].bitcast(F32),
                                                consts["sqrtc"][:, h:h + 1])

                def make_etiles(h):
                    ets = []
                    for mb in range(NB):
                        qlo = _stripe_qlo(mb)
                        et = ph.tile([128, L - qlo], BF16, tag=f"esc{mb}", bufs=2,
                                     name=f"esc_h{mb}")
                        ets.append(et)
                    etiles[h] = ets

                def score_group(h, qrt, qc, mb, qs, npair, bs_dve=False):
                    """npair chunks (1 or 2) of stripe mb starting at qs; one
                    fused exp over the pair."""
                    g = h // 4
                    qlo = _stripe_qlo(mb)
                    kb = slice(mb * 128, (mb + 1) * 128)
                    ets = etiles[h]
                    raw = ph.tile([128, 512], F32, tag="raw", bufs=5)
                    for j in range(npair):
                        cqs = qs + 256 * j
                        cqe = cqs + 256
                        psA = sa("psA")
                        psB = sb("psB")
                        nc.tensor.matmul(psA[:, 0:256], krt[g][0:64, kb],
                                         qrt[0:64, cqs:cqe])
                        nc.tensor.matmul(psA[:, 256:512], kswap[g][0:64, kb],
                                         qc[0:64, cqs:cqe])
                        nc.tensor.matmul(psB[:, 0:256], krt[g][64:128, kb],
                                         qrt[64:128, cqs:cqe])
                        nc.tensor.matmul(psB[:, 256:512], kswap[g][64:128, kb],
                                         qc[64:128, cqs:cqe])
                        bs = ph.tile([128, 512], F32, tag="bs", bufs=4)
                        if bs_dve and j == 0:
                            nc.vector.tensor_copy(bs[:], psB[:])
                        else:
                            nc.scalar.copy(bs[:], psB[:])
                        tp = ph.tile([128, 512], F32, tag="tprod", bufs=5)
                        nc.vector.tensor_mul(tp[:], psA[:], bs[:])
                        nc.gpsimd.tensor_add(raw[:, 256 * j:256 * (j + 1)],
                                             tp[:, 0:256], tp[:, 256:512])
                    w = 256 * npair
                    esl = ets[mb][:, qs - qlo: qs - qlo + w]
                    nc.scalar.activation(esl, raw[:, 0:w], Act.Exp,
                                         bias=consts["maskb"][:, mb:mb + 1],
                                         scale=consts["alpha"][:, h:h + 1])
                    if qs == qlo:
                        # causal mask on the diagonal 256 cols:
                        # keep where (qlo + col) - (128*mb + part) >= 0
                        nc.gpsimd.affine_select(
                            ets[mb][:, 0:256], ets[mb][:, 0:256],
                            pattern=[[1, 256]], compare_op=AluOp.is_ge,
                            fill=0.0, base=qlo - 128 * mb,
                            channel_multiplier=-1)

                def attnv_half(h, c, use_sasb=True):
                    g = h // 4
                    ets = etiles[h]
                    ps_o, ps_rs = sa("ps_o"), sb("ps_rs")
                    mbs = [mb for mb in range(NB) if 128 * mb < 512 * (c + 1)]
                    for i, mb in enumerate(mbs):
                        qlo = _stripe_qlo(mb)
                        os_ = max(512 * c, 128 * mb)
                        oe = 512 * (c + 1)
                        esl = ets[mb][:, os_ - qlo: oe - qlo]
                        st, sp = (i == 0), (i == len(mbs) - 1)
                        nc.tensor.matmul(ps_o[:, os_ - 512 * c: oe - 512 * c],
                                         vblk[g * NB + mb][:], esl,
                                         start=st, stop=sp)
                        nc.tensor.matmul(ps_rs[:, os_ - 512 * c: oe - 512 * c],
                                         consts["onesb"][:], esl,
                                         start=st, stop=sp)
                    rcp = ph.tile([128, 512], F32, tag="rcp", bufs=3)
                    nc.vector.reciprocal_approx_fast(rcp[:], ps_rs[:])
                    nc.vector.tensor_mul(outtn[h][:, c * 512:(c + 1) * 512],
                                         ps_o[:], rcp[:])

                # chunk groups (mb, qs, npair): a = groups with qs < 512 of
                # stripes 0-3 (cover attnv c=0), b = the rest
                a_set = [(0, 0, 2), (1, 0, 2), (2, 256, 1), (3, 256, 1)]
                b_set = [(0, 512, 2), (1, 512, 2), (2, 512, 2), (3, 512, 2),
                         (4, 512, 2), (5, 512, 2), (6, 768, 1), (7, 768, 1)]

                qrts = {0: (qrt0, qc0)}
                wq_dma(1)
                for h in range(NH):
                    make_etiles(h)
                    qrt, qc = qrts[h]
                    wq_dma(h + 2)
                    # Qproj(h+1) first: no dependency on head h's chains, so
                    # it covers the tail of head h-1's vector pipeline.
                    qt_n = qproj(h + 1) if h + 1 < NH else None
                    if h > 0:
                        attnv_half(h - 1, 1)
                    for g_ in a_set:
                        score_group(h, qrt, qc, *g_)
                    if qt_n is not None:
                        qrt_n = ph.tile([128, L], F32R, tag="qrt", bufs=2,
                                        name=f"qrt{h+1}")
                        qc_n = ph.tile([128, L], F32R, tag="qc", bufs=2,
                                       name=f"qc{h+1}")
                        qrts[h + 1] = (qrt_n, qc_n)
                        rope_q_half(h + 1, qt_n, qrt_n, qc_n, 0)
                    if h == NH - 1:
                        nc.sync.dma_start(wo_c0[:], wo[:, :, 0:512])
                    attnv0_at = 2 if h + 1 < NH else 4
                    for i, g_ in enumerate(b_set):
                        if i == attnv0_at:
                            attnv_half(h, 0)
                        if i == 2 and qt_n is not None:
                            rope_q_half(h + 1, qt_n, qrt_n, qc_n, 1)
                        score_group(h, qrt, qc, *g_)
                    qrts.pop(h)

                # first epilogue blocks (heads 0-6) cover head 7's E tail,
                # attnv(7,1), and the outtn[7] norm; their hh=7 matmuls land
                # in the epilogue. psy2/psy3 borrow the idle SA/SB rings.
                psy0 = p2("psy")
                for hh in range(NH - 1):
                    nc.tensor.matmul(psy0[:],
                                     outtn[hh][:, 0:128], wo_c0[:, hh, :],
                                     start=(hh == 0), stop=False)
                attnv_half(NH - 1, 1, use_sasb=True)
                psy23 = [sa("psy2"), sb("psy3")]
                for j, psyx in enumerate(psy23):
                    for hh in range(NH - 1):
                        nc.tensor.matmul(psyx[:],
                                         outtn[hh][:, (2 + j) * 128:(3 + j) * 128],
                                         wo_c0[:, hh, :],
                                         start=(hh == 0), stop=False)

            # ---------------- epilogue: Wo projection ----------------
            with tc.tile_pool(name="ep", bufs=1) as pe:
                def ytile_out(psy, dc, lb):
                    yt = pe.tile([128, 512], F32, tag="ytile", bufs=3)
                    nc.scalar.copy(yt[:], psy[:])
                    nc.sync.dma_start(
                        y[lb * 128:(lb + 1) * 128, dc * 512:(dc + 1) * 512],
                        yt[:])

                # finish the pre-opened blocks: lb=1's heads 0-6 cover the
                # outtn[7] norm latency, then close lb=0..3.
                psy1 = p2("psy")
                for hh in range(NH - 1):
                    nc.tensor.matmul(psy1[:],
                                     outtn[hh][:, 128:256], wo_c0[:, hh, :],
                                     start=(hh == 0), stop=False)
                for psyx, lb in [(psy0, 0), (psy1, 1), (psy23[0], 2),
                                 (psy23[1], 3)]:
                    nc.tensor.matmul(psyx[:], outtn[NH - 1][:, lb * 128:(lb + 1) * 128],
                                     wo_c0[:, NH - 1, :], start=False, stop=True)
                    ytile_out(psyx, 0, lb)

                wo_ts = {0: wo_c0}
                for dc in range(4):
                    if dc + 1 < 4:
                        wo_n = pe.tile([128, NH, 512], BF16, tag="wo_c", bufs=2,
                                       name=f"wo_c{dc+1}")
                        nc.sync.dma_start(
                            wo_n[:], wo[:, :, (dc + 1) * 512:(dc + 2) * 512])
                        wo_ts[dc + 1] = wo_n
                    wo_t = wo_ts.pop(dc)
                    for lb in range(4 if dc == 0 else 0, NB):
                        psy = (sa if lb % 2 == 0 else sb)("psy")
                        for hh in range(NH):
                            nc.tensor.matmul(
                                psy[:],
                                outtn[hh][:, lb * 128:(lb + 1) * 128],
                                wo_t[:, hh, :],
                                start=(hh == 0), stop=(hh == NH - 1))
                        if dc == 3 and lb == NB - 1:
                            for half in range(2):
                                yt = pe.tile([128, 256], F32, tag="ytl", bufs=2)
                                nc.scalar.copy(yt[:], psy[:, half * 256:(half + 1) * 256])
                                nc.sync.dma_start(
                                    y[lb * 128:(lb + 1) * 128,
                                      dc * 512 + half * 256:dc * 512 + (half + 1) * 256],
                                    yt[:])
                        else:
                            ytile_out(psy, dc, lb)

    nc.compile()
    return nc


def _host_prep(x, Wq, Wk, Wv, Wo, q_param, log_scale, cos, sin, mask):
    """Build the 8 per-core input maps."""
    x = np.asarray(x, np.float32)
    Wq = np.asarray(Wq, np.float32)
    Wk = np.asarray(Wk, np.float32)
    Wv = np.asarray(Wv, np.float32)
    Wo = np.asarray(Wo, np.float32)
    cos = np.asarray(cos, np.float32)[0]      # [L, H, 64]
    sin = np.asarray(sin, np.float32)[0]
    qp = np.asarray(q_param, np.float32).reshape(H)
    ls = np.asarray(log_scale, np.float32).reshape(H)
    mask = np.asarray(mask)

    p64 = np.arange(128) % 64
    # rope tables are identical across heads: use head 0
    cos_p = np.ascontiguousarray(cos[:, 0, :][:, p64].T)   # [128, L]
    sin_p = np.ascontiguousarray(sin[:, 0, :][:, p64].T)

    PM = np.zeros((128, 128), np.float32)
    for dp in range(128):
        base, r = (dp // 64) * 64, dp % 64
        if r < 32:
            PM[base + r + 32, dp] = -1.0
        else:
            PM[base + r - 32, dp] = 1.0
    SW = np.zeros((128, 128), np.float32)
    for dp in range(128):
        SW[(dp + 64) % 128, dp] = 1.0
    ONES = np.ones((128, 128), ml_dtypes.bfloat16)
    IDENT = np.eye(128, dtype=ml_dtypes.bfloat16)

    in_maps = []
    for core in range(8):
        b, g2 = core // 2, core % 2
        heads = list(range(g2 * NH, (g2 + 1) * NH))

        xhv = np.ascontiguousarray(x[b].T).astype(ml_dtypes.bfloat16)

        wq_c = Wq[:, g2 * NH * 128:(g2 + 1) * NH * 128]
        wk_c = Wk[:, g2 * NKV * 128:(g2 + 1) * NKV * 128]
        wv_c = Wv[:, g2 * NKV * 128:(g2 + 1) * NKV * 128]
        wo_c = Wo[g2 * NH * 128:(g2 + 1) * NH * 128, :]

        wq_p = wq_c.reshape(16, 128, NH, 128).transpose(2, 1, 0, 3).copy()
        wk_p = wk_c.reshape(16, 128, NKV * 128).transpose(1, 0, 2).copy()
        wv_p = wv_c.reshape(16, 128, NKV * 128).transpose(1, 0, 2).copy()
        wo_p = wo_c.reshape(NH, 128, D).transpose(1, 0, 2).astype(ml_dtypes.bfloat16)

        mb = np.where(mask[b].reshape(NB, 128).T.astype(bool), 0.0, -1e9)
        mb = mb.astype(np.float32)

        cpr = -2.0 * np.tanh(qp[heads])               # per-head c'
        sq = np.sqrt(np.abs(cpr))
        sqc = np.tile(sq[None, :], (128, 1))
        sqc[:64, :] *= np.sign(cpr)[None, :]
        alp = np.tile((np.exp(ls[heads]) / HD)[None, :], (128, 1))

        in_maps.append({
            "xh": xhv,
            "wq": wq_p.astype(ml_dtypes.bfloat16),
            "wk": wk_p.astype(ml_dtypes.bfloat16),
            "wv": wv_p.astype(ml_dtypes.bfloat16), "wo": wo_p,
            "costab": cos_p, "sintab": sin_p,
            "maskb": mb, "sqrtc": sqc.astype(np.float32),
            "alpha": alp.astype(np.float32),
            "pmrot": PM, "pmswap": SW, "onesb": ONES, "identb": IDENT,
        })
    return in_maps


def kernel(**inputs):
    if "nc" not in _CACHED:
        _CACHED["nc"] = build_program()
    nc = _CACHED["nc"]
    in_maps = _host_prep(**inputs)
    res = run_bass_kernel_spmd(nc, in_maps, list(range(8))).results
    out = np.empty((B, L, D), np.float32)
    for b in range(B):
        out[b] = res[2 * b]["y"] + res[2 * b + 1]["y"]
    return out
